# revision 1
# baseline (speedup 1.0000x reference)
"""Trainium2 Bass kernel for nn_KalmanFilter: EKF over T=512 steps, N=8192 chains.

Mathematical reduction (verified exact vs the reference):
  With C = [[0,0,0,1,0],[0,0,0,0,1]], rows 3,4 of the Jacobian A are zero, so
  columns 3,4 of Sigma_pred are exactly e3,e4 and S = I + R depends only on the
  per-step measurement parameters. The covariance never influences the output.
  The computation collapses to, per chain:
    S = I + L L^T,  L = [[e^l0, 0], [l1, e^l2]]
    u_{t+1} = (I - S^-1) u_t + S^-1 z_t          (u = [v, omega])
    th_{t+1} = th_t + omega_t * dt_t
    x_{t+1}  = x_t + v_t * dt_t * cos(th_t)
    y_{t+1}  = y_t + v_t * dt_t * sin(th_t)
    y_hat[t] = [x_{t+1}, y_{t+1}, th_{t+1}]
  The coupled 2-state linear recurrence is solved with Gauss-Seidel sweeps of
  hardware affine scans (tensor_tensor_scan); error contracts ~10x per sweep,
  SWEEPS=4 gives ~1e-4 absolute output error (fp32 floor is ~1.4e-5).

Sharding: data-parallel over chains, 1024 chains per NeuronCore across 8 cores.
"""
import sys
sys.path.insert(0, '/opt/trn_rl_repo')
import numpy as np
import concourse.bass as bass
from concourse import mybir
from concourse.bass_utils import run_bass_kernel_spmd

F32 = mybir.dt.float32
AF = mybir.ActivationFunctionType
A = mybir.AluOpType

N_CORES = 8
T = 512
N_TOT = 8192
NPC = N_TOT // N_CORES          # 1024 chains per core
P = 128                         # partitions
NSL = 4                         # slabs per core
CH = NPC // (NSL * P)           # chains per partition per slab = 2
SWEEPS = 4
MAGIC = float(1.5 * 2 ** 23)    # fp32 round-to-nearest trick
TWO_PI = float(2 * np.pi)
HALF_PI = float(np.pi / 2)


class _Sched:
    """Two-phase scheduler: record ops (engine, emit closure, deps), then emit
    per-engine in-order streams. Cross-engine deps become standalone wait_ge
    instructions (this walrus supports only one wait condition per inst)."""

    def __init__(self):
        self.ops = []
        self.count = {"v": 0, "g": 0, "a": 0, "s": 0}

    def __init_slots(self):
        pass

    def add(self, eng, emit_fn, deps=(), slot=None):
        self.count[eng] += 1
        ref = (eng, self.count[eng])
        if eng == "s":
            if not hasattr(self, "slot_count"):
                self.slot_count = {}
            self.slot_count[slot] = self.slot_count.get(slot, 0) + 1
            ref = ("D", slot, self.slot_count[slot])
        self.ops.append((eng, emit_fn, tuple(d for d in deps if d), ref))
        return ref

    def emit(self, eng, raw_eng, sems, dma_sems):
        last = {}
        dlast = {}
        for op_eng, emit_fn, deps, ref in self.ops:
            if op_eng != eng:
                continue
            for dep in deps:
                if dep[0] == "D":
                    _, slot, k = dep
                    if dlast.get(slot, 0) >= k:
                        continue
                    raw_eng.wait_ge(dma_sems[slot], 16 * k)
                    dlast[slot] = k
                else:
                    deng, dpos = dep
                    if deng == eng or last.get(deng, 0) >= dpos:
                        continue
                    raw_eng.wait_ge(sems[deng], dpos)
                    last[deng] = dpos
            emit_fn().then_inc(sems[eng], 1)


def _build_nc(reps=1):
    nc = bass.Bass()
    IN = nc.dram_tensor("inp", [NSL, 6, P, CH, T], F32, kind="ExternalInput")
    MU = nc.dram_tensor("mu", [NSL, 5, P, CH], F32, kind="ExternalInput")
    OUT = nc.dram_tensor("out", [NSL, 3, P, CH, T], F32, kind="ExternalOutput")

    _names = [0]

    def tile(shape):
        _names[0] += 1
        return nc.alloc_sbuf_tensor(f"tl{_names[0]}", list(shape), F32).ap()

    if True:
        consts = tile([P, T + 4])
        ones = consts[:, 0:T]
        halfpi = consts[:, T:T + 1]
        zin = [tile([P, 6, CH, T]) for _ in range(2)]
        mu_all = tile([P, 2, 5, CH])
        mu = [mu_all[:, 0], mu_all[:, 1]]
        e0sq = tile([P, CH, T]); e2sq = tile([P, CH, T])
        e0 = tile([P, CH, T]);   l1sq = tile([P, CH, T])
        t1 = tile([P, CH, T])
        t2 = tile([P, CH, T]);   d1 = tile([P, CH, T])
        det = tile([P, CH, T])
        lnd = d1                              # alias: d1 dead before Ln
        r = tile([P, CH, T])
        s01 = e0                              # alias: written in place
        m01 = tile([P, CH, T])
        s11 = tile([P, CH, T]);  u0 = tile([P, CH, T]);  u1 = tile([P, CH, T])
        m00 = tile([P, CH, T]);  m11 = tile([P, CH, T])
        p0 = tile([P, CH, T]);   q0 = tile([P, CH, T]);  b0 = tile([P, CH, T])
        p1 = e2sq                             # alias: e2sq dead after s11
        q1 = tile([P, CH, T]);   b1 = tile([P, CH, T])
        dt = tile([P, CH, T])
        big = tile([P, 12, CH, T + 1])
        v = big[:, 0]; w = big[:, 1]; th = big[:, 2]; thr = big[:, 3]
        sinf = big[:, 4]; cosf = big[:, 5]
        x = [big[:, 6], big[:, 7]]
        y = [big[:, 8], big[:, 9]]
        tho = [big[:, 10], big[:, 11]]
        kf = thr                              # in-place range reduction
        cv = tile([P, CH, T]);  cw = tile([P, CH, T])
        vdt = tile([P, CH, T])
        gx = tile([P, CH, T])
        gy = tile([P, CH, T])

        sch = _Sched()
        # `pv` holds previous slab's refs for write-after-read protection.
        pv = {}
        out_done = {}   # (rep, s) -> ("d", thr) after that slab's out-DMAs
        z_done = {}     # (rep, s) -> last reader of zin/mu buffer of slab s

        c_ones = sch.add("v", lambda: nc.vector.memset(ones, 1.0))
        c_hpi = sch.add("v", lambda: nc.vector.memset(halfpi, HALF_PI))

        S = {}          # global slab idx -> dict of refs

        def stage_dma_in(G):
            g = G % (NSL)
            s = g
            bi = s % 2
            z = zin[bi]; m_ = mu[bi]
            prev = S.get(G - 2, {})
            d_in = sch.add("s", lambda z=z, s=s: nc.sync.dma_start(
                z[:], IN[s].rearrange("k p c t -> p k c t")),
                deps=(prev.get("ysc"),), slot=s * 5 + 0)
            d_mu = sch.add("s", lambda m_=m_, s=s: nc.sync.dma_start(
                m_[:], MU[s].rearrange("k p c -> p k c")),
                deps=(prev.get("ysc"),), slot=s * 5 + 1)
            S.setdefault(G, {}).update(din=d_in, dmu=d_mu)

        def stage_act_leaf(G):
            s = G % NSL; bi = s % 2
            z = zin[bi]
            l0 = z[:, 2]; l1 = z[:, 3]; l2 = z[:, 4]
            C = S[G]; P1 = S.get(G - 1, {})
            C["a1"] = sch.add("a", lambda l0=l0: nc.scalar.activation(
                e0sq[:], l0, AF.Exp, scale=2.0), deps=(C["din"], P1.get("v1b")))
            C["a2"] = sch.add("a", lambda l2=l2: nc.scalar.activation(
                e2sq[:], l2, AF.Exp, scale=2.0), deps=(P1.get("v9"),))
            C["a3"] = sch.add("a", lambda l0=l0: nc.scalar.activation(
                e0[:], l0, AF.Exp), deps=(P1.get("v3"),))
            C["a4"] = sch.add("a", lambda l1=l1: nc.scalar.activation(
                l1sq[:], l1, AF.Square), deps=(P1.get("v2"), P1.get("g3")))

        def stage_leaffront(G):
            C = S[G]; P1 = S.get(G - 1, {})
            C["v1"] = sch.add("v", lambda: nc.vector.tensor_scalar(
                t2[:], e2sq[:], 1.0, None, op0=A.add), deps=(C["a2"], P1.get("g3")))
            C["v1b"] = sch.add("v", lambda: nc.vector.tensor_scalar(
                t1[:], e0sq[:], 1.0, None, op0=A.add), deps=(C["a1"], P1.get("g4")))
            C["g1"] = sch.add("g", lambda: nc.gpsimd.tensor_tensor(
                d1[:], t1[:], t2[:], A.mult), deps=(C["v1b"], C["v1"], P1.get("v2")))
            C["v2"] = sch.add("v", lambda: nc.vector.tensor_tensor(
                det[:], d1[:], l1sq[:], A.add), deps=(C["g1"], C["a4"], P1.get("a5")))
            C["a5"] = sch.add("a", lambda: nc.scalar.activation(
                lnd[:], det[:], AF.Ln), deps=(C["v2"],))
            C["a6"] = sch.add("a", lambda: nc.scalar.activation(
                r[:], lnd[:], AF.Exp, scale=-1.0),
                deps=(C["a5"], P1.get("g4"), P1.get("v4")))

        def stage_act_trig(G):
            if G < 0:
                return
            C = S[G]; P1 = S.get(G - 1, {})
            C["asin"] = sch.add("a", lambda: nc.scalar.activation(
                sinf[:], thr[:], AF.Sin), deps=(C["k3"], P1.get("ggy")))
            C["acos"] = sch.add("a", lambda: nc.scalar.activation(
                cosf[:], thr[:], AF.Sin, scale=0.5),
                deps=(C["k3"], P1.get("ggx"), P1.get("vcos")))
            C["acos2"] = sch.add("a", lambda: nc.scalar.activation(
                cosf[:], cosf[:], AF.Square))

        def stage_leafback(G):
            s = G % NSL; bi = s % 2
            z = zin[bi]
            z0 = z[:, 0]; z1 = z[:, 1]; l1 = z[:, 3]
            times = z[:, 5]
            C = S[G]; P1 = S.get(G - 1, {})
            C["g2"] = sch.add("g", lambda: nc.gpsimd.tensor_tensor(
                s01[:], e0[:], l1[:], A.mult), deps=(C["a3"], C["din"], P1.get("v3")))
            C["v3"] = sch.add("v", lambda: nc.vector.tensor_tensor(
                m01[:], s01[:], r[:], A.mult), deps=(C["g2"], C["a6"], P1.get("g7")))
            C["g3"] = sch.add("g", lambda: nc.gpsimd.tensor_tensor(
                s11[:], t2[:], l1sq[:], A.add), deps=(C["a4"], C["v1"], P1.get("v4")))
            C["v4"] = sch.add("v", lambda: nc.vector.tensor_tensor(
                u0[:], s11[:], r[:], A.mult), deps=(C["g3"], C["a6"], P1.get("g5")))
            C["g4"] = sch.add("g", lambda: nc.gpsimd.tensor_tensor(
                u1[:], t1[:], r[:], A.mult), deps=(C["v1b"], C["a6"], P1.get("v8")))
            C["v5"] = sch.add("v", lambda: nc.vector.tensor_scalar(
                m00[:], u0[:], -1.0, 1.0, op0=A.mult, op1=A.add), deps=(C["v4"],))
            C["v6"] = sch.add("v", lambda: nc.vector.tensor_scalar(
                m11[:], u1[:], -1.0, 1.0, op0=A.mult, op1=A.add), deps=(C["g4"],))
            C["g5"] = sch.add("g", lambda: nc.gpsimd.tensor_tensor(
                p0[:], u0[:], z0[:], A.mult), deps=(C["v4"],))
            C["v7"] = sch.add("v", lambda: nc.vector.tensor_tensor(
                q0[:], m01[:], z1[:], A.mult), deps=(C["v3"], P1.get("g6")))
            C["g6"] = sch.add("g", lambda: nc.gpsimd.tensor_tensor(
                b0[:], p0[:], q0[:], A.subtract),
                deps=(C["g5"], C["v7"], P1.get("addv_last")))
            C["v8"] = sch.add("v", lambda: nc.vector.tensor_tensor(
                p1[:], u1[:], z1[:], A.mult), deps=(C["g4"],))
            C["g7"] = sch.add("g", lambda: nc.gpsimd.tensor_tensor(
                q1[:], m01[:], z0[:], A.mult), deps=(C["v3"],))
            C["v9"] = sch.add("v", lambda: nc.vector.tensor_tensor(
                b1[:], p1[:], q1[:], A.subtract), deps=(C["v8"], C["g7"]))

        def stage_down2(G):
            if G < 0:
                return
            s = G % NSL; bi = s % 2
            m_ = mu[bi]; xo = x[bi]; yo = y[bi]; tho_ = tho[bi]
            C = S[G]; P1 = S.get(G - 1, {}); P2 = S.get(G - 2, {})
            C["vcos"] = sch.add("v", lambda: nc.vector.tensor_scalar(
                cosf[:], cosf[:], -2.0, 1.0, op0=A.mult, op1=A.add),
                deps=(C["acos2"],))
            C["gvdt"] = sch.add("g", lambda: nc.gpsimd.tensor_tensor(
                vdt[:], v[:, :, 0:T], dt[:], A.mult),
                deps=(C["lastv"], C["g9"], C["i1"], P1.get("ggx")))
            C["ggx"] = sch.add("v", lambda: nc.vector.tensor_tensor(
                gx[:], vdt[:], cosf[:, :, 0:T], A.mult),
                deps=(C["gvdt"], C["vcos"]))
            C["ggy"] = sch.add("g", lambda: nc.gpsimd.tensor_tensor(
                gy[:], vdt[:], sinf[:, :, 0:T], A.mult),
                deps=(C["gvdt"], C["asin"], C["thsc"]))
            xs = []
            for c in range(CH):
                xs.append(sch.add("v", lambda c=c, xo=xo, m_=m_: nc.vector.tensor_tensor_scan(
                    xo[:, c, 1:T + 1], ones, gx[:, c], m_[:, 0, c:c + 1],
                    A.mult, A.add), deps=(C["ggx"], C["i4"], c_ones)))
            ys = []
            for c in range(CH):
                ys.append(sch.add("v", lambda c=c, yo=yo, m_=m_: nc.vector.tensor_tensor_scan(
                    yo[:, c, 1:T + 1], ones, gy[:, c], m_[:, 1, c:c + 1],
                    A.mult, A.add), deps=(C["ggy"], C["i5"], c_ones)))
            C["xsc"] = xs[-1]; C["ysc"] = ys[-1]
            C["thcopy"] = sch.add("g", lambda tho_=tho_: nc.gpsimd.tensor_copy(
                tho_[:], th[:]),
                deps=(C["thsc"], C["i3"], S.get(G - 2, {}).get("ot")))
            s5 = (G % NSL) * 5
            C["ox"] = sch.add("s", lambda xo=xo, s=s: nc.sync.dma_start(
                OUT[s, 0], xo[:, :, 1:T + 1]), deps=(C["xsc"],), slot=s5 + 2)
            C["oy"] = sch.add("s", lambda yo=yo, s=s: nc.sync.dma_start(
                OUT[s, 1], yo[:, :, 1:T + 1]), deps=(C["ysc"],), slot=s5 + 3)
            C["ot"] = sch.add("s", lambda tho_=tho_, s=s: nc.sync.dma_start(
                OUT[s, 2], tho_[:, :, 1:T + 1]), deps=(C["thcopy"],), slot=s5 + 4)

        def stage_inits(G):
            s = G % NSL; bi = s % 2
            m_ = mu[bi]; xo = x[bi]; yo = y[bi]
            C = S[G]; P1 = S.get(G - 1, {}); P2 = S.get(G - 2, {})
            C["i1"] = sch.add("v", lambda m_=m_: nc.vector.tensor_copy(
                v[:, :, 0], m_[:, 3]), deps=(C["din"], C["dmu"], P1.get("gvdt")))
            C["i2"] = sch.add("v", lambda m_=m_: nc.vector.tensor_copy(
                w[:, :, 0], m_[:, 4]), deps=(P1.get("gth"),))
            C["i3"] = sch.add("v", lambda m_=m_: nc.vector.tensor_copy(
                th[:, :, 0], m_[:, 2]), deps=(P1.get("thcopy"),))
            C["i4"] = sch.add("v", lambda m_=m_, xo=xo: nc.vector.tensor_copy(
                xo[:, :, 0], m_[:, 0]), deps=(P2.get("ox"),))
            C["i5"] = sch.add("v", lambda m_=m_, yo=yo: nc.vector.tensor_copy(
                yo[:, :, 0], m_[:, 1]), deps=(P2.get("oy"),))

        def stage_sweeps(G):
            s = G % NSL; bi = s % 2
            m_ = mu[bi]
            C = S[G]; P1 = S.get(G - 1, {})
            last_v = None; last_w = None; addv = None
            for k in range(SWEEPS):
                if k == 0:
                    dv = b0; dep_in = (C["g6"],)
                else:
                    mulv = sch.add("v", lambda: nc.vector.tensor_tensor(
                        cv[:], m01[:], w[:, :, 0:T], A.mult),
                        deps=(last_w, C["v3"], C["i2"]))
                    addv = sch.add("v", lambda: nc.vector.tensor_tensor(
                        cv[:], cv[:], b0[:], A.add), deps=(mulv, C["g6"]))
                    dv = cv; dep_in = (addv,)
                vs = []
                for c in range(CH):
                    vs.append(sch.add("v", lambda c=c, dv=dv, m_=m_: nc.vector.tensor_tensor_scan(
                        v[:, c, 1:T + 1], m00[:, c], dv[:, c], m_[:, 3, c:c + 1],
                        A.mult, A.add), deps=dep_in + (C["v5"], C["i1"], P1.get("gvdt"))))
                last_v = vs[-1]
                mulw = sch.add("v", lambda: nc.vector.tensor_tensor(
                    cw[:], m01[:], v[:, :, 0:T], A.mult), deps=(last_v, C["v3"]))
                addw = sch.add("v", lambda: nc.vector.tensor_tensor(
                    cw[:], cw[:], b1[:], A.add), deps=(mulw, C["v9"]))
                ws = []
                for c in range(CH):
                    ws.append(sch.add("v", lambda c=c, m_=m_: nc.vector.tensor_tensor_scan(
                        w[:, c, 1:T + 1], m11[:, c], cw[:, c], m_[:, 4, c:c + 1],
                        A.mult, A.add), deps=(addw, C["v6"], C["i2"], P1.get("gth"))))
                last_w = ws[-1]
            C["lastv"] = last_v; C["lastw"] = last_w; C["addv_last"] = addv

        def stage_down1(G):
            s = G % NSL; bi = s % 2
            z = zin[bi]; m_ = mu[bi]
            times = z[:, 5]
            C = S[G]; P1 = S.get(G - 1, {})
            C["g8"] = sch.add("g", lambda times=times: nc.gpsimd.tensor_tensor(
                dt[:, :, 1:T], times[:, :, 1:T], times[:, :, 0:T - 1], A.subtract),
                deps=(C["din"], P1.get("gvdt")))
            C["g9"] = sch.add("g", lambda: nc.gpsimd.memset(dt[:, :, 0], 0.0))
            C["gth"] = sch.add("g", lambda: nc.gpsimd.tensor_tensor(
                gy[:], w[:, :, 0:T], dt[:], A.mult),
                deps=(C["lastw"], C["g9"], C["i2"], P1.get("ysc")))
            ths = []
            for c in range(CH):
                ths.append(sch.add("v", lambda c=c, m_=m_: nc.vector.tensor_tensor_scan(
                    th[:, c, 1:T + 1], ones, gy[:, c], m_[:, 2, c:c + 1],
                    A.mult, A.add), deps=(C["gth"], c_ones, C["i3"])))
            C["thsc"] = ths[-1]
            k1 = sch.add("v", lambda: nc.vector.tensor_scalar(
                kf[:], th[:], 1.0 / TWO_PI, MAGIC, op0=A.mult, op1=A.add),
                deps=(C["thsc"], P1.get("acos2")))
            k2 = sch.add("v", lambda: nc.vector.tensor_scalar(
                kf[:], kf[:], MAGIC, None, op0=A.subtract), deps=(k1,))
            C["k3"] = sch.add("v", lambda: nc.vector.scalar_tensor_tensor(
                thr[:], kf[:], -TWO_PI, th[:], A.mult, A.add), deps=(k2,))

        NG = reps * NSL
        for G in range(NG):
            stage_dma_in(G)
            stage_act_leaf(G)
            stage_leaffront(G)
            stage_act_trig(G - 1)
            stage_leafback(G)
            stage_down2(G - 1)
            stage_inits(G)
            stage_sweeps(G)
            stage_down1(G)
        stage_act_trig(NG - 1)
        stage_down2(NG - 1)

        n_slots = NSL * 5
        sem_v = nc.alloc_semaphore()
        sem_g = nc.alloc_semaphore()
        sem_a = nc.alloc_semaphore()
        dma_sems = [nc.alloc_semaphore(f"dsem{i}") for i in range(n_slots)]
        with nc.Block() as block:
            sems = {"v": sem_v, "g": sem_g, "a": sem_a}

            @block.sync
            def _(sync):
                last = {}
                dlast = {}
                for op_eng, emit_fn, deps, ref in sch.ops:
                    if op_eng != "s":
                        continue
                    for dep in deps:
                        if dep[0] == "D":
                            _, slot, k = dep
                            if dlast.get(slot, 0) >= k:
                                continue
                            sync.wait_ge(dma_sems[slot], 16 * k)
                            dlast[slot] = k
                        else:
                            deng, dpos = dep
                            if deng == "s" or last.get(deng, 0) >= dpos:
                                continue
                            sync.wait_ge(sems[deng], dpos)
                            last[deng] = dpos
                    emit_fn().then_inc(dma_sems[ref[1]], 16)

            @block.vector
            def _(vector):
                sch.emit("v", vector, sems, dma_sems)

            @block.gpsimd
            def _(gp):
                sch.emit("g", gp, sems, dma_sems)

            @block.scalar
            def _(scalar):
                sch.emit("a", scalar, sems, dma_sems)

    return nc


_cache = {}


def _get_nc(reps=1):
    if reps not in _cache:
        _cache[reps] = _build_nc(reps)
    return _cache[reps]


def _pack_core(z_core, mu_core, times_core):
    arr = np.concatenate([
        np.ascontiguousarray(z_core.transpose(2, 1, 0)),      # (5, NPC, T)
        np.ascontiguousarray(times_core.T)[None],             # (1, NPC, T)
    ], axis=0)
    IN = np.ascontiguousarray(
        arr.reshape(6, NSL, P, CH, T).transpose(1, 0, 2, 3, 4))
    MU = np.ascontiguousarray(
        mu_core.T.reshape(5, NSL, P, CH).transpose(1, 0, 2, 3))
    return {"inp": IN, "mu": MU}


def kernel(z_and_L_hat, mu0, times):
    z_and_L_hat = np.asarray(z_and_L_hat, dtype=np.float32)
    mu0 = np.asarray(mu0, dtype=np.float32)
    times = np.asarray(times, dtype=np.float32)
    nc = _get_nc()
    in_maps = []
    for k in range(N_CORES):
        sl = slice(k * NPC, (k + 1) * NPC)
        in_maps.append(_pack_core(z_and_L_hat[:, sl, :], mu0[sl], times[:, sl]))
    res = run_bass_kernel_spmd(nc, in_maps, core_ids=list(range(N_CORES)))
    out = np.empty((T, N_TOT, 3), np.float32)
    for k in range(N_CORES):
        O = res.results[k]["out"]                 # (NSL, 3, P, CH, T)
        planes = O.transpose(1, 0, 2, 3, 4).reshape(3, NPC, T)
        sl = slice(k * NPC, (k + 1) * NPC)
        out[:, sl, 0] = planes[0].T
        out[:, sl, 1] = planes[1].T
        out[:, sl, 2] = planes[2].T
    return out



# revision 4
# speedup vs baseline: 1.5614x; 1.5614x over previous
"""Trainium2 Bass kernel for nn_KalmanFilter: EKF over T=512 steps, N=8192 chains.

Mathematical reduction (verified exact vs the reference):
  With C = [[0,0,0,1,0],[0,0,0,0,1]], rows 3,4 of the Jacobian A are zero, so
  S = I + R depends only on per-step measurement params and the covariance
  never influences the output. Per chain:
    S = I + L L^T,  L = [[e^l0, 0], [l1, e^l2]]
    u_{t+1} = (I - S^-1) u_t + S^-1 z_t          (u = [v, omega])
    th_{t+1} = th_t + omega_t * dt_t
    x_{t+1}  = x_t + v_t * dt_t * cos(th_t)
    y_{t+1}  = y_t + v_t * dt_t * sin(th_t)
  The coupled 2-state linear recurrence is solved with 2 Gauss-Seidel sweeps
  of hardware affine scans (error contracts ~10x/sweep; end-to-end rel err
  ~2.4e-3 incl. fp16 quantization, vs the 2e-2 gate).

Implementation notes (from microbenchmarks on this part):
  - GpSimd shares SBUF ports with DVE and degrades it ~4x: all elementwise
    work runs on DVE (fp16, 2x packed mode, ~0.7us/1024el) + Act engine
    (affine/function passes, ~1.1us each, own ports).
  - Scans are DVE-only, ~2.2ns/el, dtype-insensitive. Both chains per
    partition are covered by ONE scan via a zero-multiplier column at each
    chain start (also injects the init value).
  - All intermediates are fp16; t1, t2, s01, s11, det carry a 2^-7 (det
    2^-14) exponent scale folded into Act scale/bias to avoid fp16 overflow;
    r7 = 2^7/det compensates exactly.
Sharding: data-parallel over chains, 1024 chains per core across 8 cores.
"""
import sys
sys.path.insert(0, '/opt/trn_rl_repo')
import numpy as np
import concourse.bass as bass
from concourse import mybir
from concourse.bass_utils import run_bass_kernel_spmd

F32 = mybir.dt.float32
F16 = mybir.dt.float16
AF = mybir.ActivationFunctionType
A = mybir.AluOpType

N_CORES = 8
T = 512
N_TOT = 8192
NPC = N_TOT // N_CORES          # 1024 chains per core
P = 128                         # partitions
NSL = 4                         # slabs per core
CH = NPC // (NSL * P)           # chains per partition per slab = 2
TP = T + 2                      # padded per-chain row: [init | T data | pad]
MAGIC = float(1.5 * 2 ** 23)
TWO_PI = float(2 * np.pi)
INV_2PI = float(1.0 / (2 * np.pi))
LN2x7 = float(7 * np.log(2.0))
HS = float(2.0 ** -3.5)         # Square scale for l1^2 * 2^-7
EPS7 = float(2.0 ** -7)


class _Sched:
    """Two-phase scheduler: record ops (engine, emit closure, deps), then emit
    per-engine in-order streams with standalone wait_ge for cross-engine deps."""

    def __init__(self):
        self.ops = []
        self.count = {"v": 0, "g": 0, "a": 0}
        self.slot_count = {}

    def add(self, eng, emit_fn, deps=(), slot=None):
        if eng == "s":
            self.slot_count[slot] = self.slot_count.get(slot, 0) + 1
            ref = ("D", slot, self.slot_count[slot])
        else:
            self.count[eng] += 1
            ref = (eng, self.count[eng])
        self.ops.append((eng, emit_fn, tuple(d for d in deps if d), ref))
        return ref

    def emit(self, eng, raw_eng, sems, dma_sems):
        last = {}
        dlast = {}
        for op_eng, emit_fn, deps, ref in self.ops:
            if op_eng != eng:
                continue
            for dep in deps:
                if dep[0] == "D":
                    _, slot, k = dep
                    if dlast.get(slot, 0) >= k:
                        continue
                    raw_eng.wait_ge(dma_sems[slot], 16 * k)
                    dlast[slot] = k
                else:
                    deng, dpos = dep
                    if deng == eng or last.get(deng, 0) >= dpos:
                        continue
                    raw_eng.wait_ge(sems[deng], dpos)
                    last[deng] = dpos
            emit_fn().then_inc(sems[eng], 1)


class _Graph:
    """Auto RAW/WAR/WAW dependency tracking over named tiles."""

    def __init__(self, sch):
        self.sch = sch
        self.w = {}
        self.r = {}

    def op(self, eng, emit_fn, reads=(), writes=(), slot=None, extra=()):
        deps = {}

        def add(ref):
            if ref is None:
                return
            key = ref[0] if ref[0] != "D" else ("D", ref[1])
            cur = deps.get(key)
            if cur is None or ref[-1] > cur[-1]:
                deps[key] = ref

        for t in reads:
            add(self.w.get(t))
        for t in writes:
            add(self.w.get(t))
            for rr in self.r.get(t, ()):
                add(rr)
        for e in extra:
            add(e)
        ref = self.sch.add(eng, emit_fn, deps=tuple(deps.values()), slot=slot)
        for t in reads:
            self.r.setdefault(t, []).append(ref)
        for t in writes:
            self.w[t] = ref
            self.r[t] = []
        return ref


def _build_nc():
    nc = bass.Bass()
    # Register activation bias constants (bass converts float biases of
    # non-Copy activations to const APs, which must pre-exist).
    for val in (-LN2x7, EPS7, MAGIC, -MAGIC):
        t = nc.alloc_sbuf_tensor(f"constf32-{val}", [128, 1], F32)
        nc.gpsimd.memset(t.ap(), val)
        nc.const_aps.aps[(F32, val)] = t.ap()
    nc.all_engine_barrier()
    IN = nc.dram_tensor("inp", [NSL, 6, P, CH, T], F16, kind="ExternalInput")
    MU = nc.dram_tensor("mu", [NSL, P, 5, CH], F32, kind="ExternalInput")
    OUT = nc.dram_tensor("out", [NSL, 3, P, CH, T], F32, kind="ExternalOutput")

    _names = [0]

    def tile(shape, dt=F16):
        _names[0] += 1
        return nc.alloc_sbuf_tensor(f"tl{_names[0]}", list(shape), dt).ap()

    def flat(ap):
        return ap.rearrange('p a b -> p (a b)')

    # constants
    ONB = tile([P, 4, TP], F32)          # ones, 0 at col0 of each chain row

    # per-parity tiles
    zin = [tile([P, 6, CH, T]) for _ in range(2)]
    mu = [tile([P, 5, CH], F32) for _ in range(2)]
    M0x = [tile([P, CH, TP]) for _ in range(2)]
    M1x = [tile([P, CH, TP]) for _ in range(2)]
    B0x = [tile([P, CH, TP]) for _ in range(2)]
    CWx = [tile([P, CH, TP]) for _ in range(2)]
    CVx = [tile([P, CH, TP]) for _ in range(2)]
    Gx = [tile([P, CH, TP]) for _ in range(2)]
    GXY = [tile([P, 4, TP]) for _ in range(2)]
    Vt = [tile([P, CH, TP]) for _ in range(2)]
    Wt = [tile([P, CH, TP]) for _ in range(2)]
    TH = [tile([P, CH, TP], F32) for _ in range(2)]
    XY = [tile([P, 4, TP], F32) for _ in range(2)]
    K1 = [tile([P, CH, T], F32) for _ in range(2)]
    # aliased fp16 scratch [P, CH, T]; one list of phys tiles per parity
    SCR = [[tile([P, CH, T]) for _ in range(11)] for _ in range(2)]

    sch = _Sched()
    g = _Graph(sch)

    # ---- preamble ----
    g.op("v", lambda: nc.vector.memset(flat(ONB), 1.0), writes=("ONB",))
    for c in range(4):
        g.op("v", lambda c=c: nc.vector.memset(ONB[:, c, 0:1], 0.0),
             writes=("ONB",))
    for bi in range(2):
        for nm, tl in (("M0x", M0x), ("M1x", M1x), ("B0x", B0x), ("CWx", CWx),
                       ("CVx", CVx), ("Gx", Gx)):
            g.op("v", lambda tl=tl, bi=bi: nc.vector.memset(tl[bi][:, :, TP - 1:TP], 0.0),
                 writes=(f"{nm}{bi}",))
        g.op("v", lambda bi=bi: nc.vector.memset(GXY[bi][:, :, TP - 1:TP], 0.0),
             writes=(f"GXY{bi}",))
        for nm, tl in (("M0x", M0x), ("M1x", M1x)):
            g.op("v", lambda tl=tl, bi=bi: nc.vector.memset(tl[bi][:, :, 0:1], 0.0),
                 writes=(f"{nm}{bi}",))

    def key(nm, G):
        return f"{nm}{G % 2}"

    # scratch allocation per logical name (phys index into SCR)
    PHYS = {"e0sq": 0, "u0": 0, "vdt": 0,
            "e2sq": 1, "u1": 1, "thr": 1,
            "e0h": 2, "m01": 2,
            "l1sq": 3, "p0": 3,
            "t1h": 4, "p1": 4,
            "t2h": 5, "q0": 5,
            "d1h": 6, "lnd": 6, "b1": 6,
            "deth": 7, "q1": 7,
            "s01h": 8, "sinf": 8,
            "s11h": 9, "cosf": 9,
            "r7": 10, "tmp": 10}

    def scr(nm, G):
        return SCR[G % 2][PHYS[nm]]

    def skey(nm, G):
        return f"S{PHYS[nm]}_{G % 2}"

    # ---------------- stages ----------------
    def dma_in(G):
        s = G % NSL
        bi = G % 2
        g.op("s", lambda: nc.sync.dma_start(
            zin[bi][:], IN[s].rearrange("k p c t -> p k c t")),
            reads=(), writes=(key("zin", G),), slot=s * 5 + 0)
        g.op("s", lambda: nc.sync.dma_start(mu[bi][:], MU[s]),
            reads=(), writes=(key("mu", G),), slot=s * 5 + 1)

    def act(out_ap, in_ap, func, bias=0.0, scale=1.0):
        return lambda: nc.scalar.activation(out_ap, in_ap, func,
                                            bias=bias, scale=scale)

    def leaf_acts(G):
        bi = G % 2
        z = zin[bi]
        l0 = z[:, 2]; l1 = z[:, 3]; l2 = z[:, 4]
        zk = key("zin", G)
        g.op("a", act(scr("e0sq", G)[:], l0, AF.Exp, bias=-LN2x7, scale=2.0),
             reads=(zk,), writes=(skey("e0sq", G),))
        g.op("a", act(scr("e2sq", G)[:], l2, AF.Exp, bias=-LN2x7, scale=2.0),
             reads=(zk,), writes=(skey("e2sq", G),))
        g.op("a", act(scr("e0h", G)[:], l0, AF.Exp, bias=-LN2x7),
             reads=(zk,), writes=(skey("e0h", G),))
        g.op("a", act(scr("l1sq", G)[:], l1, AF.Square, scale=HS),
             reads=(zk,), writes=(skey("l1sq", G),))
        g.op("a", act(scr("t1h", G)[:], scr("e0sq", G)[:], AF.Identity, bias=EPS7),
             reads=(skey("e0sq", G),), writes=(skey("t1h", G),))
        g.op("a", act(scr("t2h", G)[:], scr("e2sq", G)[:], AF.Identity, bias=EPS7),
             reads=(skey("e2sq", G),), writes=(skey("t2h", G),))

    def tt(out, in0, in1, op):
        return lambda: nc.vector.tensor_tensor(out, in0, in1, op)

    def alg_front(G):
        g.op("v", tt(scr("d1h", G)[:], scr("t1h", G)[:], scr("t2h", G)[:], A.mult),
             reads=(skey("t1h", G), skey("t2h", G)), writes=(skey("d1h", G),))
        g.op("v", tt(scr("s01h", G)[:], scr("e0h", G)[:], zin[G % 2][:, 3], A.mult),
             reads=(skey("e0h", G), key("zin", G)), writes=(skey("s01h", G),))
        g.op("v", tt(scr("s11h", G)[:], scr("t2h", G)[:], scr("l1sq", G)[:], A.add),
             reads=(skey("t2h", G), skey("l1sq", G)), writes=(skey("s11h", G),))
        g.op("v", lambda G=G: nc.vector.scalar_tensor_tensor(
            scr("deth", G)[:], scr("l1sq", G)[:], EPS7, scr("d1h", G)[:],
            A.mult, A.add),
            reads=(skey("l1sq", G), skey("d1h", G)), writes=(skey("deth", G),))

    def lnd_r7(G):
        g.op("a", act(scr("lnd", G)[:], scr("deth", G)[:], AF.Ln),
             reads=(skey("deth", G),), writes=(skey("lnd", G),))
        g.op("a", act(scr("r7", G)[:], scr("lnd", G)[:], AF.Exp,
                      bias=-LN2x7, scale=-1.0),
             reads=(skey("lnd", G),), writes=(skey("r7", G),))

    def alg_mid(G):
        z = zin[G % 2]
        z0 = z[:, 0]; z1 = z[:, 1]
        zk = key("zin", G)
        g.op("v", tt(scr("u0", G)[:], scr("s11h", G)[:], scr("r7", G)[:], A.mult),
             reads=(skey("s11h", G), skey("r7", G)), writes=(skey("u0", G),))
        g.op("v", tt(scr("u1", G)[:], scr("t1h", G)[:], scr("r7", G)[:], A.mult),
             reads=(skey("t1h", G), skey("r7", G)), writes=(skey("u1", G),))
        g.op("v", tt(scr("m01", G)[:], scr("s01h", G)[:], scr("r7", G)[:], A.mult),
             reads=(skey("s01h", G), skey("r7", G)), writes=(skey("m01", G),))
        g.op("v", tt(scr("p0", G)[:], scr("u0", G)[:], z0, A.mult),
             reads=(skey("u0", G), zk), writes=(skey("p0", G),))
        g.op("v", tt(scr("q0", G)[:], scr("m01", G)[:], z1, A.mult),
             reads=(skey("m01", G), zk), writes=(skey("q0", G),))
        g.op("v", tt(B0x[G % 2][:, :, 1:T + 1], scr("p0", G)[:], scr("q0", G)[:],
                     A.subtract),
             reads=(skey("p0", G), skey("q0", G)), writes=(key("B0x", G),))
        g.op("v", tt(scr("p1", G)[:], scr("u1", G)[:], z1, A.mult),
             reads=(skey("u1", G), zk), writes=(skey("p1", G),))
        g.op("v", tt(scr("q1", G)[:], scr("m01", G)[:], z0, A.mult),
             reads=(skey("m01", G), zk), writes=(skey("q1", G),))
        g.op("v", tt(scr("b1", G)[:], scr("p1", G)[:], scr("q1", G)[:], A.subtract),
             reads=(skey("p1", G), skey("q1", G)), writes=(skey("b1", G),))

    def m_acts(G):
        bi = G % 2
        g.op("a", act(M0x[bi][:, :, 1:T + 1], scr("u0", G)[:], AF.Identity,
                      bias=1.0, scale=-1.0),
             reads=(skey("u0", G),), writes=(key("M0x", G),))
        g.op("a", act(M1x[bi][:, :, 1:T + 1], scr("u1", G)[:], AF.Identity,
                      bias=1.0, scale=-1.0),
             reads=(skey("u1", G),), writes=(key("M1x", G),))
        m = mu[bi]
        mk = key("mu", G)
        for dst_ap, dst_key, mi in (
                (B0x[bi][:, :, 0], "B0x", 3), (CVx[bi][:, :, 0], "CVx", 3),
                (CWx[bi][:, :, 0], "CWx", 4), (Gx[bi][:, :, 0], "Gx", 2),
                (GXY[bi][:, 0:2, 0], "GXY", 0), (GXY[bi][:, 2:4, 0], "GXY", 1)):
            g.op("a", act(dst_ap, m[:, mi], AF.Identity),
                 reads=(mk,), writes=(key(dst_key, G),))

    def scan(out, d0, d1):
        return lambda: nc.vector.tensor_tensor_scan(out, d0, d1, 0.0,
                                                    A.mult, A.add)

    def sweeps_a(G):
        bi = G % 2
        m01 = scr("m01", G)
        tmp = scr("tmp", G)
        b1 = scr("b1", G)
        g.op("v", scan(flat(Vt[bi]), flat(M0x[bi]), flat(B0x[bi])),
             reads=(key("M0x", G), key("B0x", G)), writes=(key("V", G),))
        g.op("v", tt(tmp[:], m01[:], Vt[bi][:, :, 0:T], A.mult),
             reads=(skey("m01", G), key("V", G)), writes=(skey("tmp", G),))
        g.op("v", tt(CWx[bi][:, :, 1:T + 1], tmp[:], b1[:], A.add),
             reads=(skey("tmp", G), skey("b1", G)), writes=(key("CWx", G),))
        g.op("v", scan(flat(Wt[bi]), flat(M1x[bi]), flat(CWx[bi])),
             reads=(key("M1x", G), key("CWx", G)), writes=(key("W", G),))

    def sweeps_b(G):
        bi = G % 2
        m01 = scr("m01", G)
        tmp = scr("tmp", G)
        b1 = scr("b1", G)
        g.op("v", tt(tmp[:], m01[:], Wt[bi][:, :, 0:T], A.mult),
             reads=(skey("m01", G), key("W", G)), writes=(skey("tmp", G),))
        g.op("v", tt(CVx[bi][:, :, 1:T + 1], tmp[:], B0x[bi][:, :, 1:T + 1], A.add),
             reads=(skey("tmp", G), key("B0x", G)), writes=(key("CVx", G),))
        g.op("v", scan(flat(Vt[bi]), flat(M0x[bi]), flat(CVx[bi])),
             reads=(key("M0x", G), key("CVx", G)), writes=(key("V", G),))
        g.op("v", tt(tmp[:], m01[:], Vt[bi][:, :, 0:T], A.mult),
             reads=(skey("m01", G), key("V", G)), writes=(skey("tmp", G),))
        g.op("v", tt(CWx[bi][:, :, 1:T + 1], tmp[:], b1[:], A.add),
             reads=(skey("tmp", G), skey("b1", G)), writes=(key("CWx", G),))
        g.op("v", scan(flat(Wt[bi]), flat(M1x[bi]), flat(CWx[bi])),
             reads=(key("M1x", G), key("CWx", G)), writes=(key("W", G),))
        g.op("v", tt(Gx[bi][:, :, 1:T + 1], Wt[bi][:, :, 0:T], zin[bi][:, 5],
                     A.mult),
             reads=(key("W", G), key("zin", G)), writes=(key("Gx", G),))
        g.op("v", scan(flat(TH[bi]), flat(ONB[:, 0:2]), flat(Gx[bi])),
             reads=("ONB", key("Gx", G)), writes=(key("TH", G),))

    def k12(G):
        bi = G % 2
        g.op("a", act(K1[bi][:], TH[bi][:, :, 0:T], AF.Identity,
                      bias=MAGIC, scale=INV_2PI),
             reads=(key("TH", G),), writes=(key("K1", G),))
        g.op("a", act(K1[bi][:], K1[bi][:], AF.Identity, bias=-MAGIC),
             reads=(key("K1", G),), writes=(key("K1", G),))

    def k3(G):
        bi = G % 2
        g.op("v", lambda: nc.vector.scalar_tensor_tensor(
            scr("thr", G)[:], K1[bi][:], TWO_PI, TH[bi][:, :, 0:T],
            A.mult, A.subtract),
            reads=(key("K1", G), key("TH", G)), writes=(skey("thr", G),))

    def trig(G):
        thr = scr("thr", G)
        sinf = scr("sinf", G)
        cosf = scr("cosf", G)
        g.op("a", act(sinf[:], thr[:], AF.Sin, scale=-1.0),
             reads=(skey("thr", G),), writes=(skey("sinf", G),))
        g.op("a", act(cosf[:], thr[:], AF.Sin, scale=0.5),
             reads=(skey("thr", G),), writes=(skey("cosf", G),))
        g.op("a", act(cosf[:], cosf[:], AF.Square),
             reads=(skey("cosf", G),), writes=(skey("cosf", G),))
        g.op("a", act(cosf[:], cosf[:], AF.Identity, bias=1.0, scale=-2.0),
             reads=(skey("cosf", G),), writes=(skey("cosf", G),))

    def tail(G):
        bi = G % 2
        vdt = scr("vdt", G)
        g.op("v", tt(vdt[:], Vt[bi][:, :, 0:T], zin[bi][:, 5], A.mult),
             reads=(key("V", G), key("zin", G)), writes=(skey("vdt", G),))
        g.op("v", tt(GXY[bi][:, 0:2, 1:T + 1], vdt[:], scr("cosf", G)[:], A.mult),
             reads=(skey("vdt", G), skey("cosf", G)), writes=(key("GXY", G),))
        g.op("v", tt(GXY[bi][:, 2:4, 1:T + 1], vdt[:], scr("sinf", G)[:], A.mult),
             reads=(skey("vdt", G), skey("sinf", G)), writes=(key("GXY", G),))
        g.op("v", scan(flat(XY[bi]), flat(ONB), flat(GXY[bi])),
             reads=("ONB", key("GXY", G)), writes=(key("XY", G),))

    def dma_out(G):
        s = G % NSL
        bi = G % 2
        g.op("s", lambda: nc.sync.dma_start(OUT[s, 0], XY[bi][:, 0:2, 1:T + 1]),
             reads=(key("XY", G),), writes=(), slot=s * 5 + 2)
        g.op("s", lambda: nc.sync.dma_start(OUT[s, 1], XY[bi][:, 2:4, 1:T + 1]),
             reads=(key("XY", G),), writes=(), slot=s * 5 + 3)
        g.op("s", lambda: nc.sync.dma_start(OUT[s, 2], TH[bi][:, :, 1:T + 1]),
             reads=(key("TH", G),), writes=(), slot=s * 5 + 4)

    # ---------------- emission ----------------
    for G in range(NSL):
        dma_in(G)
        leaf_acts(G)
        alg_front(G)
        lnd_r7(G)
        if G > 0:
            sweeps_a(G - 1)
        alg_mid(G)
        m_acts(G)
        if G > 0:
            sweeps_b(G - 1)
            k12(G - 1)
            k3(G - 1)
            trig(G - 1)
            tail(G - 1)
            dma_out(G - 1)
    Gl = NSL - 1
    sweeps_a(Gl)
    sweeps_b(Gl)
    k12(Gl)
    k3(Gl)
    trig(Gl)
    tail(Gl)
    dma_out(Gl)

    n_slots = NSL * 5
    sem_v = nc.alloc_semaphore()
    sem_g = nc.alloc_semaphore()
    sem_a = nc.alloc_semaphore()
    dma_sems = [nc.alloc_semaphore(f"dsem{i}") for i in range(n_slots)]
    with nc.Block() as block:
        sems = {"v": sem_v, "g": sem_g, "a": sem_a}

        @block.sync
        def _(sync):
            last = {}
            dlast = {}
            for op_eng, emit_fn, deps, ref in sch.ops:
                if op_eng != "s":
                    continue
                for dep in deps:
                    if dep[0] == "D":
                        _, slot, k = dep
                        if dlast.get(slot, 0) >= k:
                            continue
                        sync.wait_ge(dma_sems[slot], 16 * k)
                        dlast[slot] = k
                    else:
                        deng, dpos = dep
                        if deng == "s" or last.get(deng, 0) >= dpos:
                            continue
                        sync.wait_ge(sems[deng], dpos)
                        last[deng] = dpos
                emit_fn().then_inc(dma_sems[ref[1]], 16)

        @block.vector
        def _(vector):
            sch.emit("v", vector, sems, dma_sems)

        @block.gpsimd
        def _(gp):
            sch.emit("g", gp, sems, dma_sems)

        @block.scalar
        def _(scalar):
            sch.emit("a", scalar, sems, dma_sems)

    return nc


_cache = {}


def _get_nc():
    if "nc" not in _cache:
        _cache["nc"] = _build_nc()
    return _cache["nc"]


def _pack_core(z_core, mu_core, times_core):
    zt = np.ascontiguousarray(z_core.transpose(2, 1, 0))       # (5, NPC, T)
    dt = np.empty_like(times_core)
    dt[0] = 0.0
    dt[1:] = times_core[1:] - times_core[:-1]
    arr = np.concatenate([zt, dt.T[None]], axis=0)             # (6, NPC, T)
    IN = np.ascontiguousarray(
        arr.reshape(6, NSL, P, CH, T).transpose(1, 0, 2, 3, 4)).astype(np.float16)
    MU = np.ascontiguousarray(
        mu_core.reshape(NSL, P, CH, 5).transpose(0, 1, 3, 2))  # (NSL,P,5,CH)
    return {"inp": IN, "mu": MU}


def kernel(z_and_L_hat, mu0, times):
    z_and_L_hat = np.asarray(z_and_L_hat, dtype=np.float32)
    mu0 = np.asarray(mu0, dtype=np.float32)
    times = np.asarray(times, dtype=np.float32)
    nc = _get_nc()
    in_maps = []
    for k in range(N_CORES):
        sl = slice(k * NPC, (k + 1) * NPC)
        in_maps.append(_pack_core(z_and_L_hat[:, sl, :], mu0[sl], times[:, sl]))
    res = run_bass_kernel_spmd(nc, in_maps, core_ids=list(range(N_CORES)))
    out = np.empty((T, N_TOT, 3), np.float32)
    for k in range(N_CORES):
        O = res.results[k]["out"]                 # (NSL, 3, P, CH, T)
        planes = O.transpose(1, 0, 2, 3, 4).reshape(3, NPC, T)
        sl = slice(k * NPC, (k + 1) * NPC)
        out[:, sl, 0] = planes[0].T
        out[:, sl, 1] = planes[1].T
        out[:, sl, 2] = planes[2].T
    return out


# revision 6
# speedup vs baseline: 1.6154x; 1.0346x over previous
"""Trainium2 Bass kernel for nn_KalmanFilter: EKF over T=512 steps, N=8192 chains.

Mathematical reduction (verified exact vs the reference):
  With C = [[0,0,0,1,0],[0,0,0,0,1]], rows 3,4 of the Jacobian A are zero, so
  S = I + R depends only on per-step measurement params and the covariance
  never influences the output. Per chain:
    S = I + L L^T,  L = [[e^l0, 0], [l1, e^l2]]
    u_{t+1} = (I - S^-1) u_t + S^-1 z_t          (u = [v, omega])
    th_{t+1} = th_t + omega_t * dt_t
    x_{t+1}  = x_t + v_t * dt_t * cos(th_t)
    y_{t+1}  = y_t + v_t * dt_t * sin(th_t)
  The coupled 2-state linear recurrence is solved with 2 Gauss-Seidel sweeps
  of hardware affine scans (error contracts ~10x/sweep; end-to-end rel err
  ~2.4e-3 incl. fp16 quantization, vs the 2e-2 gate).

Implementation notes (from microbenchmarks on this part):
  - GpSimd shares SBUF ports with DVE and degrades it ~4x: all elementwise
    work runs on DVE (fp16, 2x packed mode, ~0.7us/1024el) + Act engine
    (affine/function passes, ~1.1us each, own ports).
  - Scans are DVE-only, ~2.2ns/el, dtype-insensitive. Both chains per
    partition are covered by ONE scan via a zero-multiplier column at each
    chain start (also injects the init value).
  - All intermediates are fp16; t1, t2, s01, s11, det carry a 2^-7 (det
    2^-14) exponent scale folded into Act scale/bias to avoid fp16 overflow;
    r7 = 2^7/det compensates exactly.
Sharding: data-parallel over chains, 1024 chains per core across 8 cores.
"""
import sys
sys.path.insert(0, '/opt/trn_rl_repo')
import numpy as np
import concourse.bass as bass
from concourse import mybir
from concourse.bass_utils import run_bass_kernel_spmd

F32 = mybir.dt.float32
F16 = mybir.dt.float16
AF = mybir.ActivationFunctionType
A = mybir.AluOpType

N_CORES = 8
T = 512
N_TOT = 8192
NPC = N_TOT // N_CORES          # 1024 chains per core
P = 128                         # partitions
NSL = 4                         # slabs per core
CH = NPC // (NSL * P)           # chains per partition per slab = 2
TP = T + 2                      # padded per-chain row: [init | T data | pad]
MAGIC = float(1.5 * 2 ** 23)
TWO_PI = float(2 * np.pi)
INV_2PI = float(1.0 / (2 * np.pi))
LN2x7 = float(7 * np.log(2.0))
HS = float(2.0 ** -3.5)         # Square scale for l1^2 * 2^-7
EPS7 = float(2.0 ** -7)


class _Sched:
    """Two-phase scheduler: record ops (engine, emit closure, deps), then emit
    per-engine in-order streams with standalone wait_ge for cross-engine deps."""

    def __init__(self):
        self.ops = []
        self.count = {"v": 0, "g": 0, "a": 0}
        self.slot_count = {}

    def add(self, eng, emit_fn, deps=(), slot=None):
        if eng == "s":
            self.slot_count[slot] = self.slot_count.get(slot, 0) + 1
            ref = ("D", slot, self.slot_count[slot])
        else:
            self.count[eng] += 1
            ref = (eng, self.count[eng])
        self.ops.append((eng, emit_fn, tuple(d for d in deps if d), ref))
        return ref

    def emit(self, eng, raw_eng, sems, dma_sems):
        last = {}
        dlast = {}
        for op_eng, emit_fn, deps, ref in self.ops:
            if op_eng != eng:
                continue
            for dep in deps:
                if dep[0] == "D":
                    _, slot, k = dep
                    if dlast.get(slot, 0) >= k:
                        continue
                    raw_eng.wait_ge(dma_sems[slot], 16 * k)
                    dlast[slot] = k
                else:
                    deng, dpos = dep
                    if deng == eng or last.get(deng, 0) >= dpos:
                        continue
                    raw_eng.wait_ge(sems[deng], dpos)
                    last[deng] = dpos
            emit_fn().then_inc(sems[eng], 1)


class _Graph:
    """Auto RAW/WAR/WAW dependency tracking over named tiles."""

    def __init__(self, sch):
        self.sch = sch
        self.w = {}
        self.r = {}

    def op(self, eng, emit_fn, reads=(), writes=(), slot=None, extra=()):
        deps = {}

        def add(ref):
            if ref is None:
                return
            key = ref[0] if ref[0] != "D" else ("D", ref[1])
            cur = deps.get(key)
            if cur is None or ref[-1] > cur[-1]:
                deps[key] = ref

        for t in reads:
            add(self.w.get(t))
        for t in writes:
            add(self.w.get(t))
            for rr in self.r.get(t, ()):
                add(rr)
        for e in extra:
            add(e)
        ref = self.sch.add(eng, emit_fn, deps=tuple(deps.values()), slot=slot)
        for t in reads:
            self.r.setdefault(t, []).append(ref)
        for t in writes:
            self.w[t] = ref
            self.r[t] = []
        return ref


def _build_nc():
    nc = bass.Bass()
    # Register activation bias constants (bass converts float biases of
    # non-Copy activations to const APs, which must pre-exist).
    for val in (-LN2x7, EPS7, MAGIC, -MAGIC):
        t = nc.alloc_sbuf_tensor(f"constf32-{val}", [128, 1], F32)
        nc.gpsimd.memset(t.ap(), val)
        nc.const_aps.aps[(F32, val)] = t.ap()
    nc.all_engine_barrier()
    IN = nc.dram_tensor("inp", [NSL, 6, P, CH, T], F16, kind="ExternalInput")
    MU = nc.dram_tensor("mu", [NSL, P, 5, CH], F32, kind="ExternalInput")
    OUT = nc.dram_tensor("out", [NSL, 3, P, CH, T], F32, kind="ExternalOutput")

    _names = [0]

    def tile(shape, dt=F16):
        _names[0] += 1
        return nc.alloc_sbuf_tensor(f"tl{_names[0]}", list(shape), dt).ap()

    def flat(ap):
        return ap.rearrange('p a b -> p (a b)')

    # constants
    ONB = tile([P, 4, TP], F32)          # ones, 0 at col0 of each chain row

    # per-parity tiles
    zin = [tile([P, 6, CH, T]) for _ in range(2)]
    mu = [tile([P, 5, CH], F32) for _ in range(2)]
    M0x = [tile([P, CH, TP]) for _ in range(2)]
    M1x = [tile([P, CH, TP]) for _ in range(2)]
    B0x = [tile([P, CH, TP]) for _ in range(2)]
    CWx = [tile([P, CH, TP]) for _ in range(2)]
    CVx = [tile([P, CH, TP]) for _ in range(2)]
    Gx = [tile([P, CH, TP]) for _ in range(2)]
    GXY = [tile([P, 4, TP]) for _ in range(2)]
    Vt = [tile([P, CH, TP]) for _ in range(2)]
    Wt = [tile([P, CH, TP]) for _ in range(2)]
    TH = [tile([P, CH, TP], F32) for _ in range(2)]
    XY = [tile([P, 4, TP], F32) for _ in range(2)]
    K1 = [tile([P, CH, T], F32) for _ in range(2)]
    # aliased fp16 scratch [P, CH, T]; one list of phys tiles per parity
    SCR = [[tile([P, CH, T]) for _ in range(11)] for _ in range(2)]

    sch = _Sched()
    g = _Graph(sch)

    # ---- preamble ----
    g.op("v", lambda: nc.vector.memset(flat(ONB), 1.0), writes=("ONB",))
    for c in range(4):
        g.op("v", lambda c=c: nc.vector.memset(ONB[:, c, 0:1], 0.0),
             writes=("ONB",))
    for bi in range(2):
        for nm, tl in (("M0x", M0x), ("M1x", M1x), ("B0x", B0x), ("CWx", CWx),
                       ("CVx", CVx), ("Gx", Gx)):
            g.op("v", lambda tl=tl, bi=bi: nc.vector.memset(tl[bi][:, :, TP - 1:TP], 0.0),
                 writes=(f"{nm}{bi}",))
        g.op("v", lambda bi=bi: nc.vector.memset(GXY[bi][:, :, TP - 1:TP], 0.0),
             writes=(f"GXY{bi}",))
        for nm, tl in (("M0x", M0x), ("M1x", M1x)):
            g.op("v", lambda tl=tl, bi=bi: nc.vector.memset(tl[bi][:, :, 0:1], 0.0),
                 writes=(f"{nm}{bi}",))

    def key(nm, G):
        return f"{nm}{G % 2}"

    # scratch allocation per logical name (phys index into SCR)
    PHYS = {"e0sq": 0, "u0": 0, "vdt": 0,
            "e2sq": 1, "u1": 1, "thr": 1,
            "e0h": 2, "m01": 2,
            "l1sq": 3, "p0": 3,
            "t1h": 4, "p1": 4,
            "t2h": 5, "q0": 5,
            "d1h": 6, "lnd": 6, "b1": 6,
            "deth": 7, "q1": 7,
            "s01h": 8, "sinf": 8,
            "s11h": 9, "cosf": 9,
            "r7": 10, "tmp": 10}

    def scr(nm, G):
        return SCR[G % 2][PHYS[nm]]

    def skey(nm, G):
        return f"S{PHYS[nm]}_{G % 2}"

    # ---------------- stages ----------------
    def dma_in(G):
        s = G % NSL
        bi = G % 2
        g.op("s", lambda: nc.sync.dma_start(
            zin[bi][:], IN[s].rearrange("k p c t -> p k c t")),
            reads=(), writes=(key("zin", G),), slot=s * 5 + 0)
        g.op("s", lambda: nc.sync.dma_start(mu[bi][:], MU[s]),
            reads=(), writes=(key("mu", G),), slot=s * 5 + 1)

    def act(out_ap, in_ap, func, bias=0.0, scale=1.0):
        return lambda: nc.scalar.activation(out_ap, in_ap, func,
                                            bias=bias, scale=scale)

    def leaf_acts(G):
        bi = G % 2
        z = zin[bi]
        l0 = z[:, 2]; l1 = z[:, 3]; l2 = z[:, 4]
        zk = key("zin", G)
        g.op("a", act(scr("e0sq", G)[:], l0, AF.Exp, bias=-LN2x7, scale=2.0),
             reads=(zk,), writes=(skey("e0sq", G),))
        g.op("a", act(scr("e2sq", G)[:], l2, AF.Exp, bias=-LN2x7, scale=2.0),
             reads=(zk,), writes=(skey("e2sq", G),))
        g.op("a", act(scr("e0h", G)[:], l0, AF.Exp, bias=-LN2x7),
             reads=(zk,), writes=(skey("e0h", G),))
        g.op("a", act(scr("l1sq", G)[:], l1, AF.Square, scale=HS),
             reads=(zk,), writes=(skey("l1sq", G),))
        g.op("a", act(scr("t1h", G)[:], scr("e0sq", G)[:], AF.Identity, bias=EPS7),
             reads=(skey("e0sq", G),), writes=(skey("t1h", G),))
        g.op("a", act(scr("t2h", G)[:], scr("e2sq", G)[:], AF.Identity, bias=EPS7),
             reads=(skey("e2sq", G),), writes=(skey("t2h", G),))

    def tt(out, in0, in1, op):
        return lambda: nc.vector.tensor_tensor(out, in0, in1, op)

    def alg_front(G):
        g.op("v", tt(scr("d1h", G)[:], scr("t1h", G)[:], scr("t2h", G)[:], A.mult),
             reads=(skey("t1h", G), skey("t2h", G)), writes=(skey("d1h", G),))
        g.op("v", tt(scr("s01h", G)[:], scr("e0h", G)[:], zin[G % 2][:, 3], A.mult),
             reads=(skey("e0h", G), key("zin", G)), writes=(skey("s01h", G),))
        g.op("v", tt(scr("s11h", G)[:], scr("t2h", G)[:], scr("l1sq", G)[:], A.add),
             reads=(skey("t2h", G), skey("l1sq", G)), writes=(skey("s11h", G),))
        g.op("v", lambda G=G: nc.vector.scalar_tensor_tensor(
            scr("deth", G)[:], scr("l1sq", G)[:], EPS7, scr("d1h", G)[:],
            A.mult, A.add),
            reads=(skey("l1sq", G), skey("d1h", G)), writes=(skey("deth", G),))

    def lnd_r7(G):
        g.op("a", act(scr("lnd", G)[:], scr("deth", G)[:], AF.Ln),
             reads=(skey("deth", G),), writes=(skey("lnd", G),))
        g.op("a", act(scr("r7", G)[:], scr("lnd", G)[:], AF.Exp,
                      bias=-LN2x7, scale=-1.0),
             reads=(skey("lnd", G),), writes=(skey("r7", G),))

    def alg_mid(G):
        z = zin[G % 2]
        z0 = z[:, 0]; z1 = z[:, 1]
        zk = key("zin", G)
        g.op("v", tt(scr("u0", G)[:], scr("s11h", G)[:], scr("r7", G)[:], A.mult),
             reads=(skey("s11h", G), skey("r7", G)), writes=(skey("u0", G),))
        g.op("v", tt(scr("u1", G)[:], scr("t1h", G)[:], scr("r7", G)[:], A.mult),
             reads=(skey("t1h", G), skey("r7", G)), writes=(skey("u1", G),))
        g.op("v", tt(scr("m01", G)[:], scr("s01h", G)[:], scr("r7", G)[:], A.mult),
             reads=(skey("s01h", G), skey("r7", G)), writes=(skey("m01", G),))
        g.op("v", tt(scr("p0", G)[:], scr("u0", G)[:], z0, A.mult),
             reads=(skey("u0", G), zk), writes=(skey("p0", G),))
        g.op("v", tt(scr("q0", G)[:], scr("m01", G)[:], z1, A.mult),
             reads=(skey("m01", G), zk), writes=(skey("q0", G),))
        g.op("v", tt(B0x[G % 2][:, :, 1:T + 1], scr("p0", G)[:], scr("q0", G)[:],
                     A.subtract),
             reads=(skey("p0", G), skey("q0", G)), writes=(key("B0x", G),))
        g.op("v", tt(scr("p1", G)[:], scr("u1", G)[:], z1, A.mult),
             reads=(skey("u1", G), zk), writes=(skey("p1", G),))
        g.op("v", tt(scr("q1", G)[:], scr("m01", G)[:], z0, A.mult),
             reads=(skey("m01", G), zk), writes=(skey("q1", G),))
        g.op("v", tt(scr("b1", G)[:], scr("p1", G)[:], scr("q1", G)[:], A.subtract),
             reads=(skey("p1", G), skey("q1", G)), writes=(skey("b1", G),))

    def m_acts(G):
        bi = G % 2
        g.op("a", act(M0x[bi][:, :, 1:T + 1], scr("u0", G)[:], AF.Identity,
                      bias=1.0, scale=-1.0),
             reads=(skey("u0", G),), writes=(key("M0x", G),))
        g.op("a", act(M1x[bi][:, :, 1:T + 1], scr("u1", G)[:], AF.Identity,
                      bias=1.0, scale=-1.0),
             reads=(skey("u1", G),), writes=(key("M1x", G),))
        m = mu[bi]
        mk = key("mu", G)
        for dst_ap, dst_key, mi in (
                (B0x[bi][:, :, 0], "B0x", 3), (CVx[bi][:, :, 0], "CVx", 3),
                (CWx[bi][:, :, 0], "CWx", 4), (Gx[bi][:, :, 0], "Gx", 2),
                (GXY[bi][:, 0:2, 0], "GXY", 0), (GXY[bi][:, 2:4, 0], "GXY", 1)):
            g.op("a", act(dst_ap, m[:, mi], AF.Identity),
                 reads=(mk,), writes=(key(dst_key, G),))

    def scan(out, d0, d1):
        return lambda: nc.vector.tensor_tensor_scan(out, d0, d1, 0.0,
                                                    A.mult, A.add)

    def sweeps_a(G):
        bi = G % 2
        m01 = scr("m01", G)
        tmp = scr("tmp", G)
        b1 = scr("b1", G)
        g.op("v", scan(flat(Vt[bi]), flat(M0x[bi]), flat(B0x[bi])),
             reads=(key("M0x", G), key("B0x", G)), writes=(key("V", G),))
        g.op("v", tt(tmp[:], m01[:], Vt[bi][:, :, 0:T], A.mult),
             reads=(skey("m01", G), key("V", G)), writes=(skey("tmp", G),))
        g.op("v", tt(CWx[bi][:, :, 1:T + 1], tmp[:], b1[:], A.add),
             reads=(skey("tmp", G), skey("b1", G)), writes=(key("CWx", G),))
        g.op("v", scan(flat(Wt[bi]), flat(M1x[bi]), flat(CWx[bi])),
             reads=(key("M1x", G), key("CWx", G)), writes=(key("W", G),))

    def sweeps_b(G):
        bi = G % 2
        m01 = scr("m01", G)
        tmp = scr("tmp", G)
        b1 = scr("b1", G)
        g.op("v", tt(tmp[:], m01[:], Wt[bi][:, :, 0:T], A.mult),
             reads=(skey("m01", G), key("W", G)), writes=(skey("tmp", G),))
        g.op("v", tt(CVx[bi][:, :, 1:T + 1], tmp[:], B0x[bi][:, :, 1:T + 1], A.add),
             reads=(skey("tmp", G), key("B0x", G)), writes=(key("CVx", G),))
        g.op("v", scan(flat(Vt[bi]), flat(M0x[bi]), flat(CVx[bi])),
             reads=(key("M0x", G), key("CVx", G)), writes=(key("V", G),))
        g.op("v", tt(tmp[:], m01[:], Vt[bi][:, :, 0:T], A.mult),
             reads=(skey("m01", G), key("V", G)), writes=(skey("tmp", G),))
        g.op("v", tt(CWx[bi][:, :, 1:T + 1], tmp[:], b1[:], A.add),
             reads=(skey("tmp", G), skey("b1", G)), writes=(key("CWx", G),))
        g.op("v", scan(flat(Wt[bi]), flat(M1x[bi]), flat(CWx[bi])),
             reads=(key("M1x", G), key("CWx", G)), writes=(key("W", G),))
        g.op("v", tt(Gx[bi][:, :, 1:T + 1], Wt[bi][:, :, 0:T], zin[bi][:, 5],
                     A.mult),
             reads=(key("W", G), key("zin", G)), writes=(key("Gx", G),))
        g.op("v", scan(flat(TH[bi]), flat(ONB[:, 0:2]), flat(Gx[bi])),
             reads=("ONB", key("Gx", G)), writes=(key("TH", G),))

    def kchain(G):
        # Range reduction on DVE: k1 = th/2pi + MAGIC (round-to-int trick),
        # k2 = (k1 - MAGIC)*2pi (exact: k1-MAGIC is Sterbenz-exact),
        # thr = th - k2 in [-pi, pi].
        bi = G % 2
        g.op("v", lambda: nc.vector.tensor_scalar(
            K1[bi][:], TH[bi][:, :, 0:T], INV_2PI, MAGIC, op0=A.mult, op1=A.add),
            reads=(key("TH", G),), writes=(key("K1", G),))
        g.op("v", lambda: nc.vector.tensor_scalar(
            K1[bi][:], K1[bi][:], -MAGIC, TWO_PI, op0=A.add, op1=A.mult),
            reads=(key("K1", G),), writes=(key("K1", G),))
        g.op("v", tt(scr("thr", G)[:], TH[bi][:, :, 0:T], K1[bi][:], A.subtract),
             reads=(key("TH", G), key("K1", G)), writes=(skey("thr", G),))

    def trig(G):
        thr = scr("thr", G)
        sinf = scr("sinf", G)
        cosf = scr("cosf", G)
        g.op("a", act(sinf[:], thr[:], AF.Sin),
             reads=(skey("thr", G),), writes=(skey("sinf", G),))
        g.op("a", act(cosf[:], thr[:], AF.Sin, scale=0.5),
             reads=(skey("thr", G),), writes=(skey("cosf", G),))
        g.op("a", act(cosf[:], cosf[:], AF.Square),
             reads=(skey("cosf", G),), writes=(skey("cosf", G),))
        g.op("a", act(cosf[:], cosf[:], AF.Identity, bias=1.0, scale=-2.0),
             reads=(skey("cosf", G),), writes=(skey("cosf", G),))

    def tail(G):
        bi = G % 2
        vdt = scr("vdt", G)
        g.op("v", tt(vdt[:], Vt[bi][:, :, 0:T], zin[bi][:, 5], A.mult),
             reads=(key("V", G), key("zin", G)), writes=(skey("vdt", G),))
        g.op("v", tt(GXY[bi][:, 2:4, 1:T + 1], vdt[:], scr("sinf", G)[:], A.mult),
             reads=(skey("vdt", G), skey("sinf", G)), writes=(key("GXY", G),))
        g.op("v", tt(GXY[bi][:, 0:2, 1:T + 1], vdt[:], scr("cosf", G)[:], A.mult),
             reads=(skey("vdt", G), skey("cosf", G)), writes=(key("GXY", G),))
        g.op("v", scan(flat(XY[bi]), flat(ONB), flat(GXY[bi])),
             reads=("ONB", key("GXY", G)), writes=(key("XY", G),))

    def dma_out(G):
        s = G % NSL
        bi = G % 2
        g.op("s", lambda: nc.sync.dma_start(OUT[s, 0], XY[bi][:, 0:2, 1:T + 1]),
             reads=(key("XY", G),), writes=(), slot=s * 5 + 2)
        g.op("s", lambda: nc.sync.dma_start(OUT[s, 1], XY[bi][:, 2:4, 1:T + 1]),
             reads=(key("XY", G),), writes=(), slot=s * 5 + 3)
        g.op("s", lambda: nc.sync.dma_start(OUT[s, 2], TH[bi][:, :, 1:T + 1]),
             reads=(key("TH", G),), writes=(), slot=s * 5 + 4)

    # ---------------- emission ----------------
    for G in range(NSL):
        dma_in(G)
        leaf_acts(G)
        alg_front(G)
        lnd_r7(G)
        if G > 0:
            sweeps_a(G - 1)
        alg_mid(G)
        m_acts(G)
        if G > 0:
            sweeps_b(G - 1)
            kchain(G - 1)
            trig(G - 1)
            tail(G - 1)
            dma_out(G - 1)
    Gl = NSL - 1
    sweeps_a(Gl)
    sweeps_b(Gl)
    kchain(Gl)
    trig(Gl)
    tail(Gl)
    dma_out(Gl)

    n_slots = NSL * 5
    sem_v = nc.alloc_semaphore()
    sem_g = nc.alloc_semaphore()
    sem_a = nc.alloc_semaphore()
    dma_sems = [nc.alloc_semaphore(f"dsem{i}") for i in range(n_slots)]
    with nc.Block() as block:
        sems = {"v": sem_v, "g": sem_g, "a": sem_a}

        @block.sync
        def _(sync):
            last = {}
            dlast = {}
            for op_eng, emit_fn, deps, ref in sch.ops:
                if op_eng != "s":
                    continue
                for dep in deps:
                    if dep[0] == "D":
                        _, slot, k = dep
                        if dlast.get(slot, 0) >= k:
                            continue
                        sync.wait_ge(dma_sems[slot], 16 * k)
                        dlast[slot] = k
                    else:
                        deng, dpos = dep
                        if deng == "s" or last.get(deng, 0) >= dpos:
                            continue
                        sync.wait_ge(sems[deng], dpos)
                        last[deng] = dpos
                emit_fn().then_inc(dma_sems[ref[1]], 16)

        @block.vector
        def _(vector):
            sch.emit("v", vector, sems, dma_sems)

        @block.gpsimd
        def _(gp):
            sch.emit("g", gp, sems, dma_sems)

        @block.scalar
        def _(scalar):
            sch.emit("a", scalar, sems, dma_sems)

    return nc


_cache = {}


def _get_nc():
    if "nc" not in _cache:
        _cache["nc"] = _build_nc()
    return _cache["nc"]


def _pack_core(z_core, mu_core, times_core):
    zt = np.ascontiguousarray(z_core.transpose(2, 1, 0))       # (5, NPC, T)
    dt = np.empty_like(times_core)
    dt[0] = 0.0
    dt[1:] = times_core[1:] - times_core[:-1]
    arr = np.concatenate([zt, dt.T[None]], axis=0)             # (6, NPC, T)
    IN = np.ascontiguousarray(
        arr.reshape(6, NSL, P, CH, T).transpose(1, 0, 2, 3, 4)).astype(np.float16)
    MU = np.ascontiguousarray(
        mu_core.reshape(NSL, P, CH, 5).transpose(0, 1, 3, 2))  # (NSL,P,5,CH)
    return {"inp": IN, "mu": MU}


def kernel(z_and_L_hat, mu0, times):
    z_and_L_hat = np.asarray(z_and_L_hat, dtype=np.float32)
    mu0 = np.asarray(mu0, dtype=np.float32)
    times = np.asarray(times, dtype=np.float32)
    nc = _get_nc()
    in_maps = []
    for k in range(N_CORES):
        sl = slice(k * NPC, (k + 1) * NPC)
        in_maps.append(_pack_core(z_and_L_hat[:, sl, :], mu0[sl], times[:, sl]))
    res = run_bass_kernel_spmd(nc, in_maps, core_ids=list(range(N_CORES)))
    out = np.empty((T, N_TOT, 3), np.float32)
    for k in range(N_CORES):
        O = res.results[k]["out"]                 # (NSL, 3, P, CH, T)
        planes = O.transpose(1, 0, 2, 3, 4).reshape(3, NPC, T)
        sl = slice(k * NPC, (k + 1) * NPC)
        out[:, sl, 0] = planes[0].T
        out[:, sl, 1] = planes[1].T
        out[:, sl, 2] = planes[2].T
    return out


# revision 8
# speedup vs baseline: 1.6681x; 1.0326x over previous
"""Trainium2 Bass kernel for nn_KalmanFilter: EKF over T=512 steps, N=8192 chains.

Mathematical reduction (verified exact vs the reference):
  With C = [[0,0,0,1,0],[0,0,0,0,1]], rows 3,4 of the Jacobian A are zero, so
  S = I + R depends only on per-step measurement params and the covariance
  never influences the output. Per chain:
    S = I + L L^T,  L = [[e^l0, 0], [l1, e^l2]]
    u_{t+1} = (I - S^-1) u_t + S^-1 z_t          (u = [v, omega])
    th_{t+1} = th_t + omega_t * dt_t
    x_{t+1}  = x_t + v_t * dt_t * cos(th_t)
    y_{t+1}  = y_t + v_t * dt_t * sin(th_t)
  The coupled 2-state linear recurrence is solved with 2 Gauss-Seidel sweeps
  of hardware affine scans (error contracts ~10x/sweep; end-to-end rel err
  ~2.4e-3 incl. fp16 quantization, vs the 2e-2 gate).

Implementation notes (from microbenchmarks on this part):
  - GpSimd shares SBUF ports with DVE and degrades it ~4x: all elementwise
    work runs on DVE (fp16, 2x packed mode, ~0.7us/1024el) + Act engine
    (affine/function passes, ~1.1us each, own ports).
  - Scans are DVE-only, ~2.2ns/el, dtype-insensitive. Both chains per
    partition are covered by ONE scan via a zero-multiplier column at each
    chain start (also injects the init value).
  - All intermediates are fp16; t1, t2, s01, s11, det carry a 2^-7 (det
    2^-14) exponent scale folded into Act scale/bias to avoid fp16 overflow;
    r7 = 2^7/det compensates exactly.
Sharding: data-parallel over chains, 1024 chains per core across 8 cores.
"""
import sys
sys.path.insert(0, '/opt/trn_rl_repo')
import numpy as np
import concourse.bass as bass
from concourse import mybir
from concourse.bass_utils import run_bass_kernel_spmd

F32 = mybir.dt.float32
F16 = mybir.dt.float16
AF = mybir.ActivationFunctionType
A = mybir.AluOpType

N_CORES = 8
T = 512
N_TOT = 8192
NPC = N_TOT // N_CORES          # 1024 chains per core
P = 128                         # partitions
NSL = 4                         # slabs per core
CH = NPC // (NSL * P)           # chains per partition per slab = 2
TP = T + 2                      # padded per-chain row: [init | T data | pad]
MAGIC = float(1.5 * 2 ** 23)
TWO_PI = float(2 * np.pi)
INV_2PI = float(1.0 / (2 * np.pi))
LN2x7 = float(7 * np.log(2.0))
HS = float(2.0 ** -3.5)         # Square scale for l1^2 * 2^-7
EPS7 = float(2.0 ** -7)


class _Sched:
    """Two-phase scheduler: record ops (engine, emit closure, deps), then emit
    per-engine in-order streams with standalone wait_ge for cross-engine deps."""

    def __init__(self):
        self.ops = []
        self.count = {"v": 0, "g": 0, "a": 0}
        self.slot_count = {}

    def add(self, eng, emit_fn, deps=(), slot=None):
        if eng == "s":
            self.slot_count[slot] = self.slot_count.get(slot, 0) + 1
            ref = ("D", slot, self.slot_count[slot])
        else:
            self.count[eng] += 1
            ref = (eng, self.count[eng])
        self.ops.append((eng, emit_fn, tuple(d for d in deps if d), ref))
        return ref

    def emit(self, eng, raw_eng, sems, dma_sems):
        last = {}
        dlast = {}
        for op_eng, emit_fn, deps, ref in self.ops:
            if op_eng != eng:
                continue
            for dep in deps:
                if dep[0] == "D":
                    _, slot, k = dep
                    if dlast.get(slot, 0) >= k:
                        continue
                    raw_eng.wait_ge(dma_sems[slot], 16 * k)
                    dlast[slot] = k
                else:
                    deng, dpos = dep
                    if deng == eng or last.get(deng, 0) >= dpos:
                        continue
                    raw_eng.wait_ge(sems[deng], dpos)
                    last[deng] = dpos
            emit_fn().then_inc(sems[eng], 1)


class _Graph:
    """Auto RAW/WAR/WAW dependency tracking over named tiles."""

    def __init__(self, sch):
        self.sch = sch
        self.w = {}
        self.r = {}

    def op(self, eng, emit_fn, reads=(), writes=(), slot=None, extra=()):
        deps = {}

        def add(ref):
            if ref is None:
                return
            key = ref[0] if ref[0] != "D" else ("D", ref[1])
            cur = deps.get(key)
            if cur is None or ref[-1] > cur[-1]:
                deps[key] = ref

        for t in reads:
            add(self.w.get(t))
        for t in writes:
            add(self.w.get(t))
            for rr in self.r.get(t, ()):
                add(rr)
        for e in extra:
            add(e)
        ref = self.sch.add(eng, emit_fn, deps=tuple(deps.values()), slot=slot)
        for t in reads:
            self.r.setdefault(t, []).append(ref)
        for t in writes:
            self.w[t] = ref
            self.r[t] = []
        return ref


def _build_nc():
    nc = bass.Bass()
    # Register activation bias constants (bass converts float biases of
    # non-Copy activations to const APs, which must pre-exist).
    for val in (-LN2x7, EPS7, MAGIC, -MAGIC):
        t = nc.alloc_sbuf_tensor(f"constf32-{val}", [128, 1], F32)
        nc.gpsimd.memset(t.ap(), val)
        nc.const_aps.aps[(F32, val)] = t.ap()
    nc.all_engine_barrier()
    IN = nc.dram_tensor("inp", [NSL, 5, P, CH, T], F16, kind="ExternalInput")
    DT = nc.dram_tensor("dt", [NSL, P, CH, T], F16, kind="ExternalInput")
    MU = nc.dram_tensor("mu", [NSL, P, 5, CH], F32, kind="ExternalInput")
    OUT = nc.dram_tensor("out", [NSL, 3, P, CH, T], F32, kind="ExternalOutput")

    _names = [0]

    def tile(shape, dt=F16):
        _names[0] += 1
        return nc.alloc_sbuf_tensor(f"tl{_names[0]}", list(shape), dt).ap()

    def flat(ap):
        return ap.rearrange('p a b -> p (a b)')

    # constants
    ONB = tile([P, 4, TP], F32)          # ones, 0 at col0 of each chain row

    # per-parity tiles
    zin = [tile([P, 5, CH, T]) for _ in range(2)]
    dtz = [tile([P, CH, T]) for _ in range(2)]
    mu = [tile([P, 5, CH], F32) for _ in range(2)]
    M0x = [tile([P, CH, TP]) for _ in range(2)]
    M1x = [tile([P, CH, TP]) for _ in range(2)]
    B0x = [tile([P, CH, TP]) for _ in range(2)]
    CWx = [tile([P, CH, TP]) for _ in range(2)]
    CVx = [tile([P, CH, TP]) for _ in range(2)]
    Gx = [tile([P, CH, TP]) for _ in range(2)]
    GXY = [tile([P, 4, TP]) for _ in range(2)]
    Vt = [tile([P, CH, TP]) for _ in range(2)]
    Wt = [tile([P, CH, TP]) for _ in range(2)]
    TH = [tile([P, CH, TP], F32) for _ in range(2)]
    XY = [tile([P, 4, TP], F32) for _ in range(2)]
    K1 = [tile([P, CH, T], F32) for _ in range(2)]
    # aliased fp16 scratch [P, CH, T]; one list of phys tiles per parity
    SCR = [[tile([P, CH, T]) for _ in range(14)] for _ in range(2)]

    sch = _Sched()
    g = _Graph(sch)

    # ---- preamble ----
    g.op("v", lambda: nc.vector.memset(flat(ONB), 1.0), writes=("ONB",))
    for c in range(4):
        g.op("v", lambda c=c: nc.vector.memset(ONB[:, c, 0:1], 0.0),
             writes=("ONB",))
    for bi in range(2):
        for nm, tl in (("M0x", M0x), ("M1x", M1x), ("B0x", B0x), ("CWx", CWx),
                       ("CVx", CVx), ("Gx", Gx)):
            g.op("v", lambda tl=tl, bi=bi: nc.vector.memset(tl[bi][:, :, TP - 1:TP], 0.0),
                 writes=(f"{nm}{bi}",))
        g.op("v", lambda bi=bi: nc.vector.memset(GXY[bi][:, :, TP - 1:TP], 0.0),
             writes=(f"GXY{bi}",))
        for nm, tl in (("M0x", M0x), ("M1x", M1x)):
            g.op("v", lambda tl=tl, bi=bi: nc.vector.memset(tl[bi][:, :, 0:1], 0.0),
                 writes=(f"{nm}{bi}",))

    def key(nm, G):
        return f"{nm}{G % 2}"

    # scratch allocation per logical name (phys index into SCR)
    PHYS = {"e0sq": 0, "u0": 0,
            "e2sq": 1, "u1": 1,
            "vdt": 11, "thr": 12,
            "e0h": 2, "m01": 13,
            "l1sq": 3, "p0": 3,
            "t1h": 4, "p1": 4,
            "t2h": 5, "q0": 5,
            "d1h": 6, "lnd": 6, "b1": 6,
            "deth": 7, "q1": 7,
            "s01h": 8, "sinf": 8,
            "s11h": 9, "cosf": 9,
            "r7": 10, "tmp": 10}

    def scr(nm, G):
        return SCR[G % 2][PHYS[nm]]

    def skey(nm, G):
        return f"S{PHYS[nm]}_{G % 2}"

    # ---------------- stages ----------------
    def dma_in(G):
        s = G % NSL
        bi = G % 2
        g.op("s", lambda: nc.sync.dma_start(
            zin[bi][:], IN[s].rearrange("k p c t -> p k c t")),
            reads=(), writes=(key("zin", G),), slot=s * 6 + 0)
        g.op("s", lambda: nc.sync.dma_start(mu[bi][:], MU[s]),
            reads=(), writes=(key("mu", G),), slot=s * 6 + 1)

    def dma_dt(G):
        s = G % NSL
        bi = G % 2
        g.op("s", lambda: nc.sync.dma_start(dtz[bi][:], DT[s]),
            reads=(), writes=(key("dtz", G),), slot=s * 6 + 2)

    def act(out_ap, in_ap, func, bias=0.0, scale=1.0):
        return lambda: nc.scalar.activation(out_ap, in_ap, func,
                                            bias=bias, scale=scale)

    def leaf_acts(G):
        bi = G % 2
        z = zin[bi]
        l0 = z[:, 2]; l1 = z[:, 3]; l2 = z[:, 4]
        zk = key("zin", G)
        g.op("a", act(scr("e0sq", G)[:], l0, AF.Exp, bias=-LN2x7, scale=2.0),
             reads=(zk,), writes=(skey("e0sq", G),))
        g.op("a", act(scr("e2sq", G)[:], l2, AF.Exp, bias=-LN2x7, scale=2.0),
             reads=(zk,), writes=(skey("e2sq", G),))
        g.op("a", act(scr("e0h", G)[:], l0, AF.Exp, bias=-LN2x7),
             reads=(zk,), writes=(skey("e0h", G),))
        g.op("a", act(scr("l1sq", G)[:], l1, AF.Square, scale=HS),
             reads=(zk,), writes=(skey("l1sq", G),))
        g.op("a", act(scr("t1h", G)[:], scr("e0sq", G)[:], AF.Identity, bias=EPS7),
             reads=(skey("e0sq", G),), writes=(skey("t1h", G),))
        g.op("a", act(scr("t2h", G)[:], scr("e2sq", G)[:], AF.Identity, bias=EPS7),
             reads=(skey("e2sq", G),), writes=(skey("t2h", G),))

    def tt(out, in0, in1, op):
        return lambda: nc.vector.tensor_tensor(out, in0, in1, op)

    def alg_front(G):
        g.op("v", tt(scr("d1h", G)[:], scr("t1h", G)[:], scr("t2h", G)[:], A.mult),
             reads=(skey("t1h", G), skey("t2h", G)), writes=(skey("d1h", G),))
        g.op("v", tt(scr("s01h", G)[:], scr("e0h", G)[:], zin[G % 2][:, 3], A.mult),
             reads=(skey("e0h", G), key("zin", G)), writes=(skey("s01h", G),))
        g.op("v", tt(scr("s11h", G)[:], scr("t2h", G)[:], scr("l1sq", G)[:], A.add),
             reads=(skey("t2h", G), skey("l1sq", G)), writes=(skey("s11h", G),))
        g.op("v", lambda G=G: nc.vector.scalar_tensor_tensor(
            scr("deth", G)[:], scr("l1sq", G)[:], EPS7, scr("d1h", G)[:],
            A.mult, A.add),
            reads=(skey("l1sq", G), skey("d1h", G)), writes=(skey("deth", G),))

    def lnd_r7(G):
        g.op("a", act(scr("lnd", G)[:], scr("deth", G)[:], AF.Ln),
             reads=(skey("deth", G),), writes=(skey("lnd", G),))
        g.op("a", act(scr("r7", G)[:], scr("lnd", G)[:], AF.Exp,
                      bias=-LN2x7, scale=-1.0),
             reads=(skey("lnd", G),), writes=(skey("r7", G),))

    def alg_mid(G):
        z = zin[G % 2]
        z0 = z[:, 0]; z1 = z[:, 1]
        zk = key("zin", G)
        g.op("v", tt(scr("u0", G)[:], scr("s11h", G)[:], scr("r7", G)[:], A.mult),
             reads=(skey("s11h", G), skey("r7", G)), writes=(skey("u0", G),))
        g.op("v", tt(scr("u1", G)[:], scr("t1h", G)[:], scr("r7", G)[:], A.mult),
             reads=(skey("t1h", G), skey("r7", G)), writes=(skey("u1", G),))
        g.op("v", tt(scr("m01", G)[:], scr("s01h", G)[:], scr("r7", G)[:], A.mult),
             reads=(skey("s01h", G), skey("r7", G)), writes=(skey("m01", G),))
        g.op("v", tt(scr("p0", G)[:], scr("u0", G)[:], z0, A.mult),
             reads=(skey("u0", G), zk), writes=(skey("p0", G),))
        g.op("v", tt(scr("q0", G)[:], scr("m01", G)[:], z1, A.mult),
             reads=(skey("m01", G), zk), writes=(skey("q0", G),))
        g.op("v", tt(B0x[G % 2][:, :, 1:T + 1], scr("p0", G)[:], scr("q0", G)[:],
                     A.subtract),
             reads=(skey("p0", G), skey("q0", G)), writes=(key("B0x", G),))
        g.op("v", tt(scr("p1", G)[:], scr("u1", G)[:], z1, A.mult),
             reads=(skey("u1", G), zk), writes=(skey("p1", G),))
        g.op("v", tt(scr("q1", G)[:], scr("m01", G)[:], z0, A.mult),
             reads=(skey("m01", G), zk), writes=(skey("q1", G),))
        g.op("v", tt(scr("b1", G)[:], scr("p1", G)[:], scr("q1", G)[:], A.subtract),
             reads=(skey("p1", G), skey("q1", G)), writes=(skey("b1", G),))

    def m_acts(G):
        bi = G % 2
        g.op("a", act(M0x[bi][:, :, 1:T + 1], scr("u0", G)[:], AF.Identity,
                      bias=1.0, scale=-1.0),
             reads=(skey("u0", G),), writes=(key("M0x", G),))
        g.op("a", act(M1x[bi][:, :, 1:T + 1], scr("u1", G)[:], AF.Identity,
                      bias=1.0, scale=-1.0),
             reads=(skey("u1", G),), writes=(key("M1x", G),))
        m = mu[bi]
        mk = key("mu", G)
        for dst_ap, dst_key, mi in (
                (B0x[bi][:, :, 0], "B0x", 3), (CVx[bi][:, :, 0], "CVx", 3),
                (CWx[bi][:, :, 0], "CWx", 4), (Gx[bi][:, :, 0], "Gx", 2),
                (GXY[bi][:, 0:2, 0], "GXY", 0), (GXY[bi][:, 2:4, 0], "GXY", 1)):
            g.op("a", act(dst_ap, m[:, mi], AF.Identity),
                 reads=(mk,), writes=(key(dst_key, G),))

    def scan(out, d0, d1):
        return lambda: nc.vector.tensor_tensor_scan(out, d0, d1, 0.0,
                                                    A.mult, A.add)

    def sweeps_a(G):
        bi = G % 2
        m01 = scr("m01", G)
        tmp = scr("tmp", G)
        b1 = scr("b1", G)
        g.op("v", scan(flat(Vt[bi]), flat(M0x[bi]), flat(B0x[bi])),
             reads=(key("M0x", G), key("B0x", G)), writes=(key("V", G),))
        g.op("v", tt(tmp[:], m01[:], Vt[bi][:, :, 0:T], A.mult),
             reads=(skey("m01", G), key("V", G)), writes=(skey("tmp", G),))
        g.op("v", tt(CWx[bi][:, :, 1:T + 1], tmp[:], b1[:], A.add),
             reads=(skey("tmp", G), skey("b1", G)), writes=(key("CWx", G),))
        g.op("v", scan(flat(Wt[bi]), flat(M1x[bi]), flat(CWx[bi])),
             reads=(key("M1x", G), key("CWx", G)), writes=(key("W", G),))

    def sweeps_b(G):
        bi = G % 2
        m01 = scr("m01", G)
        tmp = scr("tmp", G)
        b1 = scr("b1", G)
        g.op("v", tt(tmp[:], m01[:], Wt[bi][:, :, 0:T], A.mult),
             reads=(skey("m01", G), key("W", G)), writes=(skey("tmp", G),))
        g.op("v", tt(CVx[bi][:, :, 1:T + 1], tmp[:], B0x[bi][:, :, 1:T + 1], A.add),
             reads=(skey("tmp", G), key("B0x", G)), writes=(key("CVx", G),))
        g.op("v", scan(flat(Vt[bi]), flat(M0x[bi]), flat(CVx[bi])),
             reads=(key("M0x", G), key("CVx", G)), writes=(key("V", G),))
        g.op("v", tt(tmp[:], m01[:], Vt[bi][:, :, 0:T], A.mult),
             reads=(skey("m01", G), key("V", G)), writes=(skey("tmp", G),))
        g.op("v", tt(CWx[bi][:, :, 1:T + 1], tmp[:], b1[:], A.add),
             reads=(skey("tmp", G), skey("b1", G)), writes=(key("CWx", G),))
        g.op("v", scan(flat(Wt[bi]), flat(M1x[bi]), flat(CWx[bi])),
             reads=(key("M1x", G), key("CWx", G)), writes=(key("W", G),))
        g.op("v", tt(Gx[bi][:, :, 1:T + 1], Wt[bi][:, :, 0:T], dtz[bi][:],
                     A.mult),
             reads=(key("W", G), key("dtz", G)), writes=(key("Gx", G),))
        g.op("v", scan(flat(TH[bi]), flat(ONB[:, 0:2]), flat(Gx[bi])),
             reads=("ONB", key("Gx", G)), writes=(key("TH", G),))

    def kchain(G):
        # Range reduction on DVE: k1 = th/2pi + MAGIC (round-to-int trick),
        # k2 = (k1 - MAGIC)*2pi (exact: k1-MAGIC is Sterbenz-exact),
        # thr = th - k2 in [-pi, pi].
        bi = G % 2
        g.op("v", lambda: nc.vector.tensor_scalar(
            K1[bi][:], TH[bi][:, :, 0:T], INV_2PI, MAGIC, op0=A.mult, op1=A.add),
            reads=(key("TH", G),), writes=(key("K1", G),))
        g.op("v", lambda: nc.vector.tensor_scalar(
            K1[bi][:], K1[bi][:], -MAGIC, TWO_PI, op0=A.add, op1=A.mult),
            reads=(key("K1", G),), writes=(key("K1", G),))
        g.op("v", tt(scr("thr", G)[:], TH[bi][:, :, 0:T], K1[bi][:], A.subtract),
             reads=(key("TH", G), key("K1", G)), writes=(skey("thr", G),))

    def trig(G):
        thr = scr("thr", G)
        sinf = scr("sinf", G)
        cosf = scr("cosf", G)
        g.op("a", act(sinf[:], thr[:], AF.Sin),
             reads=(skey("thr", G),), writes=(skey("sinf", G),))
        g.op("a", act(cosf[:], thr[:], AF.Sin, scale=0.5),
             reads=(skey("thr", G),), writes=(skey("cosf", G),))
        g.op("a", act(cosf[:], cosf[:], AF.Square),
             reads=(skey("cosf", G),), writes=(skey("cosf", G),))
        g.op("a", act(cosf[:], cosf[:], AF.Identity, bias=1.0, scale=-2.0),
             reads=(skey("cosf", G),), writes=(skey("cosf", G),))

    def tail(G):
        bi = G % 2
        vdt = scr("vdt", G)
        g.op("v", tt(vdt[:], Vt[bi][:, :, 0:T], dtz[bi][:], A.mult),
             reads=(key("V", G), key("dtz", G)), writes=(skey("vdt", G),))
        g.op("v", tt(GXY[bi][:, 2:4, 1:T + 1], vdt[:], scr("sinf", G)[:], A.mult),
             reads=(skey("vdt", G), skey("sinf", G)), writes=(key("GXY", G),))
        g.op("v", tt(GXY[bi][:, 0:2, 1:T + 1], vdt[:], scr("cosf", G)[:], A.mult),
             reads=(skey("vdt", G), skey("cosf", G)), writes=(key("GXY", G),))
        g.op("v", scan(flat(XY[bi]), flat(ONB), flat(GXY[bi])),
             reads=("ONB", key("GXY", G)), writes=(key("XY", G),))

    def dma_out(G):
        s = G % NSL
        bi = G % 2
        g.op("s", lambda: nc.sync.dma_start(OUT[s, 0], XY[bi][:, 0:2, 1:T + 1]),
             reads=(key("XY", G),), writes=(), slot=s * 6 + 3)
        g.op("s", lambda: nc.sync.dma_start(OUT[s, 1], XY[bi][:, 2:4, 1:T + 1]),
             reads=(key("XY", G),), writes=(), slot=s * 6 + 4)
        g.op("s", lambda: nc.sync.dma_start(OUT[s, 2], TH[bi][:, :, 1:T + 1]),
             reads=(key("TH", G),), writes=(), slot=s * 6 + 5)

    # ---------------- emission ----------------
    # dma+leaf are hoisted one slab ahead so Act's leaf work never blocks
    # the next slab's algebra behind trig of an older slab.
    dma_in(0)
    dma_dt(0)
    leaf_acts(0)
    for G in range(NSL):
        alg_front(G)
        lnd_r7(G)
        if G > 0:
            sweeps_a(G - 1)
        alg_mid(G)
        m_acts(G)
        if G + 1 < NSL:
            dma_in(G + 1)
            leaf_acts(G + 1)
        if G > 0:
            sweeps_b(G - 1)
            kchain(G - 1)
            trig(G - 1)
            tail(G - 1)
            dma_out(G - 1)
        if G + 1 < NSL:
            dma_dt(G + 1)
    Gl = NSL - 1
    sweeps_a(Gl)
    sweeps_b(Gl)
    kchain(Gl)
    trig(Gl)
    tail(Gl)
    dma_out(Gl)

    n_slots = NSL * 6
    sem_v = nc.alloc_semaphore()
    sem_g = nc.alloc_semaphore()
    sem_a = nc.alloc_semaphore()
    dma_sems = [nc.alloc_semaphore(f"dsem{i}") for i in range(n_slots)]
    with nc.Block() as block:
        sems = {"v": sem_v, "g": sem_g, "a": sem_a}

        @block.sync
        def _(sync):
            last = {}
            dlast = {}
            for op_eng, emit_fn, deps, ref in sch.ops:
                if op_eng != "s":
                    continue
                for dep in deps:
                    if dep[0] == "D":
                        _, slot, k = dep
                        if dlast.get(slot, 0) >= k:
                            continue
                        sync.wait_ge(dma_sems[slot], 16 * k)
                        dlast[slot] = k
                    else:
                        deng, dpos = dep
                        if deng == "s" or last.get(deng, 0) >= dpos:
                            continue
                        sync.wait_ge(sems[deng], dpos)
                        last[deng] = dpos
                emit_fn().then_inc(dma_sems[ref[1]], 16)

        @block.vector
        def _(vector):
            sch.emit("v", vector, sems, dma_sems)

        @block.gpsimd
        def _(gp):
            sch.emit("g", gp, sems, dma_sems)

        @block.scalar
        def _(scalar):
            sch.emit("a", scalar, sems, dma_sems)

    return nc


_cache = {}


def _get_nc():
    if "nc" not in _cache:
        _cache["nc"] = _build_nc()
    return _cache["nc"]


def _pack_core(z_core, mu_core, times_core):
    zt = np.ascontiguousarray(z_core.transpose(2, 1, 0))       # (5, NPC, T)
    dt = np.empty_like(times_core)
    dt[0] = 0.0
    dt[1:] = times_core[1:] - times_core[:-1]
    IN = np.ascontiguousarray(
        zt.reshape(5, NSL, P, CH, T).transpose(1, 0, 2, 3, 4)).astype(np.float16)
    DTa = np.ascontiguousarray(dt.T.reshape(NSL, P, CH, T)).astype(np.float16)
    MU = np.ascontiguousarray(
        mu_core.reshape(NSL, P, CH, 5).transpose(0, 1, 3, 2))  # (NSL,P,5,CH)
    return {"inp": IN, "mu": MU, "dt": DTa}


def kernel(z_and_L_hat, mu0, times):
    z_and_L_hat = np.asarray(z_and_L_hat, dtype=np.float32)
    mu0 = np.asarray(mu0, dtype=np.float32)
    times = np.asarray(times, dtype=np.float32)
    nc = _get_nc()
    in_maps = []
    for k in range(N_CORES):
        sl = slice(k * NPC, (k + 1) * NPC)
        in_maps.append(_pack_core(z_and_L_hat[:, sl, :], mu0[sl], times[:, sl]))
    res = run_bass_kernel_spmd(nc, in_maps, core_ids=list(range(N_CORES)))
    out = np.empty((T, N_TOT, 3), np.float32)
    for k in range(N_CORES):
        O = res.results[k]["out"]                 # (NSL, 3, P, CH, T)
        planes = O.transpose(1, 0, 2, 3, 4).reshape(3, NPC, T)
        sl = slice(k * NPC, (k + 1) * NPC)
        out[:, sl, 0] = planes[0].T
        out[:, sl, 1] = planes[1].T
        out[:, sl, 2] = planes[2].T
    return out


# revision 9
# speedup vs baseline: 2.1045x; 1.2616x over previous
"""Trainium2 Bass kernel for nn_KalmanFilter: EKF over T=512 steps, N=8192 chains.

Mathematical reduction (verified exact vs the reference):
  With C = [[0,0,0,1,0],[0,0,0,0,1]], rows 3,4 of the Jacobian A are zero, so
  S = I + R depends only on per-step measurement params and the covariance
  never influences the output. Per chain:
    S = I + L L^T,  L = [[e^l0, 0], [l1, e^l2]]
    u_{t+1} = (I - S^-1) u_t + S^-1 z_t          (u = [v, omega])
    th_{t+1} = th_t + omega_t * dt_t
    x_{t+1}  = x_t + v_t * dt_t * cos(th_t)
    y_{t+1}  = y_t + v_t * dt_t * sin(th_t)
  The coupled 2-state linear recurrence is solved with 2 Gauss-Seidel sweeps
  of hardware affine scans (error contracts ~10x/sweep; end-to-end rel err
  ~3e-3 incl. fp16 quantization, vs the 2e-2 gate).

Implementation notes (from microbenchmarks on this part):
  - GpSimd shares SBUF ports with DVE and degrades it ~4x when running big
    ops: all full-size elementwise work runs on DVE (fp16 packed 2x mode,
    ~0.7us/1024el) + Act engine (function passes ~1.1us). GpSimd only does
    tiny per-slab init-column copies.
  - Scans are DVE-only, ~2.2ns/el, dtype-insensitive. Both chains per
    partition are covered by ONE scan via a zero-multiplier column at each
    chain start (which also injects the init value).
  - All intermediates fp16; t1, t2, s01, s11 carry a 2^-7 (det 2^-14)
    exponent scale folded into Act scale/bias so fp16 never overflows;
    r7 = 2^7/det compensates exactly.
  - x,y are produced and DMA'd as fp16 (host upcasts); th stays fp32 since
    range reduction needs it.
Sharding: data-parallel over chains, 1024 chains per core across 8 cores.
"""
import sys
sys.path.insert(0, '/opt/trn_rl_repo')
import numpy as np
import concourse.bass as bass
from concourse import mybir
from concourse.bass_utils import run_bass_kernel_spmd

F32 = mybir.dt.float32
F16 = mybir.dt.float16
AF = mybir.ActivationFunctionType
A = mybir.AluOpType

N_CORES = 8
T = 512
N_TOT = 8192
NPC = N_TOT // N_CORES          # 1024 chains per core
P = 128                         # partitions
NSL = 4                         # slabs per core
CH = NPC // (NSL * P)           # chains per partition per slab = 2
TP = T + 2                      # padded per-chain row: [init | T data | pad]
MAGIC = float(1.5 * 2 ** 23)
TWO_PI = float(2 * np.pi)
INV_2PI = float(1.0 / (2 * np.pi))
LN2x7 = float(7 * np.log(2.0))
HS = float(2.0 ** -3.5)         # Square scale for l1^2 * 2^-7
EPS7 = float(2.0 ** -7)


class _Sched:
    """Two-phase scheduler: record ops (engine, emit closure, deps), then emit
    per-engine in-order streams with standalone wait_ge for cross-engine deps."""

    def __init__(self):
        self.ops = []
        self.count = {"v": 0, "g": 0, "a": 0}
        self.slot_count = {}

    def add(self, eng, emit_fn, deps=(), slot=None):
        if eng == "s":
            self.slot_count[slot] = self.slot_count.get(slot, 0) + 1
            ref = ("D", slot, self.slot_count[slot])
        else:
            self.count[eng] += 1
            ref = (eng, self.count[eng])
        self.ops.append((eng, emit_fn, tuple(d for d in deps if d), ref))
        return ref

    def emit(self, eng, raw_eng, sems, dma_sems):
        last = {}
        dlast = {}
        for op_eng, emit_fn, deps, ref in self.ops:
            if op_eng != eng:
                continue
            for dep in deps:
                if dep[0] == "D":
                    _, slot, k = dep
                    if dlast.get(slot, 0) >= k:
                        continue
                    raw_eng.wait_ge(dma_sems[slot], 16 * k)
                    dlast[slot] = k
                else:
                    deng, dpos = dep
                    if deng == eng or last.get(deng, 0) >= dpos:
                        continue
                    raw_eng.wait_ge(sems[deng], dpos)
                    last[deng] = dpos
            emit_fn().then_inc(sems[eng], 1)


class _Graph:
    """Auto RAW/WAR/WAW dependency tracking over named tiles."""

    def __init__(self, sch):
        self.sch = sch
        self.w = {}
        self.r = {}

    def op(self, eng, emit_fn, reads=(), writes=(), slot=None, extra=()):
        deps = {}

        def add(ref):
            if ref is None:
                return
            key = ref[0] if ref[0] != "D" else ("D", ref[1])
            cur = deps.get(key)
            if cur is None or ref[-1] > cur[-1]:
                deps[key] = ref

        for t in reads:
            add(self.w.get(t))
        for t in writes:
            add(self.w.get(t))
            for rr in self.r.get(t, ()):
                add(rr)
        for e in extra:
            add(e)
        ref = self.sch.add(eng, emit_fn, deps=tuple(deps.values()), slot=slot)
        for t in reads:
            self.r.setdefault(t, []).append(ref)
        for t in writes:
            self.w[t] = ref
            self.r[t] = []
        return ref


def _build_nc():
    nc = bass.Bass()
    # Register activation bias constants (bass converts float biases of
    # non-Copy activations to const APs, which must pre-exist).
    for val in (-LN2x7, EPS7):
        t = nc.alloc_sbuf_tensor(f"constf32-{val}", [128, 1], F32)
        nc.gpsimd.memset(t.ap(), val)
        nc.const_aps.aps[(F32, val)] = t.ap()
    nc.all_engine_barrier()
    IN = nc.dram_tensor("inp", [NSL, 5, P, CH, T], F16, kind="ExternalInput")
    DT = nc.dram_tensor("dt", [NSL, P, CH, T], F16, kind="ExternalInput")
    MU = nc.dram_tensor("mu", [NSL, P, 5, CH], F32, kind="ExternalInput")
    OXY = nc.dram_tensor("oxy", [NSL, 2, P, CH, T], F16, kind="ExternalOutput")
    OTH = nc.dram_tensor("oth", [NSL, P, CH, T], F32, kind="ExternalOutput")

    _names = [0]

    def tile(shape, dt=F16):
        _names[0] += 1
        return nc.alloc_sbuf_tensor(f"tl{_names[0]}", list(shape), dt).ap()

    def flat(ap):
        return ap.rearrange('p a b -> p (a b)')

    # constants: fp16 ones with 0 at col0 of each chain row
    ONB = tile([P, 4, TP])

    # per-parity tiles
    zin = [tile([P, 5, CH, T]) for _ in range(2)]
    dtz = [tile([P, CH, T]) for _ in range(2)]
    mu = [tile([P, 5, CH], F32) for _ in range(2)]
    M0x = [tile([P, CH, TP]) for _ in range(2)]
    M1x = [tile([P, CH, TP]) for _ in range(2)]
    B0x = [tile([P, CH, TP]) for _ in range(2)]
    CWx = [tile([P, CH, TP]) for _ in range(2)]
    CVx = [tile([P, CH, TP]) for _ in range(2)]
    Gx = [tile([P, CH, TP]) for _ in range(2)]
    GXY = [tile([P, 4, TP]) for _ in range(2)]
    Vt = [tile([P, CH, TP]) for _ in range(2)]
    Wt = [tile([P, CH, TP]) for _ in range(2)]
    TH = [tile([P, CH, TP], F32) for _ in range(2)]
    XY = [tile([P, 4, TP]) for _ in range(2)]
    K1 = [tile([P, CH, T], F32) for _ in range(2)]
    SCR = [[tile([P, CH, T]) for _ in range(14)] for _ in range(2)]

    sch = _Sched()
    g = _Graph(sch)

    # ---- preamble ----
    g.op("v", lambda: nc.vector.memset(flat(ONB), 1.0), writes=("ONB",))
    for c in range(4):
        g.op("v", lambda c=c: nc.vector.memset(ONB[:, c, 0:1], 0.0),
             writes=("ONB",))
    for bi in range(2):
        for nm, tl in (("M0x", M0x), ("M1x", M1x), ("B0x", B0x), ("CWx", CWx),
                       ("CVx", CVx), ("Gx", Gx)):
            g.op("v", lambda tl=tl, bi=bi: nc.vector.memset(
                tl[bi][:, :, TP - 1:TP], 0.0), writes=(f"{nm}{bi}",))
        g.op("v", lambda bi=bi: nc.vector.memset(GXY[bi][:, :, TP - 1:TP], 0.0),
             writes=(f"GXY{bi}",))
        for nm, tl in (("M0x", M0x), ("M1x", M1x)):
            g.op("v", lambda tl=tl, bi=bi: nc.vector.memset(
                tl[bi][:, :, 0:1], 0.0), writes=(f"{nm}{bi}",))

    def key(nm, G):
        return f"{nm}{G % 2}"

    # scratch phys allocation: names sharing an index alias the same tile
    PHYS = {"e0sq": 0, "u0": 0,
            "e2sq": 1, "u1": 1,
            "e0h": 2,
            "l1sq": 3, "p0": 3,
            "t1h": 4, "p1": 4,
            "t2h": 5, "q0": 5,
            "d1h": 6, "lnd": 6, "b1": 6,
            "deth": 7, "q1": 7,
            "s01h": 8, "sinf": 8,
            "s11h": 9, "cosf": 9,
            "r7": 10, "tmp": 10,
            "vdt": 11, "thr": 12, "m01": 13}

    def scr(nm, G):
        return SCR[G % 2][PHYS[nm]]

    def skey(nm, G):
        return f"S{PHYS[nm]}_{G % 2}"

    # ---------------- stages ----------------
    def dma_in(G):
        s = G % NSL
        bi = G % 2
        g.op("s", lambda: nc.sync.dma_start(
            zin[bi][:, 2:5], IN[s, 2:5].rearrange("k p c t -> p k c t")),
            writes=(key("zinL", G),), slot=s * 7 + 0)
        g.op("s", lambda: nc.sync.dma_start(
            zin[bi][:, 0:2], IN[s, 0:2].rearrange("k p c t -> p k c t")),
            writes=(key("zinZ", G),), slot=s * 7 + 1)
        g.op("s", lambda: nc.sync.dma_start(mu[bi][:], MU[s]),
            writes=(key("mu", G),), slot=s * 7 + 2)

    def dma_dt(G):
        s = G % NSL
        bi = G % 2
        g.op("s", lambda: nc.sync.dma_start(dtz[bi][:], DT[s]),
            writes=(key("dtz", G),), slot=s * 7 + 3)

    def act(out_ap, in_ap, func, bias=0.0, scale=1.0):
        return lambda: nc.scalar.activation(out_ap, in_ap, func,
                                            bias=bias, scale=scale)

    def leaf_acts(G):
        bi = G % 2
        z = zin[bi]
        l0 = z[:, 2]; l1 = z[:, 3]; l2 = z[:, 4]
        zk = key("zinL", G)
        g.op("a", act(scr("e0sq", G)[:], l0, AF.Exp, bias=-LN2x7, scale=2.0),
             reads=(zk,), writes=(skey("e0sq", G),))
        g.op("a", act(scr("e2sq", G)[:], l2, AF.Exp, bias=-LN2x7, scale=2.0),
             reads=(zk,), writes=(skey("e2sq", G),))
        g.op("a", act(scr("e0h", G)[:], l0, AF.Exp, bias=-LN2x7),
             reads=(zk,), writes=(skey("e0h", G),))
        g.op("a", act(scr("l1sq", G)[:], l1, AF.Square, scale=HS),
             reads=(zk,), writes=(skey("l1sq", G),))
        g.op("a", act(scr("t1h", G)[:], scr("e0sq", G)[:], AF.Identity, bias=EPS7),
             reads=(skey("e0sq", G),), writes=(skey("t1h", G),))
        g.op("a", act(scr("t2h", G)[:], scr("e2sq", G)[:], AF.Identity, bias=EPS7),
             reads=(skey("e2sq", G),), writes=(skey("t2h", G),))

    def tt(out, in0, in1, op):
        return lambda: nc.vector.tensor_tensor(out, in0, in1, op)

    def alg_front(G):
        g.op("v", tt(scr("d1h", G)[:], scr("t1h", G)[:], scr("t2h", G)[:], A.mult),
             reads=(skey("t1h", G), skey("t2h", G)), writes=(skey("d1h", G),))
        g.op("v", tt(scr("s01h", G)[:], scr("e0h", G)[:], zin[G % 2][:, 3], A.mult),
             reads=(skey("e0h", G), key("zinL", G)), writes=(skey("s01h", G),))
        g.op("v", tt(scr("s11h", G)[:], scr("t2h", G)[:], scr("l1sq", G)[:], A.add),
             reads=(skey("t2h", G), skey("l1sq", G)), writes=(skey("s11h", G),))
        g.op("v", lambda G=G: nc.vector.scalar_tensor_tensor(
            scr("deth", G)[:], scr("l1sq", G)[:], EPS7, scr("d1h", G)[:],
            A.mult, A.add),
            reads=(skey("l1sq", G), skey("d1h", G)), writes=(skey("deth", G),))

    def lnd_r7(G):
        g.op("a", act(scr("lnd", G)[:], scr("deth", G)[:], AF.Ln),
             reads=(skey("deth", G),), writes=(skey("lnd", G),))
        g.op("a", act(scr("r7", G)[:], scr("lnd", G)[:], AF.Exp,
                      bias=-LN2x7, scale=-1.0),
             reads=(skey("lnd", G),), writes=(skey("r7", G),))

    def alg_mid(G):
        bi = G % 2
        z = zin[bi]
        z0 = z[:, 0]; z1 = z[:, 1]
        zk = key("zinZ", G)
        g.op("v", tt(scr("u0", G)[:], scr("s11h", G)[:], scr("r7", G)[:], A.mult),
             reads=(skey("s11h", G), skey("r7", G)), writes=(skey("u0", G),))
        g.op("v", tt(scr("u1", G)[:], scr("t1h", G)[:], scr("r7", G)[:], A.mult),
             reads=(skey("t1h", G), skey("r7", G)), writes=(skey("u1", G),))
        # m00/m11 on DVE (fp16 TS, 2x packed) right where they're produced
        g.op("v", lambda bi=bi, G=G: nc.vector.tensor_scalar(
            M0x[bi][:, :, 1:T + 1], scr("u0", G)[:], -1.0, 1.0,
            op0=A.mult, op1=A.add),
            reads=(skey("u0", G),), writes=(key("M0x", G),))
        g.op("v", lambda bi=bi, G=G: nc.vector.tensor_scalar(
            M1x[bi][:, :, 1:T + 1], scr("u1", G)[:], -1.0, 1.0,
            op0=A.mult, op1=A.add),
            reads=(skey("u1", G),), writes=(key("M1x", G),))
        g.op("v", tt(scr("m01", G)[:], scr("s01h", G)[:], scr("r7", G)[:], A.mult),
             reads=(skey("s01h", G), skey("r7", G)), writes=(skey("m01", G),))
        g.op("v", tt(scr("p0", G)[:], scr("u0", G)[:], z0, A.mult),
             reads=(skey("u0", G), zk), writes=(skey("p0", G),))
        g.op("v", tt(scr("q0", G)[:], scr("m01", G)[:], z1, A.mult),
             reads=(skey("m01", G), zk), writes=(skey("q0", G),))
        g.op("v", tt(B0x[bi][:, :, 1:T + 1], scr("p0", G)[:], scr("q0", G)[:],
                     A.subtract),
             reads=(skey("p0", G), skey("q0", G)), writes=(key("B0x", G),))
        g.op("v", tt(scr("p1", G)[:], scr("u1", G)[:], z1, A.mult),
             reads=(skey("u1", G), zk), writes=(skey("p1", G),))
        g.op("v", tt(scr("q1", G)[:], scr("m01", G)[:], z0, A.mult),
             reads=(skey("m01", G), zk), writes=(skey("q1", G),))
        g.op("v", tt(scr("b1", G)[:], scr("p1", G)[:], scr("q1", G)[:], A.subtract),
             reads=(skey("p1", G), skey("q1", G)), writes=(skey("b1", G),))

    def col_inits(G):
        # tiny [P,CH] init-column copies; on GpSimd (idle, negligible size)
        bi = G % 2
        m = mu[bi]
        mk = key("mu", G)
        for dst_ap, dst_key, mi in (
                (B0x[bi][:, :, 0], "B0x", 3), (CVx[bi][:, :, 0], "CVx", 3),
                (CWx[bi][:, :, 0], "CWx", 4), (Gx[bi][:, :, 0], "Gx", 2),
                (GXY[bi][:, 0:2, 0], "GXY", 0), (GXY[bi][:, 2:4, 0], "GXY", 1)):
            g.op("g", (lambda dst_ap=dst_ap, m=m, mi=mi:
                       nc.gpsimd.tensor_copy(dst_ap, m[:, mi])),
                 reads=(mk,), writes=(key(dst_key, G),))

    def scan(out, d0, d1):
        return lambda: nc.vector.tensor_tensor_scan(out, d0, d1, 0.0,
                                                    A.mult, A.add)

    def sweeps_a(G):
        bi = G % 2
        m01 = scr("m01", G)
        tmp = scr("tmp", G)
        b1 = scr("b1", G)
        g.op("v", scan(flat(Vt[bi]), flat(M0x[bi]), flat(B0x[bi])),
             reads=(key("M0x", G), key("B0x", G)), writes=(key("V", G),))
        g.op("v", tt(tmp[:], m01[:], Vt[bi][:, :, 0:T], A.mult),
             reads=(skey("m01", G), key("V", G)), writes=(skey("tmp", G),))
        g.op("v", tt(CWx[bi][:, :, 1:T + 1], tmp[:], b1[:], A.add),
             reads=(skey("tmp", G), skey("b1", G)), writes=(key("CWx", G),))
        g.op("v", scan(flat(Wt[bi]), flat(M1x[bi]), flat(CWx[bi])),
             reads=(key("M1x", G), key("CWx", G)), writes=(key("W", G),))

    def sweeps_b(G):
        bi = G % 2
        m01 = scr("m01", G)
        tmp = scr("tmp", G)
        b1 = scr("b1", G)
        g.op("v", tt(tmp[:], m01[:], Wt[bi][:, :, 0:T], A.mult),
             reads=(skey("m01", G), key("W", G)), writes=(skey("tmp", G),))
        g.op("v", tt(CVx[bi][:, :, 1:T + 1], tmp[:], B0x[bi][:, :, 1:T + 1], A.add),
             reads=(skey("tmp", G), key("B0x", G)), writes=(key("CVx", G),))
        g.op("v", scan(flat(Vt[bi]), flat(M0x[bi]), flat(CVx[bi])),
             reads=(key("M0x", G), key("CVx", G)), writes=(key("V", G),))
        g.op("v", tt(tmp[:], m01[:], Vt[bi][:, :, 0:T], A.mult),
             reads=(skey("m01", G), key("V", G)), writes=(skey("tmp", G),))
        g.op("v", tt(CWx[bi][:, :, 1:T + 1], tmp[:], b1[:], A.add),
             reads=(skey("tmp", G), skey("b1", G)), writes=(key("CWx", G),))
        g.op("v", scan(flat(Wt[bi]), flat(M1x[bi]), flat(CWx[bi])),
             reads=(key("M1x", G), key("CWx", G)), writes=(key("W", G),))
        g.op("v", tt(Gx[bi][:, :, 1:T + 1], Wt[bi][:, :, 0:T], dtz[bi][:],
                     A.mult),
             reads=(key("W", G), key("dtz", G)), writes=(key("Gx", G),))
        g.op("v", scan(flat(TH[bi]), flat(ONB[:, 0:2]), flat(Gx[bi])),
             reads=("ONB", key("Gx", G)), writes=(key("TH", G),))

    def kchain(G):
        # k1 = th/2pi + MAGIC (round-to-int trick), k2 = (k1-MAGIC)*2pi
        # (exact: k1-MAGIC is Sterbenz-exact), thr = th - k2 in [-pi, pi].
        bi = G % 2
        g.op("v", lambda bi=bi: nc.vector.tensor_scalar(
            K1[bi][:], TH[bi][:, :, 0:T], INV_2PI, MAGIC, op0=A.mult, op1=A.add),
            reads=(key("TH", G),), writes=(key("K1", G),))
        g.op("v", lambda bi=bi: nc.vector.tensor_scalar(
            K1[bi][:], K1[bi][:], -MAGIC, TWO_PI, op0=A.add, op1=A.mult),
            reads=(key("K1", G),), writes=(key("K1", G),))
        g.op("v", tt(scr("thr", G)[:], TH[bi][:, :, 0:T], K1[bi][:], A.subtract),
             reads=(key("TH", G), key("K1", G)), writes=(skey("thr", G),))

    def trig(G):
        thr = scr("thr", G)
        g.op("a", act(scr("sinf", G)[:], thr[:], AF.Sin),
             reads=(skey("thr", G),), writes=(skey("sinf", G),))
        g.op("a", act(scr("cosf", G)[:], thr[:], AF.Sin, scale=0.5),
             reads=(skey("thr", G),), writes=(skey("cosf", G),))

    def tail(G):
        bi = G % 2
        vdt = scr("vdt", G)
        cosf = scr("cosf", G)
        g.op("v", tt(vdt[:], Vt[bi][:, :, 0:T], dtz[bi][:], A.mult),
             reads=(key("V", G), key("dtz", G)), writes=(skey("vdt", G),))
        g.op("v", tt(GXY[bi][:, 2:4, 1:T + 1], vdt[:], scr("sinf", G)[:], A.mult),
             reads=(skey("vdt", G), skey("sinf", G)), writes=(key("GXY", G),))
        # cos(th) = 1 - 2*sin(th/2)^2, square+affine on DVE
        g.op("v", tt(cosf[:], cosf[:], cosf[:], A.mult),
             reads=(skey("cosf", G),), writes=(skey("cosf", G),))
        g.op("v", lambda G=G: nc.vector.tensor_scalar(
            cosf[:], cosf[:], -2.0, 1.0, op0=A.mult, op1=A.add),
            reads=(skey("cosf", G),), writes=(skey("cosf", G),))
        g.op("v", tt(GXY[bi][:, 0:2, 1:T + 1], vdt[:], cosf[:], A.mult),
             reads=(skey("vdt", G), skey("cosf", G)), writes=(key("GXY", G),))
        g.op("v", scan(flat(XY[bi]), flat(ONB), flat(GXY[bi])),
             reads=("ONB", key("GXY", G)), writes=(key("XY", G),))

    def dma_out(G):
        s = G % NSL
        bi = G % 2
        g.op("s", lambda: nc.sync.dma_start(OXY[s, 0], XY[bi][:, 0:2, 1:T + 1]),
             reads=(key("XY", G),), slot=s * 7 + 4)
        g.op("s", lambda: nc.sync.dma_start(OXY[s, 1], XY[bi][:, 2:4, 1:T + 1]),
             reads=(key("XY", G),), slot=s * 7 + 5)
        g.op("s", lambda: nc.sync.dma_start(OTH[s], TH[bi][:, :, 1:T + 1]),
             reads=(key("TH", G),), slot=s * 7 + 6)

    # ---------------- emission ----------------
    # dma+leaf are hoisted one slab ahead so Act's leaf work never blocks
    # the next slab's algebra behind trig of an older slab.
    dma_in(0)
    dma_dt(0)
    leaf_acts(0)
    for G in range(NSL):
        alg_front(G)
        lnd_r7(G)
        if G > 0:
            sweeps_a(G - 1)
        alg_mid(G)
        col_inits(G)
        if G + 1 < NSL:
            dma_in(G + 1)
            leaf_acts(G + 1)
        if G > 0:
            sweeps_b(G - 1)
            kchain(G - 1)
            trig(G - 1)
            tail(G - 1)
            dma_out(G - 1)
        if G + 1 < NSL:
            dma_dt(G + 1)
    Gl = NSL - 1
    sweeps_a(Gl)
    sweeps_b(Gl)
    kchain(Gl)
    trig(Gl)
    tail(Gl)
    dma_out(Gl)

    n_slots = NSL * 7
    sem_v = nc.alloc_semaphore()
    sem_g = nc.alloc_semaphore()
    sem_a = nc.alloc_semaphore()
    dma_sems = [nc.alloc_semaphore(f"dsem{i}") for i in range(n_slots)]
    with nc.Block() as block:
        sems = {"v": sem_v, "g": sem_g, "a": sem_a}

        @block.sync
        def _(sync):
            last = {}
            dlast = {}
            for op_eng, emit_fn, deps, ref in sch.ops:
                if op_eng != "s":
                    continue
                for dep in deps:
                    if dep[0] == "D":
                        _, slot, k = dep
                        if dlast.get(slot, 0) >= k:
                            continue
                        sync.wait_ge(dma_sems[slot], 16 * k)
                        dlast[slot] = k
                    else:
                        deng, dpos = dep
                        if deng == "s" or last.get(deng, 0) >= dpos:
                            continue
                        sync.wait_ge(sems[deng], dpos)
                        last[deng] = dpos
                emit_fn().then_inc(dma_sems[ref[1]], 16)

        @block.vector
        def _(vector):
            sch.emit("v", vector, sems, dma_sems)

        @block.gpsimd
        def _(gp):
            sch.emit("g", gp, sems, dma_sems)

        @block.scalar
        def _(scalar):
            sch.emit("a", scalar, sems, dma_sems)

    return nc


_cache = {}


def _get_nc():
    if "nc" not in _cache:
        _cache["nc"] = _build_nc()
    return _cache["nc"]


def _pack_core(z_core, mu_core, times_core):
    zt = np.ascontiguousarray(z_core.transpose(2, 1, 0))       # (5, NPC, T)
    dt = np.empty_like(times_core)
    dt[0] = 0.0
    dt[1:] = times_core[1:] - times_core[:-1]
    IN = np.ascontiguousarray(
        zt.reshape(5, NSL, P, CH, T).transpose(1, 0, 2, 3, 4)).astype(np.float16)
    DTa = np.ascontiguousarray(dt.T.reshape(NSL, P, CH, T)).astype(np.float16)
    MU = np.ascontiguousarray(
        mu_core.reshape(NSL, P, CH, 5).transpose(0, 1, 3, 2))  # (NSL,P,5,CH)
    return {"inp": IN, "mu": MU, "dt": DTa}


def kernel(z_and_L_hat, mu0, times):
    z_and_L_hat = np.asarray(z_and_L_hat, dtype=np.float32)
    mu0 = np.asarray(mu0, dtype=np.float32)
    times = np.asarray(times, dtype=np.float32)
    nc = _get_nc()
    in_maps = []
    for k in range(N_CORES):
        sl = slice(k * NPC, (k + 1) * NPC)
        in_maps.append(_pack_core(z_and_L_hat[:, sl, :], mu0[sl], times[:, sl]))
    res = run_bass_kernel_spmd(nc, in_maps, core_ids=list(range(N_CORES)))
    out = np.empty((T, N_TOT, 3), np.float32)
    for k in range(N_CORES):
        oxy = res.results[k]["oxy"]               # (NSL, 2, P, CH, T) f16
        oth = res.results[k]["oth"]               # (NSL, P, CH, T) f32
        sl = slice(k * NPC, (k + 1) * NPC)
        out[:, sl, 0] = oxy[:, 0].astype(np.float32).reshape(NPC, T).T
        out[:, sl, 1] = oxy[:, 1].astype(np.float32).reshape(NPC, T).T
        out[:, sl, 2] = oth.astype(np.float32).reshape(NPC, T).T
    return out


# revision 10
# speedup vs baseline: 2.1116x; 1.0034x over previous
"""Trainium2 Bass kernel for nn_KalmanFilter: EKF over T=512 steps, N=8192 chains.

Mathematical reduction (verified exact vs the reference):
  With C = [[0,0,0,1,0],[0,0,0,0,1]], rows 3,4 of the Jacobian A are zero, so
  S = I + R depends only on per-step measurement params and the covariance
  never influences the output. Per chain:
    S = I + L L^T,  L = [[e^l0, 0], [l1, e^l2]]
    u_{t+1} = (I - S^-1) u_t + S^-1 z_t          (u = [v, omega])
    th_{t+1} = th_t + omega_t * dt_t
    x_{t+1}  = x_t + v_t * dt_t * cos(th_t)
    y_{t+1}  = y_t + v_t * dt_t * sin(th_t)
  The coupled 2-state linear recurrence is solved with 2 Gauss-Seidel sweeps
  of hardware affine scans (error contracts ~10x/sweep; end-to-end rel err
  ~3e-3 incl. fp16 quantization, vs the 2e-2 gate).

Implementation notes (from microbenchmarks on this part):
  - GpSimd shares SBUF ports with DVE and degrades it ~4x when running big
    ops: all full-size elementwise work runs on DVE (fp16 packed 2x mode,
    ~0.7us/1024el) + Act engine (function passes ~1.1us). GpSimd only does
    tiny per-slab init-column copies.
  - Scans are DVE-only, ~2.2ns/el, dtype-insensitive. Both chains per
    partition are covered by ONE scan via a zero-multiplier column at each
    chain start (which also injects the init value).
  - All intermediates fp16; t1, t2, s01, s11 carry a 2^-7 (det 2^-14)
    exponent scale folded into Act scale/bias so fp16 never overflows;
    r7 = 2^7/det compensates exactly.
  - x,y are produced and DMA'd as fp16 (host upcasts); th stays fp32 since
    range reduction needs it.
Sharding: data-parallel over chains, 1024 chains per core across 8 cores.
"""
import sys
sys.path.insert(0, '/opt/trn_rl_repo')
import numpy as np
import concourse.bass as bass
from concourse import mybir
from concourse.bass_utils import run_bass_kernel_spmd

F32 = mybir.dt.float32
F16 = mybir.dt.float16
AF = mybir.ActivationFunctionType
A = mybir.AluOpType

N_CORES = 8
T = 512
N_TOT = 8192
NPC = N_TOT // N_CORES          # 1024 chains per core
P = 128                         # partitions
NSL = 4                         # slabs per core
CH = NPC // (NSL * P)           # chains per partition per slab = 2
TP = T + 2                      # padded per-chain row: [init | T data | pad]
MAGIC = float(1.5 * 2 ** 23)
TWO_PI = float(2 * np.pi)
INV_2PI = float(1.0 / (2 * np.pi))
LN2x7 = float(7 * np.log(2.0))
HS = float(2.0 ** -3.5)         # Square scale for l1^2 * 2^-7
EPS7 = float(2.0 ** -7)


class _Sched:
    """Two-phase scheduler: record ops (engine, emit closure, deps), then emit
    per-engine in-order streams with standalone wait_ge for cross-engine deps."""

    def __init__(self):
        self.ops = []
        self.count = {"v": 0, "g": 0, "a": 0}
        self.slot_count = {}

    def add(self, eng, emit_fn, deps=(), slot=None):
        if eng == "s":
            self.slot_count[slot] = self.slot_count.get(slot, 0) + 1
            ref = ("D", slot, self.slot_count[slot])
        else:
            self.count[eng] += 1
            ref = (eng, self.count[eng])
        self.ops.append((eng, emit_fn, tuple(d for d in deps if d), ref))
        return ref

    def emit(self, eng, raw_eng, sems, dma_sems):
        last = {}
        dlast = {}
        for op_eng, emit_fn, deps, ref in self.ops:
            if op_eng != eng:
                continue
            for dep in deps:
                if dep[0] == "D":
                    _, slot, k = dep
                    if dlast.get(slot, 0) >= k:
                        continue
                    raw_eng.wait_ge(dma_sems[slot], 16 * k)
                    dlast[slot] = k
                else:
                    deng, dpos = dep
                    if deng == eng or last.get(deng, 0) >= dpos:
                        continue
                    raw_eng.wait_ge(sems[deng], dpos)
                    last[deng] = dpos
            emit_fn().then_inc(sems[eng], 1)


class _Graph:
    """Auto RAW/WAR/WAW dependency tracking over named tiles."""

    def __init__(self, sch):
        self.sch = sch
        self.w = {}
        self.r = {}

    def op(self, eng, emit_fn, reads=(), writes=(), slot=None, extra=()):
        deps = {}

        def add(ref):
            if ref is None:
                return
            key = ref[0] if ref[0] != "D" else ("D", ref[1])
            cur = deps.get(key)
            if cur is None or ref[-1] > cur[-1]:
                deps[key] = ref

        for t in reads:
            add(self.w.get(t))
        for t in writes:
            add(self.w.get(t))
            for rr in self.r.get(t, ()):
                add(rr)
        for e in extra:
            add(e)
        ref = self.sch.add(eng, emit_fn, deps=tuple(deps.values()), slot=slot)
        for t in reads:
            self.r.setdefault(t, []).append(ref)
        for t in writes:
            self.w[t] = ref
            self.r[t] = []
        return ref


def _build_nc():
    nc = bass.Bass()
    # Register activation bias constants (bass converts float biases of
    # non-Copy activations to const APs, which must pre-exist).
    for val in (-LN2x7, EPS7):
        t = nc.alloc_sbuf_tensor(f"constf32-{val}", [128, 1], F32)
        nc.gpsimd.memset(t.ap(), val)
        nc.const_aps.aps[(F32, val)] = t.ap()
    nc.all_engine_barrier()
    IN = nc.dram_tensor("inp", [NSL, 5, P, CH, T], F16, kind="ExternalInput")
    DT = nc.dram_tensor("dt", [NSL, P, CH, T], F16, kind="ExternalInput")
    MU = nc.dram_tensor("mu", [NSL, P, 5, CH], F32, kind="ExternalInput")
    OXY = nc.dram_tensor("oxy", [NSL, 2, P, CH, T], F16, kind="ExternalOutput")
    OTH = nc.dram_tensor("oth", [NSL, P, CH, T], F32, kind="ExternalOutput")

    _names = [0]

    def tile(shape, dt=F16):
        _names[0] += 1
        return nc.alloc_sbuf_tensor(f"tl{_names[0]}", list(shape), dt).ap()

    def flat(ap):
        return ap.rearrange('p a b -> p (a b)')

    # constants: fp16 ones with 0 at col0 of each chain row
    ONB = tile([P, 4, TP])

    # per-parity tiles
    zin = [tile([P, 5, CH, T]) for _ in range(2)]
    dtz = [tile([P, CH, T]) for _ in range(2)]
    mu = [tile([P, 5, CH], F32) for _ in range(2)]
    M0x = [tile([P, CH, TP]) for _ in range(2)]
    M1x = [tile([P, CH, TP]) for _ in range(2)]
    B0x = [tile([P, CH, TP]) for _ in range(2)]
    CWx = [tile([P, CH, TP]) for _ in range(2)]
    CVx = [tile([P, CH, TP]) for _ in range(2)]
    Gx = [tile([P, CH, TP]) for _ in range(2)]
    GXY = [tile([P, 4, TP]) for _ in range(2)]
    Vt = [tile([P, CH, TP]) for _ in range(2)]
    Wt = [tile([P, CH, TP]) for _ in range(2)]
    TH = [tile([P, CH, TP], F32) for _ in range(2)]
    XY = [tile([P, 4, TP]) for _ in range(2)]
    K1 = [tile([P, CH, T], F32) for _ in range(2)]
    SCR = [[tile([P, CH, T]) for _ in range(15)] for _ in range(2)]

    sch = _Sched()
    g = _Graph(sch)

    # ---- preamble ----
    g.op("v", lambda: nc.vector.memset(flat(ONB), 1.0), writes=("ONB",))
    for c in range(4):
        g.op("v", lambda c=c: nc.vector.memset(ONB[:, c, 0:1], 0.0),
             writes=("ONB",))
    for bi in range(2):
        for nm, tl in (("M0x", M0x), ("M1x", M1x), ("B0x", B0x), ("CWx", CWx),
                       ("CVx", CVx), ("Gx", Gx)):
            g.op("v", lambda tl=tl, bi=bi: nc.vector.memset(
                tl[bi][:, :, TP - 1:TP], 0.0), writes=(f"{nm}{bi}",))
        g.op("v", lambda bi=bi: nc.vector.memset(GXY[bi][:, :, TP - 1:TP], 0.0),
             writes=(f"GXY{bi}",))
        for nm, tl in (("M0x", M0x), ("M1x", M1x)):
            g.op("v", lambda tl=tl, bi=bi: nc.vector.memset(
                tl[bi][:, :, 0:1], 0.0), writes=(f"{nm}{bi}",))

    def key(nm, G):
        return f"{nm}{G % 2}"

    # scratch phys allocation: names sharing an index alias the same tile
    PHYS = {"e0sq": 0, "u0": 0,
            "e2sq": 1, "u1": 1,
            "e0h": 2,
            "l1sq": 3, "p0": 3,
            "t1h": 4, "p1": 4,
            "t2h": 5, "q0": 5,
            "d1h": 6, "lnd": 6, "b1": 6,
            "deth": 7, "q1": 7,
            "s01h": 8, "sinf": 8,
            "s11h": 9, "cosf": 9,
            "r7": 10, "tmp": 10,
            "vdt": 11, "thr": 12, "m01": 13, "l1sq14": 14}

    def scr(nm, G):
        return SCR[G % 2][PHYS[nm]]

    def skey(nm, G):
        return f"S{PHYS[nm]}_{G % 2}"

    # ---------------- stages ----------------
    def dma_in(G):
        s = G % NSL
        bi = G % 2
        for pl in (2, 3, 4):
            g.op("s", lambda pl=pl: nc.sync.dma_start(
                zin[bi][:, pl], IN[s, pl]),
                writes=(key(f"zinL{pl}", G),), slot=s * 9 + (pl - 2))
        g.op("s", lambda: nc.sync.dma_start(
            zin[bi][:, 0:2], IN[s, 0:2].rearrange("k p c t -> p k c t")),
            writes=(key("zinZ", G),), slot=s * 9 + 3)
        g.op("s", lambda: nc.sync.dma_start(mu[bi][:], MU[s]),
            writes=(key("mu", G),), slot=s * 9 + 4)

    def dma_dt(G):
        s = G % NSL
        bi = G % 2
        g.op("s", lambda: nc.sync.dma_start(dtz[bi][:], DT[s]),
            writes=(key("dtz", G),), slot=s * 9 + 5)

    def act(out_ap, in_ap, func, bias=0.0, scale=1.0):
        return lambda: nc.scalar.activation(out_ap, in_ap, func,
                                            bias=bias, scale=scale)

    def leaf_acts_a(G):
        bi = G % 2
        z = zin[bi]
        l0 = z[:, 2]; l1 = z[:, 3]; l2 = z[:, 4]
        g.op("a", act(scr("e0sq", G)[:], l0, AF.Exp, bias=-LN2x7, scale=2.0),
             reads=(key("zinL2", G),), writes=(skey("e0sq", G),))
        g.op("a", act(scr("e2sq", G)[:], l2, AF.Exp, bias=-LN2x7, scale=2.0),
             reads=(key("zinL4", G),), writes=(skey("e2sq", G),))
        g.op("a", act(scr("e0h", G)[:], l0, AF.Exp, bias=-LN2x7),
             reads=(key("zinL2", G),), writes=(skey("e0h", G),))
        g.op("a", act(scr("l1sq", G)[:], l1, AF.Square, scale=HS),
             reads=(key("zinL3", G),), writes=(skey("l1sq", G),))
        g.op("a", act(scr("l1sq14", G)[:], l1, AF.Square, scale=EPS7),
             reads=(key("zinL3", G),), writes=(skey("l1sq14", G),))

    def leaf_acts_t(G):
        g.op("a", act(scr("t1h", G)[:], scr("e0sq", G)[:], AF.Identity, bias=EPS7),
             reads=(skey("e0sq", G),), writes=(skey("t1h", G),))
        g.op("a", act(scr("t2h", G)[:], scr("e2sq", G)[:], AF.Identity, bias=EPS7),
             reads=(skey("e2sq", G),), writes=(skey("t2h", G),))

    def tt(out, in0, in1, op):
        return lambda: nc.vector.tensor_tensor(out, in0, in1, op)

    def alg_front(G):
        g.op("v", tt(scr("d1h", G)[:], scr("t1h", G)[:], scr("t2h", G)[:], A.mult),
             reads=(skey("t1h", G), skey("t2h", G)), writes=(skey("d1h", G),))
        g.op("v", tt(scr("s01h", G)[:], scr("e0h", G)[:], zin[G % 2][:, 3], A.mult),
             reads=(skey("e0h", G), key("zinL3", G)), writes=(skey("s01h", G),))
        g.op("v", tt(scr("s11h", G)[:], scr("t2h", G)[:], scr("l1sq", G)[:], A.add),
             reads=(skey("t2h", G), skey("l1sq", G)), writes=(skey("s11h", G),))
        g.op("v", tt(scr("deth", G)[:], scr("d1h", G)[:], scr("l1sq14", G)[:],
                     A.add),
             reads=(skey("d1h", G), skey("l1sq14", G)), writes=(skey("deth", G),))

    def lnd_r7(G):
        g.op("a", act(scr("lnd", G)[:], scr("deth", G)[:], AF.Ln),
             reads=(skey("deth", G),), writes=(skey("lnd", G),))
        g.op("a", act(scr("r7", G)[:], scr("lnd", G)[:], AF.Exp,
                      bias=-LN2x7, scale=-1.0),
             reads=(skey("lnd", G),), writes=(skey("r7", G),))

    def alg_mid(G):
        bi = G % 2
        z = zin[bi]
        z0 = z[:, 0]; z1 = z[:, 1]
        zk = key("zinZ", G)
        g.op("v", tt(scr("u0", G)[:], scr("s11h", G)[:], scr("r7", G)[:], A.mult),
             reads=(skey("s11h", G), skey("r7", G)), writes=(skey("u0", G),))
        g.op("v", tt(scr("u1", G)[:], scr("t1h", G)[:], scr("r7", G)[:], A.mult),
             reads=(skey("t1h", G), skey("r7", G)), writes=(skey("u1", G),))
        # m00/m11 on DVE (fp16 TS, 2x packed) right where they're produced
        g.op("v", lambda bi=bi, G=G: nc.vector.tensor_scalar(
            M0x[bi][:, :, 1:T + 1], scr("u0", G)[:], -1.0, 1.0,
            op0=A.mult, op1=A.add),
            reads=(skey("u0", G),), writes=(key("M0x", G),))
        g.op("v", lambda bi=bi, G=G: nc.vector.tensor_scalar(
            M1x[bi][:, :, 1:T + 1], scr("u1", G)[:], -1.0, 1.0,
            op0=A.mult, op1=A.add),
            reads=(skey("u1", G),), writes=(key("M1x", G),))
        g.op("v", tt(scr("m01", G)[:], scr("s01h", G)[:], scr("r7", G)[:], A.mult),
             reads=(skey("s01h", G), skey("r7", G)), writes=(skey("m01", G),))
        g.op("v", tt(scr("p0", G)[:], scr("u0", G)[:], z0, A.mult),
             reads=(skey("u0", G), zk), writes=(skey("p0", G),))
        g.op("v", tt(scr("q0", G)[:], scr("m01", G)[:], z1, A.mult),
             reads=(skey("m01", G), zk), writes=(skey("q0", G),))
        g.op("v", tt(B0x[bi][:, :, 1:T + 1], scr("p0", G)[:], scr("q0", G)[:],
                     A.subtract),
             reads=(skey("p0", G), skey("q0", G)), writes=(key("B0x", G),))
        g.op("v", tt(scr("p1", G)[:], scr("u1", G)[:], z1, A.mult),
             reads=(skey("u1", G), zk), writes=(skey("p1", G),))
        g.op("v", tt(scr("q1", G)[:], scr("m01", G)[:], z0, A.mult),
             reads=(skey("m01", G), zk), writes=(skey("q1", G),))
        g.op("v", tt(scr("b1", G)[:], scr("p1", G)[:], scr("q1", G)[:], A.subtract),
             reads=(skey("p1", G), skey("q1", G)), writes=(skey("b1", G),))

    def col_inits(G):
        # tiny [P,CH] init-column copies; on GpSimd (idle, negligible size)
        bi = G % 2
        m = mu[bi]
        mk = key("mu", G)
        for dst_ap, dst_key, mi in (
                (B0x[bi][:, :, 0], "B0x", 3), (CVx[bi][:, :, 0], "CVx", 3),
                (CWx[bi][:, :, 0], "CWx", 4), (Gx[bi][:, :, 0], "Gx", 2),
                (GXY[bi][:, 0:2, 0], "GXY", 0), (GXY[bi][:, 2:4, 0], "GXY", 1)):
            g.op("a", (lambda dst_ap=dst_ap, m=m, mi=mi:
                       nc.scalar.activation(dst_ap, m[:, mi], AF.Identity)),
                 reads=(mk,), writes=(key(dst_key, G),))

    def scan(out, d0, d1):
        return lambda: nc.vector.tensor_tensor_scan(out, d0, d1, 0.0,
                                                    A.mult, A.add)

    def sweeps_a(G):
        bi = G % 2
        m01 = scr("m01", G)
        tmp = scr("tmp", G)
        b1 = scr("b1", G)
        g.op("v", scan(flat(Vt[bi]), flat(M0x[bi]), flat(B0x[bi])),
             reads=(key("M0x", G), key("B0x", G)), writes=(key("V", G),))
        g.op("v", tt(tmp[:], m01[:], Vt[bi][:, :, 0:T], A.mult),
             reads=(skey("m01", G), key("V", G)), writes=(skey("tmp", G),))
        g.op("v", tt(CWx[bi][:, :, 1:T + 1], tmp[:], b1[:], A.add),
             reads=(skey("tmp", G), skey("b1", G)), writes=(key("CWx", G),))
        g.op("v", scan(flat(Wt[bi]), flat(M1x[bi]), flat(CWx[bi])),
             reads=(key("M1x", G), key("CWx", G)), writes=(key("W", G),))

    def sweeps_b(G):
        bi = G % 2
        m01 = scr("m01", G)
        tmp = scr("tmp", G)
        b1 = scr("b1", G)
        g.op("v", tt(tmp[:], m01[:], Wt[bi][:, :, 0:T], A.mult),
             reads=(skey("m01", G), key("W", G)), writes=(skey("tmp", G),))
        g.op("v", tt(CVx[bi][:, :, 1:T + 1], tmp[:], B0x[bi][:, :, 1:T + 1], A.add),
             reads=(skey("tmp", G), key("B0x", G)), writes=(key("CVx", G),))
        g.op("v", scan(flat(Vt[bi]), flat(M0x[bi]), flat(CVx[bi])),
             reads=(key("M0x", G), key("CVx", G)), writes=(key("V", G),))
        g.op("v", tt(tmp[:], m01[:], Vt[bi][:, :, 0:T], A.mult),
             reads=(skey("m01", G), key("V", G)), writes=(skey("tmp", G),))
        g.op("v", tt(CWx[bi][:, :, 1:T + 1], tmp[:], b1[:], A.add),
             reads=(skey("tmp", G), skey("b1", G)), writes=(key("CWx", G),))
        g.op("v", scan(flat(Wt[bi]), flat(M1x[bi]), flat(CWx[bi])),
             reads=(key("M1x", G), key("CWx", G)), writes=(key("W", G),))
        g.op("v", tt(Gx[bi][:, :, 1:T + 1], Wt[bi][:, :, 0:T], dtz[bi][:],
                     A.mult),
             reads=(key("W", G), key("dtz", G)), writes=(key("Gx", G),))
        g.op("v", scan(flat(TH[bi]), flat(ONB[:, 0:2]), flat(Gx[bi])),
             reads=("ONB", key("Gx", G)), writes=(key("TH", G),))

    def kchain(G):
        # k1 = th/2pi + MAGIC (round-to-int trick), k2 = (k1-MAGIC)*2pi
        # (exact: k1-MAGIC is Sterbenz-exact), thr = th - k2 in [-pi, pi].
        bi = G % 2
        g.op("v", lambda bi=bi: nc.vector.tensor_scalar(
            K1[bi][:], TH[bi][:, :, 0:T], INV_2PI, MAGIC, op0=A.mult, op1=A.add),
            reads=(key("TH", G),), writes=(key("K1", G),))
        g.op("v", lambda bi=bi: nc.vector.tensor_scalar(
            K1[bi][:], K1[bi][:], -MAGIC, TWO_PI, op0=A.add, op1=A.mult),
            reads=(key("K1", G),), writes=(key("K1", G),))
        g.op("v", tt(scr("thr", G)[:], TH[bi][:, :, 0:T], K1[bi][:], A.subtract),
             reads=(key("TH", G), key("K1", G)), writes=(skey("thr", G),))

    def trig(G):
        thr = scr("thr", G)
        g.op("a", act(scr("sinf", G)[:], thr[:], AF.Sin),
             reads=(skey("thr", G),), writes=(skey("sinf", G),))
        g.op("a", act(scr("cosf", G)[:], thr[:], AF.Sin, scale=0.5),
             reads=(skey("thr", G),), writes=(skey("cosf", G),))

    def tail(G):
        bi = G % 2
        vdt = scr("vdt", G)
        cosf = scr("cosf", G)
        g.op("v", tt(vdt[:], Vt[bi][:, :, 0:T], dtz[bi][:], A.mult),
             reads=(key("V", G), key("dtz", G)), writes=(skey("vdt", G),))
        g.op("v", tt(GXY[bi][:, 2:4, 1:T + 1], vdt[:], scr("sinf", G)[:], A.mult),
             reads=(skey("vdt", G), skey("sinf", G)), writes=(key("GXY", G),))
        # cos(th) = 1 - 2*sin(th/2)^2, square+affine on DVE
        g.op("v", tt(cosf[:], cosf[:], cosf[:], A.mult),
             reads=(skey("cosf", G),), writes=(skey("cosf", G),))
        g.op("v", lambda G=G: nc.vector.tensor_scalar(
            cosf[:], cosf[:], -2.0, 1.0, op0=A.mult, op1=A.add),
            reads=(skey("cosf", G),), writes=(skey("cosf", G),))
        g.op("v", tt(GXY[bi][:, 0:2, 1:T + 1], vdt[:], cosf[:], A.mult),
             reads=(skey("vdt", G), skey("cosf", G)), writes=(key("GXY", G),))
        g.op("v", scan(flat(XY[bi]), flat(ONB), flat(GXY[bi])),
             reads=("ONB", key("GXY", G)), writes=(key("XY", G),))

    def dma_out(G):
        s = G % NSL
        bi = G % 2
        g.op("s", lambda: nc.sync.dma_start(OXY[s, 0], XY[bi][:, 0:2, 1:T + 1]),
             reads=(key("XY", G),), slot=s * 9 + 6)
        g.op("s", lambda: nc.sync.dma_start(OXY[s, 1], XY[bi][:, 2:4, 1:T + 1]),
             reads=(key("XY", G),), slot=s * 9 + 7)
        g.op("s", lambda: nc.sync.dma_start(OTH[s], TH[bi][:, :, 1:T + 1]),
             reads=(key("TH", G),), slot=s * 9 + 8)

    # ---------------- emission ----------------
    # dma+leaf are hoisted one slab ahead so Act's leaf work never blocks
    # the next slab's algebra behind trig of an older slab.
    dma_in(0)
    dma_dt(0)
    leaf_acts_a(0)
    leaf_acts_t(0)
    for G in range(NSL):
        alg_front(G)
        lnd_r7(G)
        if G + 1 < NSL:
            dma_in(G + 1)
            leaf_acts_a(G + 1)
        if G > 0:
            sweeps_a(G - 1)
        alg_mid(G)
        col_inits(G)
        if G > 0:
            sweeps_b(G - 1)
            kchain(G - 1)
            trig(G - 1)
            tail(G - 1)
            dma_out(G - 1)
        if G + 1 < NSL:
            leaf_acts_t(G + 1)
            dma_dt(G + 1)
    Gl = NSL - 1
    sweeps_a(Gl)
    sweeps_b(Gl)
    kchain(Gl)
    trig(Gl)
    tail(Gl)
    dma_out(Gl)

    n_slots = NSL * 9
    sem_v = nc.alloc_semaphore()
    sem_g = nc.alloc_semaphore()
    sem_a = nc.alloc_semaphore()
    dma_sems = [nc.alloc_semaphore(f"dsem{i}") for i in range(n_slots)]
    with nc.Block() as block:
        sems = {"v": sem_v, "g": sem_g, "a": sem_a}

        @block.sync
        def _(sync):
            last = {}
            dlast = {}
            for op_eng, emit_fn, deps, ref in sch.ops:
                if op_eng != "s":
                    continue
                for dep in deps:
                    if dep[0] == "D":
                        _, slot, k = dep
                        if dlast.get(slot, 0) >= k:
                            continue
                        sync.wait_ge(dma_sems[slot], 16 * k)
                        dlast[slot] = k
                    else:
                        deng, dpos = dep
                        if deng == "s" or last.get(deng, 0) >= dpos:
                            continue
                        sync.wait_ge(sems[deng], dpos)
                        last[deng] = dpos
                emit_fn().then_inc(dma_sems[ref[1]], 16)

        @block.vector
        def _(vector):
            sch.emit("v", vector, sems, dma_sems)

        @block.gpsimd
        def _(gp):
            sch.emit("g", gp, sems, dma_sems)

        @block.scalar
        def _(scalar):
            sch.emit("a", scalar, sems, dma_sems)

    return nc


_cache = {}


def _get_nc():
    if "nc" not in _cache:
        _cache["nc"] = _build_nc()
    return _cache["nc"]


def _pack_core(z_core, mu_core, times_core):
    zt = np.ascontiguousarray(z_core.transpose(2, 1, 0))       # (5, NPC, T)
    dt = np.empty_like(times_core)
    dt[0] = 0.0
    dt[1:] = times_core[1:] - times_core[:-1]
    IN = np.ascontiguousarray(
        zt.reshape(5, NSL, P, CH, T).transpose(1, 0, 2, 3, 4)).astype(np.float16)
    DTa = np.ascontiguousarray(dt.T.reshape(NSL, P, CH, T)).astype(np.float16)
    MU = np.ascontiguousarray(
        mu_core.reshape(NSL, P, CH, 5).transpose(0, 1, 3, 2))  # (NSL,P,5,CH)
    return {"inp": IN, "mu": MU, "dt": DTa}


def kernel(z_and_L_hat, mu0, times):
    z_and_L_hat = np.asarray(z_and_L_hat, dtype=np.float32)
    mu0 = np.asarray(mu0, dtype=np.float32)
    times = np.asarray(times, dtype=np.float32)
    nc = _get_nc()
    in_maps = []
    for k in range(N_CORES):
        sl = slice(k * NPC, (k + 1) * NPC)
        in_maps.append(_pack_core(z_and_L_hat[:, sl, :], mu0[sl], times[:, sl]))
    res = run_bass_kernel_spmd(nc, in_maps, core_ids=list(range(N_CORES)))
    out = np.empty((T, N_TOT, 3), np.float32)
    for k in range(N_CORES):
        oxy = res.results[k]["oxy"]               # (NSL, 2, P, CH, T) f16
        oth = res.results[k]["oth"]               # (NSL, P, CH, T) f32
        sl = slice(k * NPC, (k + 1) * NPC)
        out[:, sl, 0] = oxy[:, 0].astype(np.float32).reshape(NPC, T).T
        out[:, sl, 1] = oxy[:, 1].astype(np.float32).reshape(NPC, T).T
        out[:, sl, 2] = oth.astype(np.float32).reshape(NPC, T).T
    return out


# revision 11
# speedup vs baseline: 2.2354x; 1.0586x over previous
"""Trainium2 Bass kernel for nn_KalmanFilter: EKF over T=512 steps, N=8192 chains.

Mathematical reduction (verified exact vs the reference):
  With C = [[0,0,0,1,0],[0,0,0,0,1]], rows 3,4 of the Jacobian A are zero, so
  S = I + R depends only on per-step measurement params and the covariance
  never influences the output. Per chain:
    S = I + L L^T,  L = [[e^l0, 0], [l1, e^l2]]
    u_{t+1} = (I - S^-1) u_t + S^-1 z_t          (u = [v, omega])
    th_{t+1} = th_t + omega_t * dt_t
    x_{t+1}  = x_t + v_t * dt_t * cos(th_t)
    y_{t+1}  = y_t + v_t * dt_t * sin(th_t)
  The coupled 2-state linear recurrence is solved with 2 Gauss-Seidel sweeps
  of hardware affine scans (error contracts ~10x/sweep; end-to-end rel err
  ~3e-3 incl. fp16 quantization, vs the 2e-2 gate).

Implementation notes (from microbenchmarks on this part):
  - GpSimd shares SBUF ports with DVE and degrades it ~4x when running big
    ops: all full-size elementwise work runs on DVE (fp16 packed 2x mode,
    ~0.7us/1024el) + Act engine (function passes ~1.1us). GpSimd only does
    tiny per-slab init-column copies.
  - Scans are DVE-only, ~2.2ns/el, dtype-insensitive. Both chains per
    partition are covered by ONE scan via a zero-multiplier column at each
    chain start (which also injects the init value).
  - All intermediates fp16; t1, t2, s01, s11 carry a 2^-7 (det 2^-14)
    exponent scale folded into Act scale/bias so fp16 never overflows;
    r7 = 2^7/det compensates exactly.
  - x,y are produced and DMA'd as fp16 (host upcasts); th stays fp32 since
    range reduction needs it.
Sharding: data-parallel over chains, 1024 chains per core across 8 cores.
"""
import sys
sys.path.insert(0, '/opt/trn_rl_repo')
import numpy as np
import concourse.bass as bass
from concourse import mybir
from concourse.bass_utils import run_bass_kernel_spmd

F32 = mybir.dt.float32
F16 = mybir.dt.float16
AF = mybir.ActivationFunctionType
A = mybir.AluOpType

N_CORES = 8
T = 512
N_TOT = 8192
NPC = N_TOT // N_CORES          # 1024 chains per core
P = 128                         # partitions
NSL = 4                         # slabs per core
CH = NPC // (NSL * P)           # chains per partition per slab = 2
TP = T + 2                      # padded per-chain row: [init | T data | pad]
MAGIC = float(1.5 * 2 ** 23)
TWO_PI = float(2 * np.pi)
INV_2PI = float(1.0 / (2 * np.pi))
LN2x7 = float(7 * np.log(2.0))
HS = float(2.0 ** -3.5)         # Square scale for l1^2 * 2^-7
EPS7 = float(2.0 ** -7)


class _Sched:
    """Two-phase scheduler: record ops (engine, emit closure, deps), then emit
    per-engine in-order streams with standalone wait_ge for cross-engine deps."""

    def __init__(self):
        self.ops = []
        self.count = {"v": 0, "g": 0, "a": 0}
        self.slot_count = {}

    def add(self, eng, emit_fn, deps=(), slot=None):
        if eng == "s":
            self.slot_count[slot] = self.slot_count.get(slot, 0) + 1
            ref = ("D", slot, self.slot_count[slot])
        else:
            self.count[eng] += 1
            ref = (eng, self.count[eng])
        self.ops.append((eng, emit_fn, tuple(d for d in deps if d), ref))
        return ref

    def emit(self, eng, raw_eng, sems, dma_sems):
        last = {}
        dlast = {}
        for op_eng, emit_fn, deps, ref in self.ops:
            if op_eng != eng:
                continue
            for dep in deps:
                if dep[0] == "D":
                    _, slot, k = dep
                    if dlast.get(slot, 0) >= k:
                        continue
                    raw_eng.wait_ge(dma_sems[slot], 16 * k)
                    dlast[slot] = k
                else:
                    deng, dpos = dep
                    if deng == eng or last.get(deng, 0) >= dpos:
                        continue
                    raw_eng.wait_ge(sems[deng], dpos)
                    last[deng] = dpos
            emit_fn().then_inc(sems[eng], 1)


class _Graph:
    """Auto RAW/WAR/WAW dependency tracking over named tiles."""

    def __init__(self, sch):
        self.sch = sch
        self.w = {}
        self.r = {}

    def op(self, eng, emit_fn, reads=(), writes=(), slot=None, extra=()):
        deps = {}

        def add(ref):
            if ref is None:
                return
            key = ref[0] if ref[0] != "D" else ("D", ref[1])
            cur = deps.get(key)
            if cur is None or ref[-1] > cur[-1]:
                deps[key] = ref

        for t in reads:
            add(self.w.get(t))
        for t in writes:
            add(self.w.get(t))
            for rr in self.r.get(t, ()):
                add(rr)
        for e in extra:
            add(e)
        ref = self.sch.add(eng, emit_fn, deps=tuple(deps.values()), slot=slot)
        for t in reads:
            self.r.setdefault(t, []).append(ref)
        for t in writes:
            self.w[t] = ref
            self.r[t] = []
        return ref


def _build_nc():
    nc = bass.Bass()
    # Register activation bias constants (bass converts float biases of
    # non-Copy activations to const APs, which must pre-exist).
    for val in (-LN2x7, EPS7):
        t = nc.alloc_sbuf_tensor(f"constf32-{val}", [128, 1], F32)
        nc.gpsimd.memset(t.ap(), val)
        nc.const_aps.aps[(F32, val)] = t.ap()
    nc.all_engine_barrier()
    IN = nc.dram_tensor("inp", [NSL, 5, P, CH, T], F16, kind="ExternalInput")
    DT = nc.dram_tensor("dt", [NSL, P, CH, T], F16, kind="ExternalInput")
    MU = nc.dram_tensor("mu", [NSL, P, 5, CH], F32, kind="ExternalInput")
    OXY = nc.dram_tensor("oxy", [NSL, 2, P, CH, T], F16, kind="ExternalOutput")
    OTH = nc.dram_tensor("oth", [NSL, P, CH, T], F32, kind="ExternalOutput")

    _names = [0]

    def tile(shape, dt=F16):
        _names[0] += 1
        return nc.alloc_sbuf_tensor(f"tl{_names[0]}", list(shape), dt).ap()

    def flat(ap):
        return ap.rearrange('p a b -> p (a b)')

    # constants: fp16 ones with 0 at col0 of each chain row
    ONB = tile([P, 4, TP])

    # per-parity tiles
    zin = [tile([P, 5, CH, T]) for _ in range(2)]
    dtz = [tile([P, CH, T]) for _ in range(2)]
    mu = [tile([P, 5, CH], F32) for _ in range(2)]
    M0x = [tile([P, CH, TP]) for _ in range(2)]
    M1x = [tile([P, CH, TP]) for _ in range(2)]
    B0x = [tile([P, CH, TP]) for _ in range(2)]
    CWx = [tile([P, CH, TP]) for _ in range(2)]
    CVx = [tile([P, CH, TP]) for _ in range(2)]
    Gx = [tile([P, CH, TP]) for _ in range(2)]
    GXY = [tile([P, 4, TP]) for _ in range(2)]
    Vt = [tile([P, CH, TP]) for _ in range(2)]
    Wt = [tile([P, CH, TP]) for _ in range(2)]
    TH = [tile([P, CH, TP], F32) for _ in range(2)]
    XY = [tile([P, 4, TP]) for _ in range(2)]
    K1 = [tile([P, CH, T], F32) for _ in range(2)]
    SCR = [[tile([P, CH, T]) for _ in range(17)] for _ in range(2)]

    sch = _Sched()
    g = _Graph(sch)

    # ---- preamble ----
    g.op("v", lambda: nc.vector.memset(flat(ONB), 1.0), writes=("ONB",))
    for c in range(4):
        g.op("v", lambda c=c: nc.vector.memset(ONB[:, c, 0:1], 0.0),
             writes=("ONB",))
    for bi in range(2):
        for nm, tl in (("M0x", M0x), ("M1x", M1x), ("B0x", B0x), ("CWx", CWx),
                       ("CVx", CVx), ("Gx", Gx)):
            g.op("v", lambda tl=tl, bi=bi: nc.vector.memset(
                tl[bi][:, :, TP - 1:TP], 0.0), writes=(f"{nm}{bi}",))
        g.op("v", lambda bi=bi: nc.vector.memset(GXY[bi][:, :, TP - 1:TP], 0.0),
             writes=(f"GXY{bi}",))
        for nm, tl in (("M0x", M0x), ("M1x", M1x)):
            g.op("v", lambda tl=tl, bi=bi: nc.vector.memset(
                tl[bi][:, :, 0:1], 0.0), writes=(f"{nm}{bi}",))

    def key(nm, G):
        return f"{nm}{G % 2}"

    # scratch phys allocation: names sharing an index alias the same tile
    PHYS = {"e0sq": 0, "u0": 0,
            "e2sq": 1, "u1": 1,
            "e0h": 2,
            "l1sq": 3, "p0": 3,
            "t1h": 4, "p1": 4,
            "t2h": 5, "q0": 5,
            "d1h": 6, "lnd": 6, "b1": 6,
            "deth": 7, "q1": 7,
            "s01h": 8, "s11h": 9,
            "sinf": 15, "cosf": 16,
            "r7": 10, "tmp": 10,
            "vdt": 11, "thr": 12, "m01": 13, "l1sq14": 14}

    def scr(nm, G):
        return SCR[G % 2][PHYS[nm]]

    def skey(nm, G):
        return f"S{PHYS[nm]}_{G % 2}"

    # ---------------- stages ----------------
    def dma_in(G):
        s = G % NSL
        bi = G % 2
        for pl in (2, 3, 4):
            g.op("s", lambda pl=pl: nc.sync.dma_start(
                zin[bi][:, pl], IN[s, pl]),
                writes=(key(f"zinL{pl}", G),), slot=s * 9 + (pl - 2))
        g.op("s", lambda: nc.sync.dma_start(
            zin[bi][:, 0:2], IN[s, 0:2].rearrange("k p c t -> p k c t")),
            writes=(key("zinZ", G),), slot=s * 9 + 3)
        g.op("s", lambda: nc.sync.dma_start(mu[bi][:], MU[s]),
            writes=(key("mu", G),), slot=s * 9 + 4)

    def dma_dt(G):
        s = G % NSL
        bi = G % 2
        g.op("s", lambda: nc.sync.dma_start(dtz[bi][:], DT[s]),
            writes=(key("dtz", G),), slot=s * 9 + 5)

    def act(out_ap, in_ap, func, bias=0.0, scale=1.0):
        return lambda: nc.scalar.activation(out_ap, in_ap, func,
                                            bias=bias, scale=scale)

    def leaf_acts_a(G):
        bi = G % 2
        z = zin[bi]
        l0 = z[:, 2]; l1 = z[:, 3]; l2 = z[:, 4]
        g.op("a", act(scr("e0sq", G)[:], l0, AF.Exp, bias=-LN2x7, scale=2.0),
             reads=(key("zinL2", G),), writes=(skey("e0sq", G),))
        g.op("a", act(scr("e2sq", G)[:], l2, AF.Exp, bias=-LN2x7, scale=2.0),
             reads=(key("zinL4", G),), writes=(skey("e2sq", G),))
        g.op("a", act(scr("e0h", G)[:], l0, AF.Exp, bias=-LN2x7),
             reads=(key("zinL2", G),), writes=(skey("e0h", G),))
        g.op("a", act(scr("l1sq", G)[:], l1, AF.Square, scale=HS),
             reads=(key("zinL3", G),), writes=(skey("l1sq", G),))
        g.op("a", act(scr("l1sq14", G)[:], l1, AF.Square, scale=EPS7),
             reads=(key("zinL3", G),), writes=(skey("l1sq14", G),))

    def t12(G):
        g.op("v", lambda G=G: nc.vector.tensor_scalar(
            scr("t1h", G)[:], scr("e0sq", G)[:], EPS7, None, op0=A.add),
            reads=(skey("e0sq", G),), writes=(skey("t1h", G),))
        g.op("v", lambda G=G: nc.vector.tensor_scalar(
            scr("t2h", G)[:], scr("e2sq", G)[:], EPS7, None, op0=A.add),
            reads=(skey("e2sq", G),), writes=(skey("t2h", G),))

    def tt(out, in0, in1, op):
        return lambda: nc.vector.tensor_tensor(out, in0, in1, op)

    def alg_front(G):
        g.op("v", tt(scr("d1h", G)[:], scr("t1h", G)[:], scr("t2h", G)[:], A.mult),
             reads=(skey("t1h", G), skey("t2h", G)), writes=(skey("d1h", G),))
        g.op("v", tt(scr("s01h", G)[:], scr("e0h", G)[:], zin[G % 2][:, 3], A.mult),
             reads=(skey("e0h", G), key("zinL3", G)), writes=(skey("s01h", G),))
        g.op("v", tt(scr("s11h", G)[:], scr("t2h", G)[:], scr("l1sq", G)[:], A.add),
             reads=(skey("t2h", G), skey("l1sq", G)), writes=(skey("s11h", G),))
        g.op("v", tt(scr("deth", G)[:], scr("d1h", G)[:], scr("l1sq14", G)[:],
                     A.add),
             reads=(skey("d1h", G), skey("l1sq14", G)), writes=(skey("deth", G),))

    def lnd_r7(G):
        g.op("a", act(scr("lnd", G)[:], scr("deth", G)[:], AF.Ln),
             reads=(skey("deth", G),), writes=(skey("lnd", G),))
        g.op("a", act(scr("r7", G)[:], scr("lnd", G)[:], AF.Exp,
                      bias=-LN2x7, scale=-1.0),
             reads=(skey("lnd", G),), writes=(skey("r7", G),))

    def alg_mid(G):
        bi = G % 2
        z = zin[bi]
        z0 = z[:, 0]; z1 = z[:, 1]
        zk = key("zinZ", G)
        g.op("v", tt(scr("u0", G)[:], scr("s11h", G)[:], scr("r7", G)[:], A.mult),
             reads=(skey("s11h", G), skey("r7", G)), writes=(skey("u0", G),))
        g.op("v", tt(scr("u1", G)[:], scr("t1h", G)[:], scr("r7", G)[:], A.mult),
             reads=(skey("t1h", G), skey("r7", G)), writes=(skey("u1", G),))
        # m00/m11 on DVE (fp16 TS, 2x packed) right where they're produced
        g.op("v", lambda bi=bi, G=G: nc.vector.tensor_scalar(
            M0x[bi][:, :, 1:T + 1], scr("u0", G)[:], -1.0, 1.0,
            op0=A.mult, op1=A.add),
            reads=(skey("u0", G),), writes=(key("M0x", G),))
        g.op("v", lambda bi=bi, G=G: nc.vector.tensor_scalar(
            M1x[bi][:, :, 1:T + 1], scr("u1", G)[:], -1.0, 1.0,
            op0=A.mult, op1=A.add),
            reads=(skey("u1", G),), writes=(key("M1x", G),))
        g.op("v", tt(scr("m01", G)[:], scr("s01h", G)[:], scr("r7", G)[:], A.mult),
             reads=(skey("s01h", G), skey("r7", G)), writes=(skey("m01", G),))
        g.op("v", tt(scr("p0", G)[:], scr("u0", G)[:], z0, A.mult),
             reads=(skey("u0", G), zk), writes=(skey("p0", G),))
        g.op("v", tt(scr("q0", G)[:], scr("m01", G)[:], z1, A.mult),
             reads=(skey("m01", G), zk), writes=(skey("q0", G),))
        g.op("v", tt(B0x[bi][:, :, 1:T + 1], scr("p0", G)[:], scr("q0", G)[:],
                     A.subtract),
             reads=(skey("p0", G), skey("q0", G)), writes=(key("B0x", G),))
        g.op("v", tt(scr("p1", G)[:], scr("u1", G)[:], z1, A.mult),
             reads=(skey("u1", G), zk), writes=(skey("p1", G),))
        g.op("v", tt(scr("q1", G)[:], scr("m01", G)[:], z0, A.mult),
             reads=(skey("m01", G), zk), writes=(skey("q1", G),))
        g.op("v", tt(scr("b1", G)[:], scr("p1", G)[:], scr("q1", G)[:], A.subtract),
             reads=(skey("p1", G), skey("q1", G)), writes=(skey("b1", G),))

    def col_inits(G):
        # tiny [P,CH] init-column copies; on GpSimd (idle, negligible size)
        bi = G % 2
        m = mu[bi]
        mk = key("mu", G)
        for dst_ap, dst_key, mi in (
                (B0x[bi][:, :, 0], "B0x", 3), (CVx[bi][:, :, 0], "CVx", 3),
                (CWx[bi][:, :, 0], "CWx", 4), (Gx[bi][:, :, 0], "Gx", 2),
                (GXY[bi][:, 0:2, 0], "GXY", 0), (GXY[bi][:, 2:4, 0], "GXY", 1)):
            g.op("a", (lambda dst_ap=dst_ap, m=m, mi=mi:
                       nc.scalar.activation(dst_ap, m[:, mi], AF.Identity)),
                 reads=(mk,), writes=(key(dst_key, G),))

    def scan(out, d0, d1):
        return lambda: nc.vector.tensor_tensor_scan(out, d0, d1, 0.0,
                                                    A.mult, A.add)

    def sweeps_a(G):
        bi = G % 2
        m01 = scr("m01", G)
        tmp = scr("tmp", G)
        b1 = scr("b1", G)
        g.op("v", scan(flat(Vt[bi]), flat(M0x[bi]), flat(B0x[bi])),
             reads=(key("M0x", G), key("B0x", G)), writes=(key("V", G),))
        g.op("v", tt(tmp[:], m01[:], Vt[bi][:, :, 0:T], A.mult),
             reads=(skey("m01", G), key("V", G)), writes=(skey("tmp", G),))
        g.op("v", tt(CWx[bi][:, :, 1:T + 1], tmp[:], b1[:], A.add),
             reads=(skey("tmp", G), skey("b1", G)), writes=(key("CWx", G),))
        g.op("v", scan(flat(Wt[bi]), flat(M1x[bi]), flat(CWx[bi])),
             reads=(key("M1x", G), key("CWx", G)), writes=(key("W", G),))

    def sweeps_b(G):
        bi = G % 2
        m01 = scr("m01", G)
        tmp = scr("tmp", G)
        b1 = scr("b1", G)
        g.op("v", tt(tmp[:], m01[:], Wt[bi][:, :, 0:T], A.mult),
             reads=(skey("m01", G), key("W", G)), writes=(skey("tmp", G),))
        g.op("v", tt(CVx[bi][:, :, 1:T + 1], tmp[:], B0x[bi][:, :, 1:T + 1], A.add),
             reads=(skey("tmp", G), key("B0x", G)), writes=(key("CVx", G),))
        g.op("v", scan(flat(Vt[bi]), flat(M0x[bi]), flat(CVx[bi])),
             reads=(key("M0x", G), key("CVx", G)), writes=(key("V", G),))
        g.op("v", tt(tmp[:], m01[:], Vt[bi][:, :, 0:T], A.mult),
             reads=(skey("m01", G), key("V", G)), writes=(skey("tmp", G),))
        g.op("v", tt(CWx[bi][:, :, 1:T + 1], tmp[:], b1[:], A.add),
             reads=(skey("tmp", G), skey("b1", G)), writes=(key("CWx", G),))
        g.op("v", scan(flat(Wt[bi]), flat(M1x[bi]), flat(CWx[bi])),
             reads=(key("M1x", G), key("CWx", G)), writes=(key("W", G),))
        g.op("v", tt(Gx[bi][:, :, 1:T + 1], Wt[bi][:, :, 0:T], dtz[bi][:],
                     A.mult),
             reads=(key("W", G), key("dtz", G)), writes=(key("Gx", G),))
        g.op("v", scan(flat(TH[bi]), flat(ONB[:, 0:2]), flat(Gx[bi])),
             reads=("ONB", key("Gx", G)), writes=(key("TH", G),))

    def kchain(G):
        # k1 = th/2pi + MAGIC (round-to-int trick), k2 = (k1-MAGIC)*2pi
        # (exact: k1-MAGIC is Sterbenz-exact), thr = th - k2 in [-pi, pi].
        bi = G % 2
        g.op("v", lambda bi=bi: nc.vector.tensor_scalar(
            K1[bi][:], TH[bi][:, :, 0:T], INV_2PI, MAGIC, op0=A.mult, op1=A.add),
            reads=(key("TH", G),), writes=(key("K1", G),))
        g.op("v", lambda bi=bi: nc.vector.tensor_scalar(
            K1[bi][:], K1[bi][:], -MAGIC, TWO_PI, op0=A.add, op1=A.mult),
            reads=(key("K1", G),), writes=(key("K1", G),))
        g.op("v", tt(scr("thr", G)[:], TH[bi][:, :, 0:T], K1[bi][:], A.subtract),
             reads=(key("TH", G), key("K1", G)), writes=(skey("thr", G),))

    def trig(G):
        thr = scr("thr", G)
        g.op("a", act(scr("sinf", G)[:], thr[:], AF.Sin),
             reads=(skey("thr", G),), writes=(skey("sinf", G),))
        g.op("a", act(scr("cosf", G)[:], thr[:], AF.Sin, scale=0.5),
             reads=(skey("thr", G),), writes=(skey("cosf", G),))

    def tail(G):
        bi = G % 2
        vdt = scr("vdt", G)
        cosf = scr("cosf", G)
        g.op("v", tt(vdt[:], Vt[bi][:, :, 0:T], dtz[bi][:], A.mult),
             reads=(key("V", G), key("dtz", G)), writes=(skey("vdt", G),))
        g.op("v", tt(GXY[bi][:, 2:4, 1:T + 1], vdt[:], scr("sinf", G)[:], A.mult),
             reads=(skey("vdt", G), skey("sinf", G)), writes=(key("GXY", G),))
        # cos(th) = 1 - 2*sin(th/2)^2, square+affine on DVE
        g.op("v", tt(cosf[:], cosf[:], cosf[:], A.mult),
             reads=(skey("cosf", G),), writes=(skey("cosf", G),))
        g.op("v", lambda G=G: nc.vector.tensor_scalar(
            cosf[:], cosf[:], -2.0, 1.0, op0=A.mult, op1=A.add),
            reads=(skey("cosf", G),), writes=(skey("cosf", G),))
        g.op("v", tt(GXY[bi][:, 0:2, 1:T + 1], vdt[:], cosf[:], A.mult),
             reads=(skey("vdt", G), skey("cosf", G)), writes=(key("GXY", G),))
        g.op("v", scan(flat(XY[bi]), flat(ONB), flat(GXY[bi])),
             reads=("ONB", key("GXY", G)), writes=(key("XY", G),))

    def dma_out(G):
        s = G % NSL
        bi = G % 2
        g.op("s", lambda: nc.sync.dma_start(OXY[s, 0], XY[bi][:, 0:2, 1:T + 1]),
             reads=(key("XY", G),), slot=s * 9 + 6)
        g.op("s", lambda: nc.sync.dma_start(OXY[s, 1], XY[bi][:, 2:4, 1:T + 1]),
             reads=(key("XY", G),), slot=s * 9 + 7)
        g.op("s", lambda: nc.sync.dma_start(OTH[s], TH[bi][:, :, 1:T + 1]),
             reads=(key("TH", G),), slot=s * 9 + 8)

    # ---------------- emission ----------------
    # dma+leaf are hoisted one slab ahead so Act's leaf work never blocks
    # the next slab's algebra behind trig of an older slab.
    dma_in(0)
    dma_dt(0)
    leaf_acts_a(0)
    for G in range(NSL):
        if G == 0:
            t12(0)
            alg_front(0)
            lnd_r7(0)
        if G + 1 < NSL:
            dma_in(G + 1)
            leaf_acts_a(G + 1)
        if G > 0:
            sweeps_a(G - 1)
        alg_mid(G)
        col_inits(G)
        if G > 0:
            sweeps_b(G - 1)
            kchain(G - 1)
            trig(G - 1)
        if G + 1 < NSL:
            t12(G + 1)
            alg_front(G + 1)
            lnd_r7(G + 1)
        if G > 0:
            tail(G - 1)
            dma_out(G - 1)
        if G + 1 < NSL:
            dma_dt(G + 1)
    Gl = NSL - 1
    sweeps_a(Gl)
    sweeps_b(Gl)
    kchain(Gl)
    trig(Gl)
    tail(Gl)
    dma_out(Gl)

    n_slots = NSL * 9
    sem_v = nc.alloc_semaphore()
    sem_g = nc.alloc_semaphore()
    sem_a = nc.alloc_semaphore()
    dma_sems = [nc.alloc_semaphore(f"dsem{i}") for i in range(n_slots)]
    with nc.Block() as block:
        sems = {"v": sem_v, "g": sem_g, "a": sem_a}

        @block.sync
        def _(sync):
            last = {}
            dlast = {}
            for op_eng, emit_fn, deps, ref in sch.ops:
                if op_eng != "s":
                    continue
                for dep in deps:
                    if dep[0] == "D":
                        _, slot, k = dep
                        if dlast.get(slot, 0) >= k:
                            continue
                        sync.wait_ge(dma_sems[slot], 16 * k)
                        dlast[slot] = k
                    else:
                        deng, dpos = dep
                        if deng == "s" or last.get(deng, 0) >= dpos:
                            continue
                        sync.wait_ge(sems[deng], dpos)
                        last[deng] = dpos
                emit_fn().then_inc(dma_sems[ref[1]], 16)

        @block.vector
        def _(vector):
            sch.emit("v", vector, sems, dma_sems)

        @block.gpsimd
        def _(gp):
            sch.emit("g", gp, sems, dma_sems)

        @block.scalar
        def _(scalar):
            sch.emit("a", scalar, sems, dma_sems)

    return nc


_cache = {}


def _get_nc():
    if "nc" not in _cache:
        _cache["nc"] = _build_nc()
    return _cache["nc"]


def _pack_core(z_core, mu_core, times_core):
    zt = np.ascontiguousarray(z_core.transpose(2, 1, 0))       # (5, NPC, T)
    dt = np.empty_like(times_core)
    dt[0] = 0.0
    dt[1:] = times_core[1:] - times_core[:-1]
    IN = np.ascontiguousarray(
        zt.reshape(5, NSL, P, CH, T).transpose(1, 0, 2, 3, 4)).astype(np.float16)
    DTa = np.ascontiguousarray(dt.T.reshape(NSL, P, CH, T)).astype(np.float16)
    MU = np.ascontiguousarray(
        mu_core.reshape(NSL, P, CH, 5).transpose(0, 1, 3, 2))  # (NSL,P,5,CH)
    return {"inp": IN, "mu": MU, "dt": DTa}


def kernel(z_and_L_hat, mu0, times):
    z_and_L_hat = np.asarray(z_and_L_hat, dtype=np.float32)
    mu0 = np.asarray(mu0, dtype=np.float32)
    times = np.asarray(times, dtype=np.float32)
    nc = _get_nc()
    in_maps = []
    for k in range(N_CORES):
        sl = slice(k * NPC, (k + 1) * NPC)
        in_maps.append(_pack_core(z_and_L_hat[:, sl, :], mu0[sl], times[:, sl]))
    res = run_bass_kernel_spmd(nc, in_maps, core_ids=list(range(N_CORES)))
    out = np.empty((T, N_TOT, 3), np.float32)
    for k in range(N_CORES):
        oxy = res.results[k]["oxy"]               # (NSL, 2, P, CH, T) f16
        oth = res.results[k]["oth"]               # (NSL, P, CH, T) f32
        sl = slice(k * NPC, (k + 1) * NPC)
        out[:, sl, 0] = oxy[:, 0].astype(np.float32).reshape(NPC, T).T
        out[:, sl, 1] = oxy[:, 1].astype(np.float32).reshape(NPC, T).T
        out[:, sl, 2] = oth.astype(np.float32).reshape(NPC, T).T
    return out


# revision 12
# speedup vs baseline: 2.2544x; 1.0085x over previous
"""Trainium2 Bass kernel for nn_KalmanFilter: EKF over T=512 steps, N=8192 chains.

Mathematical reduction (verified exact vs the reference):
  With C = [[0,0,0,1,0],[0,0,0,0,1]], rows 3,4 of the Jacobian A are zero, so
  S = I + R depends only on per-step measurement params and the covariance
  never influences the output. Per chain:
    S = I + L L^T,  L = [[e^l0, 0], [l1, e^l2]]
    u_{t+1} = (I - S^-1) u_t + S^-1 z_t          (u = [v, omega])
    th_{t+1} = th_t + omega_t * dt_t
    x_{t+1}  = x_t + v_t * dt_t * cos(th_t)
    y_{t+1}  = y_t + v_t * dt_t * sin(th_t)
  The coupled 2-state linear recurrence is solved with 2 Gauss-Seidel sweeps
  of hardware affine scans (error contracts ~10x/sweep; end-to-end rel err
  ~3e-3 incl. fp16 quantization, vs the 2e-2 gate).

Implementation notes (from microbenchmarks on this part):
  - GpSimd shares SBUF ports with DVE and degrades it ~4x when running big
    ops: all full-size elementwise work runs on DVE (fp16 packed 2x mode,
    ~0.7us/1024el) + Act engine (function passes ~1.1us). GpSimd only does
    tiny per-slab init-column copies.
  - Scans are DVE-only, ~2.2ns/el, dtype-insensitive. Both chains per
    partition are covered by ONE scan via a zero-multiplier column at each
    chain start (which also injects the init value).
  - All intermediates fp16; t1, t2, s01, s11 carry a 2^-7 (det 2^-14)
    exponent scale folded into Act scale/bias so fp16 never overflows;
    r7 = 2^7/det compensates exactly.
  - x,y are produced and DMA'd as fp16 (host upcasts); th stays fp32 since
    range reduction needs it.
Sharding: data-parallel over chains, 1024 chains per core across 8 cores.
"""
import sys
sys.path.insert(0, '/opt/trn_rl_repo')
import numpy as np
import concourse.bass as bass
from concourse import mybir
from concourse.bass_utils import run_bass_kernel_spmd

F32 = mybir.dt.float32
F16 = mybir.dt.float16
AF = mybir.ActivationFunctionType
A = mybir.AluOpType

N_CORES = 8
T = 512
N_TOT = 8192
NPC = N_TOT // N_CORES          # 1024 chains per core
P = 128                         # partitions
NSL = 4                         # slabs per core
CH = NPC // (NSL * P)           # chains per partition per slab = 2
TP = T + 2                      # padded per-chain row: [init | T data | pad]
MAGIC = float(1.5 * 2 ** 23)
TWO_PI = float(2 * np.pi)
INV_2PI = float(1.0 / (2 * np.pi))
LN2x7 = float(7 * np.log(2.0))
HS = float(2.0 ** -3.5)         # Square scale for l1^2 * 2^-7
EPS7 = float(2.0 ** -7)


class _Sched:
    """Two-phase scheduler: record ops (engine, emit closure, deps), then emit
    per-engine in-order streams with standalone wait_ge for cross-engine deps."""

    def __init__(self):
        self.ops = []
        self.count = {"v": 0, "g": 0, "a": 0}
        self.slot_count = {}

    def add(self, eng, emit_fn, deps=(), slot=None):
        if eng == "s":
            self.slot_count[slot] = self.slot_count.get(slot, 0) + 1
            ref = ("D", slot, self.slot_count[slot])
        else:
            self.count[eng] += 1
            ref = (eng, self.count[eng])
        self.ops.append((eng, emit_fn, tuple(d for d in deps if d), ref))
        return ref

    def emit(self, eng, raw_eng, sems, dma_sems):
        last = {}
        dlast = {}
        for op_eng, emit_fn, deps, ref in self.ops:
            if op_eng != eng:
                continue
            for dep in deps:
                if dep[0] == "D":
                    _, slot, k = dep
                    if dlast.get(slot, 0) >= k:
                        continue
                    raw_eng.wait_ge(dma_sems[slot], 16 * k)
                    dlast[slot] = k
                else:
                    deng, dpos = dep
                    if deng == eng or last.get(deng, 0) >= dpos:
                        continue
                    raw_eng.wait_ge(sems[deng], dpos)
                    last[deng] = dpos
            emit_fn().then_inc(sems[eng], 1)


class _Graph:
    """Auto RAW/WAR/WAW dependency tracking over named tiles."""

    def __init__(self, sch):
        self.sch = sch
        self.w = {}
        self.r = {}

    def op(self, eng, emit_fn, reads=(), writes=(), slot=None, extra=()):
        deps = {}

        def add(ref):
            if ref is None:
                return
            key = ref[0] if ref[0] != "D" else ("D", ref[1])
            cur = deps.get(key)
            if cur is None or ref[-1] > cur[-1]:
                deps[key] = ref

        for t in reads:
            add(self.w.get(t))
        for t in writes:
            add(self.w.get(t))
            for rr in self.r.get(t, ()):
                add(rr)
        for e in extra:
            add(e)
        ref = self.sch.add(eng, emit_fn, deps=tuple(deps.values()), slot=slot)
        for t in reads:
            self.r.setdefault(t, []).append(ref)
        for t in writes:
            self.w[t] = ref
            self.r[t] = []
        return ref


def _build_nc():
    nc = bass.Bass()
    # Register activation bias constants (bass converts float biases of
    # non-Copy activations to const APs, which must pre-exist).
    for val in (-LN2x7, EPS7):
        t = nc.alloc_sbuf_tensor(f"constf32-{val}", [128, 1], F32)
        nc.gpsimd.memset(t.ap(), val)
        nc.const_aps.aps[(F32, val)] = t.ap()
    nc.all_engine_barrier()
    IN = nc.dram_tensor("inp", [NSL, 5, P, CH, T], F16, kind="ExternalInput")
    DT = nc.dram_tensor("dt", [NSL, P, CH, T], F16, kind="ExternalInput")
    MU = nc.dram_tensor("mu", [NSL, P, 5, CH], F32, kind="ExternalInput")
    OXY = nc.dram_tensor("oxy", [NSL, 2, P, CH, T], F16, kind="ExternalOutput")
    OTH = nc.dram_tensor("oth", [NSL, P, CH, T], F32, kind="ExternalOutput")

    _names = [0]

    def tile(shape, dt=F16):
        _names[0] += 1
        return nc.alloc_sbuf_tensor(f"tl{_names[0]}", list(shape), dt).ap()

    def flat(ap):
        return ap.rearrange('p a b -> p (a b)')

    # constants: fp16 ones with 0 at col0 of each chain row
    ONB = tile([P, 4, TP])

    # per-parity tiles
    zin = [tile([P, 5, CH, T]) for _ in range(2)]
    dtz = [tile([P, CH, T]) for _ in range(2)]
    mu = [tile([P, 5, CH], F32) for _ in range(2)]
    M0x = [tile([P, CH, TP]) for _ in range(2)]
    M1x = [tile([P, CH, TP]) for _ in range(2)]
    B0x = [tile([P, CH, TP]) for _ in range(2)]
    CWx = [tile([P, CH, TP]) for _ in range(2)]
    CVx = [tile([P, CH, TP]) for _ in range(2)]
    Gx = [tile([P, CH, TP]) for _ in range(2)]
    GXt = [tile([P, CH, TP]) for _ in range(2)]
    GYt = [tile([P, CH, TP]) for _ in range(2)]
    Vt = [tile([P, CH, TP]) for _ in range(2)]
    Wt = [tile([P, CH, TP]) for _ in range(2)]
    TH = [tile([P, CH, TP], F32) for _ in range(2)]
    XT = [tile([P, CH, TP]) for _ in range(2)]
    YT = [tile([P, CH, TP]) for _ in range(2)]
    K1 = [tile([P, CH, T], F32) for _ in range(2)]
    SCR = [[tile([P, CH, T]) for _ in range(17)] for _ in range(2)]

    sch = _Sched()
    g = _Graph(sch)

    # ---- preamble ----
    g.op("v", lambda: nc.vector.memset(flat(ONB), 1.0), writes=("ONB",))
    for c in range(4):
        g.op("v", lambda c=c: nc.vector.memset(ONB[:, c, 0:1], 0.0),
             writes=("ONB",))
    for bi in range(2):
        for nm, tl in (("M0x", M0x), ("M1x", M1x), ("B0x", B0x), ("CWx", CWx),
                       ("CVx", CVx), ("Gx", Gx)):
            g.op("v", lambda tl=tl, bi=bi: nc.vector.memset(
                tl[bi][:, :, TP - 1:TP], 0.0), writes=(f"{nm}{bi}",))
        g.op("v", lambda bi=bi: nc.vector.memset(GXt[bi][:, :, TP - 1:TP], 0.0),
             writes=(f"GX{bi}",))
        g.op("v", lambda bi=bi: nc.vector.memset(GYt[bi][:, :, TP - 1:TP], 0.0),
             writes=(f"GY{bi}",))
        for nm, tl in (("M0x", M0x), ("M1x", M1x)):
            g.op("v", lambda tl=tl, bi=bi: nc.vector.memset(
                tl[bi][:, :, 0:1], 0.0), writes=(f"{nm}{bi}",))

    def key(nm, G):
        return f"{nm}{G % 2}"

    # scratch phys allocation: names sharing an index alias the same tile
    PHYS = {"e0sq": 0, "u0": 0,
            "e2sq": 1, "u1": 1,
            "e0h": 2,
            "l1sq": 3, "p0": 3,
            "t1h": 4, "p1": 4,
            "t2h": 5, "q0": 5,
            "d1h": 6, "lnd": 6, "b1": 6,
            "deth": 7, "q1": 7,
            "s01h": 8, "s11h": 9,
            "sinf": 15, "cosf": 16,
            "r7": 10, "tmp": 10,
            "vdt": 11, "thr": 12, "m01": 13, "l1sq14": 14}

    def scr(nm, G):
        return SCR[G % 2][PHYS[nm]]

    def skey(nm, G):
        return f"S{PHYS[nm]}_{G % 2}"

    # ---------------- stages ----------------
    def dma_in(G):
        s = G % NSL
        bi = G % 2
        for pl in (2, 3, 4):
            g.op("s", lambda pl=pl: nc.sync.dma_start(
                zin[bi][:, pl], IN[s, pl]),
                writes=(key(f"zinL{pl}", G),), slot=s * 9 + (pl - 2))
        g.op("s", lambda: nc.sync.dma_start(
            zin[bi][:, 0:2], IN[s, 0:2].rearrange("k p c t -> p k c t")),
            writes=(key("zinZ", G),), slot=s * 9 + 3)
        g.op("s", lambda: nc.sync.dma_start(mu[bi][:], MU[s]),
            writes=(key("mu", G),), slot=s * 9 + 4)

    def dma_dt(G):
        s = G % NSL
        bi = G % 2
        g.op("s", lambda: nc.sync.dma_start(dtz[bi][:], DT[s]),
            writes=(key("dtz", G),), slot=s * 9 + 5)

    def act(out_ap, in_ap, func, bias=0.0, scale=1.0):
        return lambda: nc.scalar.activation(out_ap, in_ap, func,
                                            bias=bias, scale=scale)

    def leaf_acts_a(G):
        bi = G % 2
        z = zin[bi]
        l0 = z[:, 2]; l1 = z[:, 3]; l2 = z[:, 4]
        g.op("a", act(scr("e0sq", G)[:], l0, AF.Exp, bias=-LN2x7, scale=2.0),
             reads=(key("zinL2", G),), writes=(skey("e0sq", G),))
        g.op("a", act(scr("e2sq", G)[:], l2, AF.Exp, bias=-LN2x7, scale=2.0),
             reads=(key("zinL4", G),), writes=(skey("e2sq", G),))
        g.op("a", act(scr("e0h", G)[:], l0, AF.Exp, bias=-LN2x7),
             reads=(key("zinL2", G),), writes=(skey("e0h", G),))
        g.op("a", act(scr("l1sq", G)[:], l1, AF.Square, scale=HS),
             reads=(key("zinL3", G),), writes=(skey("l1sq", G),))
        g.op("a", act(scr("l1sq14", G)[:], l1, AF.Square, scale=EPS7),
             reads=(key("zinL3", G),), writes=(skey("l1sq14", G),))

    def t12(G):
        g.op("v", lambda G=G: nc.vector.tensor_scalar(
            scr("t1h", G)[:], scr("e0sq", G)[:], EPS7, None, op0=A.add),
            reads=(skey("e0sq", G),), writes=(skey("t1h", G),))
        g.op("v", lambda G=G: nc.vector.tensor_scalar(
            scr("t2h", G)[:], scr("e2sq", G)[:], EPS7, None, op0=A.add),
            reads=(skey("e2sq", G),), writes=(skey("t2h", G),))

    def tt(out, in0, in1, op):
        return lambda: nc.vector.tensor_tensor(out, in0, in1, op)

    def alg_front(G):
        g.op("v", tt(scr("d1h", G)[:], scr("t1h", G)[:], scr("t2h", G)[:], A.mult),
             reads=(skey("t1h", G), skey("t2h", G)), writes=(skey("d1h", G),))
        g.op("v", tt(scr("s01h", G)[:], scr("e0h", G)[:], zin[G % 2][:, 3], A.mult),
             reads=(skey("e0h", G), key("zinL3", G)), writes=(skey("s01h", G),))
        g.op("v", tt(scr("s11h", G)[:], scr("t2h", G)[:], scr("l1sq", G)[:], A.add),
             reads=(skey("t2h", G), skey("l1sq", G)), writes=(skey("s11h", G),))
        g.op("v", tt(scr("deth", G)[:], scr("d1h", G)[:], scr("l1sq14", G)[:],
                     A.add),
             reads=(skey("d1h", G), skey("l1sq14", G)), writes=(skey("deth", G),))

    def lnd_r7(G):
        g.op("a", act(scr("lnd", G)[:], scr("deth", G)[:], AF.Ln),
             reads=(skey("deth", G),), writes=(skey("lnd", G),))
        g.op("a", act(scr("r7", G)[:], scr("lnd", G)[:], AF.Exp,
                      bias=-LN2x7, scale=-1.0),
             reads=(skey("lnd", G),), writes=(skey("r7", G),))

    def alg_mid(G):
        bi = G % 2
        z = zin[bi]
        z0 = z[:, 0]; z1 = z[:, 1]
        zk = key("zinZ", G)
        g.op("v", tt(scr("u0", G)[:], scr("s11h", G)[:], scr("r7", G)[:], A.mult),
             reads=(skey("s11h", G), skey("r7", G)), writes=(skey("u0", G),))
        g.op("v", tt(scr("u1", G)[:], scr("t1h", G)[:], scr("r7", G)[:], A.mult),
             reads=(skey("t1h", G), skey("r7", G)), writes=(skey("u1", G),))
        # m00/m11 on DVE (fp16 TS, 2x packed) right where they're produced
        g.op("v", lambda bi=bi, G=G: nc.vector.tensor_scalar(
            M0x[bi][:, :, 1:T + 1], scr("u0", G)[:], -1.0, 1.0,
            op0=A.mult, op1=A.add),
            reads=(skey("u0", G),), writes=(key("M0x", G),))
        g.op("v", lambda bi=bi, G=G: nc.vector.tensor_scalar(
            M1x[bi][:, :, 1:T + 1], scr("u1", G)[:], -1.0, 1.0,
            op0=A.mult, op1=A.add),
            reads=(skey("u1", G),), writes=(key("M1x", G),))
        g.op("v", tt(scr("m01", G)[:], scr("s01h", G)[:], scr("r7", G)[:], A.mult),
             reads=(skey("s01h", G), skey("r7", G)), writes=(skey("m01", G),))
        g.op("v", tt(scr("p0", G)[:], scr("u0", G)[:], z0, A.mult),
             reads=(skey("u0", G), zk), writes=(skey("p0", G),))
        g.op("v", tt(scr("q0", G)[:], scr("m01", G)[:], z1, A.mult),
             reads=(skey("m01", G), zk), writes=(skey("q0", G),))
        g.op("v", tt(B0x[bi][:, :, 1:T + 1], scr("p0", G)[:], scr("q0", G)[:],
                     A.subtract),
             reads=(skey("p0", G), skey("q0", G)), writes=(key("B0x", G),))
        g.op("v", tt(scr("p1", G)[:], scr("u1", G)[:], z1, A.mult),
             reads=(skey("u1", G), zk), writes=(skey("p1", G),))
        g.op("v", tt(scr("q1", G)[:], scr("m01", G)[:], z0, A.mult),
             reads=(skey("m01", G), zk), writes=(skey("q1", G),))
        g.op("v", tt(scr("b1", G)[:], scr("p1", G)[:], scr("q1", G)[:], A.subtract),
             reads=(skey("p1", G), skey("q1", G)), writes=(skey("b1", G),))

    def col_inits(G):
        # tiny [P,CH] init-column copies; on GpSimd (idle, negligible size)
        bi = G % 2
        m = mu[bi]
        mk = key("mu", G)
        for dst_ap, dst_key, mi in (
                (B0x[bi][:, :, 0], "B0x", 3), (CVx[bi][:, :, 0], "CVx", 3),
                (CWx[bi][:, :, 0], "CWx", 4), (Gx[bi][:, :, 0], "Gx", 2),
                (GXt[bi][:, :, 0], "GX", 0), (GYt[bi][:, :, 0], "GY", 1)):
            g.op("a", (lambda dst_ap=dst_ap, m=m, mi=mi:
                       nc.scalar.activation(dst_ap, m[:, mi], AF.Identity)),
                 reads=(mk,), writes=(key(dst_key, G),))

    def scan(out, d0, d1):
        return lambda: nc.vector.tensor_tensor_scan(out, d0, d1, 0.0,
                                                    A.mult, A.add)

    def sweeps_a(G):
        bi = G % 2
        m01 = scr("m01", G)
        tmp = scr("tmp", G)
        b1 = scr("b1", G)
        g.op("v", scan(flat(Vt[bi]), flat(M0x[bi]), flat(B0x[bi])),
             reads=(key("M0x", G), key("B0x", G)), writes=(key("V", G),))
        g.op("v", tt(tmp[:], m01[:], Vt[bi][:, :, 0:T], A.mult),
             reads=(skey("m01", G), key("V", G)), writes=(skey("tmp", G),))
        g.op("v", tt(CWx[bi][:, :, 1:T + 1], tmp[:], b1[:], A.add),
             reads=(skey("tmp", G), skey("b1", G)), writes=(key("CWx", G),))
        g.op("v", scan(flat(Wt[bi]), flat(M1x[bi]), flat(CWx[bi])),
             reads=(key("M1x", G), key("CWx", G)), writes=(key("W", G),))

    def sweeps_b(G):
        bi = G % 2
        m01 = scr("m01", G)
        tmp = scr("tmp", G)
        b1 = scr("b1", G)
        g.op("v", tt(tmp[:], m01[:], Wt[bi][:, :, 0:T], A.mult),
             reads=(skey("m01", G), key("W", G)), writes=(skey("tmp", G),))
        g.op("v", tt(CVx[bi][:, :, 1:T + 1], tmp[:], B0x[bi][:, :, 1:T + 1], A.add),
             reads=(skey("tmp", G), key("B0x", G)), writes=(key("CVx", G),))
        g.op("v", scan(flat(Vt[bi]), flat(M0x[bi]), flat(CVx[bi])),
             reads=(key("M0x", G), key("CVx", G)), writes=(key("V", G),))
        g.op("v", tt(tmp[:], m01[:], Vt[bi][:, :, 0:T], A.mult),
             reads=(skey("m01", G), key("V", G)), writes=(skey("tmp", G),))
        g.op("v", tt(CWx[bi][:, :, 1:T + 1], tmp[:], b1[:], A.add),
             reads=(skey("tmp", G), skey("b1", G)), writes=(key("CWx", G),))
        g.op("v", scan(flat(Wt[bi]), flat(M1x[bi]), flat(CWx[bi])),
             reads=(key("M1x", G), key("CWx", G)), writes=(key("W", G),))
        g.op("v", tt(Gx[bi][:, :, 1:T + 1], Wt[bi][:, :, 0:T], dtz[bi][:],
                     A.mult),
             reads=(key("W", G), key("dtz", G)), writes=(key("Gx", G),))
        g.op("v", scan(flat(TH[bi]), flat(ONB[:, 0:2]), flat(Gx[bi])),
             reads=("ONB", key("Gx", G)), writes=(key("TH", G),))

    def kchain(G):
        # k1 = th/2pi + MAGIC (round-to-int trick), k2 = (k1-MAGIC)*2pi
        # (exact: k1-MAGIC is Sterbenz-exact), thr = th - k2 in [-pi, pi].
        bi = G % 2
        g.op("v", lambda bi=bi: nc.vector.tensor_scalar(
            K1[bi][:], TH[bi][:, :, 0:T], INV_2PI, MAGIC, op0=A.mult, op1=A.add),
            reads=(key("TH", G),), writes=(key("K1", G),))
        g.op("v", lambda bi=bi: nc.vector.tensor_scalar(
            K1[bi][:], K1[bi][:], -MAGIC, TWO_PI, op0=A.add, op1=A.mult),
            reads=(key("K1", G),), writes=(key("K1", G),))
        g.op("v", tt(scr("thr", G)[:], TH[bi][:, :, 0:T], K1[bi][:], A.subtract),
             reads=(key("TH", G), key("K1", G)), writes=(skey("thr", G),))

    def trig(G):
        thr = scr("thr", G)
        g.op("a", act(scr("sinf", G)[:], thr[:], AF.Sin),
             reads=(skey("thr", G),), writes=(skey("sinf", G),))
        g.op("a", act(scr("cosf", G)[:], thr[:], AF.Sin, scale=0.5),
             reads=(skey("thr", G),), writes=(skey("cosf", G),))

    def tail(G):
        bi = G % 2
        vdt = scr("vdt", G)
        cosf = scr("cosf", G)
        g.op("v", tt(vdt[:], Vt[bi][:, :, 0:T], dtz[bi][:], A.mult),
             reads=(key("V", G), key("dtz", G)), writes=(skey("vdt", G),))
        g.op("v", tt(GYt[bi][:, :, 1:T + 1], vdt[:], scr("sinf", G)[:], A.mult),
             reads=(skey("vdt", G), skey("sinf", G)), writes=(key("GY", G),))
        # y-scan runs while Act finishes the cos half-angle pass
        g.op("v", scan(flat(YT[bi]), flat(ONB[:, 0:2]), flat(GYt[bi])),
             reads=("ONB", key("GY", G)), writes=(key("YT", G),))
        # cos(th) = 1 - 2*sin(th/2)^2, square+affine on DVE
        g.op("v", tt(cosf[:], cosf[:], cosf[:], A.mult),
             reads=(skey("cosf", G),), writes=(skey("cosf", G),))
        g.op("v", lambda G=G: nc.vector.tensor_scalar(
            cosf[:], cosf[:], -2.0, 1.0, op0=A.mult, op1=A.add),
            reads=(skey("cosf", G),), writes=(skey("cosf", G),))
        g.op("v", tt(GXt[bi][:, :, 1:T + 1], vdt[:], cosf[:], A.mult),
             reads=(skey("vdt", G), skey("cosf", G)), writes=(key("GX", G),))
        g.op("v", scan(flat(XT[bi]), flat(ONB[:, 0:2]), flat(GXt[bi])),
             reads=("ONB", key("GX", G)), writes=(key("XT", G),))

    def dma_out(G):
        s = G % NSL
        bi = G % 2
        g.op("s", lambda: nc.sync.dma_start(OXY[s, 1], YT[bi][:, :, 1:T + 1]),
             reads=(key("YT", G),), slot=s * 9 + 7)
        g.op("s", lambda: nc.sync.dma_start(OXY[s, 0], XT[bi][:, :, 1:T + 1]),
             reads=(key("XT", G),), slot=s * 9 + 6)
        g.op("s", lambda: nc.sync.dma_start(OTH[s], TH[bi][:, :, 1:T + 1]),
             reads=(key("TH", G),), slot=s * 9 + 8)

    # ---------------- emission ----------------
    # dma+leaf are hoisted one slab ahead so Act's leaf work never blocks
    # the next slab's algebra behind trig of an older slab.
    dma_in(0)
    dma_dt(0)
    leaf_acts_a(0)
    for G in range(NSL):
        if G == 0:
            t12(0)
            alg_front(0)
            lnd_r7(0)
        if G + 1 < NSL:
            dma_in(G + 1)
            leaf_acts_a(G + 1)
        if G > 0:
            sweeps_a(G - 1)
        alg_mid(G)
        col_inits(G)
        if G > 0:
            sweeps_b(G - 1)
            kchain(G - 1)
            trig(G - 1)
        if G + 1 < NSL:
            t12(G + 1)
            alg_front(G + 1)
            lnd_r7(G + 1)
        if G > 0:
            tail(G - 1)
            dma_out(G - 1)
        if G + 1 < NSL:
            dma_dt(G + 1)
    Gl = NSL - 1
    sweeps_a(Gl)
    sweeps_b(Gl)
    kchain(Gl)
    trig(Gl)
    tail(Gl)
    dma_out(Gl)

    n_slots = NSL * 9
    sem_v = nc.alloc_semaphore()
    sem_g = nc.alloc_semaphore()
    sem_a = nc.alloc_semaphore()
    dma_sems = [nc.alloc_semaphore(f"dsem{i}") for i in range(n_slots)]
    with nc.Block() as block:
        sems = {"v": sem_v, "g": sem_g, "a": sem_a}

        @block.sync
        def _(sync):
            last = {}
            dlast = {}
            for op_eng, emit_fn, deps, ref in sch.ops:
                if op_eng != "s":
                    continue
                for dep in deps:
                    if dep[0] == "D":
                        _, slot, k = dep
                        if dlast.get(slot, 0) >= k:
                            continue
                        sync.wait_ge(dma_sems[slot], 16 * k)
                        dlast[slot] = k
                    else:
                        deng, dpos = dep
                        if deng == "s" or last.get(deng, 0) >= dpos:
                            continue
                        sync.wait_ge(sems[deng], dpos)
                        last[deng] = dpos
                emit_fn().then_inc(dma_sems[ref[1]], 16)

        @block.vector
        def _(vector):
            sch.emit("v", vector, sems, dma_sems)

        @block.gpsimd
        def _(gp):
            sch.emit("g", gp, sems, dma_sems)

        @block.scalar
        def _(scalar):
            sch.emit("a", scalar, sems, dma_sems)

    return nc


_cache = {}


def _get_nc():
    if "nc" not in _cache:
        _cache["nc"] = _build_nc()
    return _cache["nc"]


def _pack_core(z_core, mu_core, times_core):
    zt = np.ascontiguousarray(z_core.transpose(2, 1, 0))       # (5, NPC, T)
    dt = np.empty_like(times_core)
    dt[0] = 0.0
    dt[1:] = times_core[1:] - times_core[:-1]
    IN = np.ascontiguousarray(
        zt.reshape(5, NSL, P, CH, T).transpose(1, 0, 2, 3, 4)).astype(np.float16)
    DTa = np.ascontiguousarray(dt.T.reshape(NSL, P, CH, T)).astype(np.float16)
    MU = np.ascontiguousarray(
        mu_core.reshape(NSL, P, CH, 5).transpose(0, 1, 3, 2))  # (NSL,P,5,CH)
    return {"inp": IN, "mu": MU, "dt": DTa}


def kernel(z_and_L_hat, mu0, times):
    z_and_L_hat = np.asarray(z_and_L_hat, dtype=np.float32)
    mu0 = np.asarray(mu0, dtype=np.float32)
    times = np.asarray(times, dtype=np.float32)
    nc = _get_nc()
    in_maps = []
    for k in range(N_CORES):
        sl = slice(k * NPC, (k + 1) * NPC)
        in_maps.append(_pack_core(z_and_L_hat[:, sl, :], mu0[sl], times[:, sl]))
    res = run_bass_kernel_spmd(nc, in_maps, core_ids=list(range(N_CORES)))
    out = np.empty((T, N_TOT, 3), np.float32)
    for k in range(N_CORES):
        oxy = res.results[k]["oxy"]               # (NSL, 2, P, CH, T) f16
        oth = res.results[k]["oth"]               # (NSL, P, CH, T) f32
        sl = slice(k * NPC, (k + 1) * NPC)
        out[:, sl, 0] = oxy[:, 0].astype(np.float32).reshape(NPC, T).T
        out[:, sl, 1] = oxy[:, 1].astype(np.float32).reshape(NPC, T).T
        out[:, sl, 2] = oth.astype(np.float32).reshape(NPC, T).T
    return out


# revision 13
# speedup vs baseline: 2.2627x; 1.0037x over previous
"""Trainium2 Bass kernel for nn_KalmanFilter: EKF over T=512 steps, N=8192 chains.

Mathematical reduction (verified exact vs the reference):
  With C = [[0,0,0,1,0],[0,0,0,0,1]], rows 3,4 of the Jacobian A are zero, so
  S = I + R depends only on per-step measurement params and the covariance
  never influences the output. Per chain:
    S = I + L L^T,  L = [[e^l0, 0], [l1, e^l2]]
    u_{t+1} = (I - S^-1) u_t + S^-1 z_t          (u = [v, omega])
    th_{t+1} = th_t + omega_t * dt_t
    x_{t+1}  = x_t + v_t * dt_t * cos(th_t)
    y_{t+1}  = y_t + v_t * dt_t * sin(th_t)
  The coupled 2-state linear recurrence is solved with 2 Gauss-Seidel sweeps
  of hardware affine scans (error contracts ~10x/sweep; end-to-end rel err
  ~3e-3 incl. fp16 quantization, vs the 2e-2 gate).

Implementation notes (from microbenchmarks on this part):
  - GpSimd shares SBUF ports with DVE and degrades it ~4x when running big
    ops: all full-size elementwise work runs on DVE (fp16 packed 2x mode,
    ~0.7us/1024el) + Act engine (function passes ~1.1us). GpSimd only does
    tiny per-slab init-column copies.
  - Scans are DVE-only, ~2.2ns/el, dtype-insensitive. Both chains per
    partition are covered by ONE scan via a zero-multiplier column at each
    chain start (which also injects the init value).
  - All intermediates fp16; t1, t2, s01, s11 carry a 2^-7 (det 2^-14)
    exponent scale folded into Act scale/bias so fp16 never overflows;
    r7 = 2^7/det compensates exactly.
  - x,y are produced and DMA'd as fp16 (host upcasts); th stays fp32 since
    range reduction needs it.
Sharding: data-parallel over chains, 1024 chains per core across 8 cores.
"""
import sys
sys.path.insert(0, '/opt/trn_rl_repo')
import numpy as np
import concourse.bass as bass
from concourse import mybir
from concourse.bass_utils import run_bass_kernel_spmd

F32 = mybir.dt.float32
F16 = mybir.dt.float16
AF = mybir.ActivationFunctionType
A = mybir.AluOpType

N_CORES = 8
T = 512
N_TOT = 8192
NPC = N_TOT // N_CORES          # 1024 chains per core
P = 128                         # partitions
NSL = 4                         # slabs per core
CH = NPC // (NSL * P)           # chains per partition per slab = 2
TP = T + 2                      # padded per-chain row: [init | T data | pad]
MAGIC = float(1.5 * 2 ** 23)
TWO_PI = float(2 * np.pi)
INV_2PI = float(1.0 / (2 * np.pi))
LN2x7 = float(7 * np.log(2.0))
HS = float(2.0 ** -3.5)         # Square scale for l1^2 * 2^-7
EPS7 = float(2.0 ** -7)


class _Sched:
    """Two-phase scheduler: record ops (engine, emit closure, deps), then emit
    per-engine in-order streams with standalone wait_ge for cross-engine deps."""

    def __init__(self):
        self.ops = []
        self.count = {"v": 0, "g": 0, "a": 0}
        self.slot_count = {}

    def add(self, eng, emit_fn, deps=(), slot=None):
        if eng == "s":
            self.slot_count[slot] = self.slot_count.get(slot, 0) + 1
            ref = ("D", slot, self.slot_count[slot])
        else:
            self.count[eng] += 1
            ref = (eng, self.count[eng])
        self.ops.append((eng, emit_fn, tuple(d for d in deps if d), ref))
        return ref

    def emit(self, eng, raw_eng, sems, dma_sems):
        last = {}
        dlast = {}
        for op_eng, emit_fn, deps, ref in self.ops:
            if op_eng != eng:
                continue
            for dep in deps:
                if dep[0] == "D":
                    _, slot, k = dep
                    if dlast.get(slot, 0) >= k:
                        continue
                    raw_eng.wait_ge(dma_sems[slot], 16 * k)
                    dlast[slot] = k
                else:
                    deng, dpos = dep
                    if deng == eng or last.get(deng, 0) >= dpos:
                        continue
                    raw_eng.wait_ge(sems[deng], dpos)
                    last[deng] = dpos
            emit_fn().then_inc(sems[eng], 1)


class _Graph:
    """Auto RAW/WAR/WAW dependency tracking over named tiles."""

    def __init__(self, sch):
        self.sch = sch
        self.w = {}
        self.r = {}

    def op(self, eng, emit_fn, reads=(), writes=(), slot=None, extra=()):
        deps = {}

        def add(ref):
            if ref is None:
                return
            key = ref[0] if ref[0] != "D" else ("D", ref[1])
            cur = deps.get(key)
            if cur is None or ref[-1] > cur[-1]:
                deps[key] = ref

        for t in reads:
            add(self.w.get(t))
        for t in writes:
            add(self.w.get(t))
            for rr in self.r.get(t, ()):
                add(rr)
        for e in extra:
            add(e)
        ref = self.sch.add(eng, emit_fn, deps=tuple(deps.values()), slot=slot)
        for t in reads:
            self.r.setdefault(t, []).append(ref)
        for t in writes:
            self.w[t] = ref
            self.r[t] = []
        return ref


def _build_nc():
    nc = bass.Bass()
    # Register activation bias constants (bass converts float biases of
    # non-Copy activations to const APs, which must pre-exist).
    for val in (-LN2x7, EPS7):
        t = nc.alloc_sbuf_tensor(f"constf32-{val}", [128, 1], F32)
        nc.gpsimd.memset(t.ap(), val)
        nc.const_aps.aps[(F32, val)] = t.ap()
    nc.all_engine_barrier()
    IN = nc.dram_tensor("inp", [NSL, 5, P, CH, T], F16, kind="ExternalInput")
    DT = nc.dram_tensor("dt", [NSL, P, CH, T], F16, kind="ExternalInput")
    MU = nc.dram_tensor("mu", [NSL, P, 5, CH], F32, kind="ExternalInput")
    OXY = nc.dram_tensor("oxy", [NSL, 2, P, CH, T], F16, kind="ExternalOutput")
    OTH = nc.dram_tensor("oth", [NSL, P, CH, T], F32, kind="ExternalOutput")

    _names = [0]

    def tile(shape, dt=F16):
        _names[0] += 1
        return nc.alloc_sbuf_tensor(f"tl{_names[0]}", list(shape), dt).ap()

    def flat(ap):
        return ap.rearrange('p a b -> p (a b)')

    # constants: fp16 ones with 0 at col0 of each chain row
    ONB = tile([P, 4, TP])

    # per-parity tiles
    zin = [tile([P, 5, CH, T]) for _ in range(2)]
    dtz = [tile([P, CH, T]) for _ in range(2)]
    mu = [tile([P, 5, CH], F32) for _ in range(2)]
    M0x = [tile([P, CH, TP]) for _ in range(2)]
    M1x = [tile([P, CH, TP]) for _ in range(2)]
    B0x = [tile([P, CH, TP]) for _ in range(2)]
    CWx = [tile([P, CH, TP]) for _ in range(2)]
    CVx = [tile([P, CH, TP]) for _ in range(2)]
    Gx = [tile([P, CH, TP]) for _ in range(2)]
    GXt = [tile([P, CH, TP]) for _ in range(2)]
    GYt = [tile([P, CH, TP]) for _ in range(2)]
    Vt = [tile([P, CH, TP]) for _ in range(2)]
    Wt = [tile([P, CH, TP]) for _ in range(2)]
    TH = [tile([P, CH, TP], F32) for _ in range(2)]
    XT = [tile([P, CH, TP]) for _ in range(2)]
    YT = [tile([P, CH, TP]) for _ in range(2)]
    K1 = [tile([P, CH, T], F32) for _ in range(2)]
    SCR = [[tile([P, CH, T]) for _ in range(17)] for _ in range(2)]

    sch = _Sched()
    g = _Graph(sch)

    # ---- preamble ----
    g.op("v", lambda: nc.vector.memset(flat(ONB), 1.0), writes=("ONB",))
    for c in range(4):
        g.op("v", lambda c=c: nc.vector.memset(ONB[:, c, 0:1], 0.0),
             writes=("ONB",))
    for bi in range(2):
        for nm, tl in (("M0x", M0x), ("M1x", M1x), ("B0x", B0x), ("CWx", CWx),
                       ("CVx", CVx), ("Gx", Gx)):
            g.op("v", lambda tl=tl, bi=bi: nc.vector.memset(
                tl[bi][:, :, TP - 1:TP], 0.0), writes=(f"{nm}{bi}",))
        g.op("v", lambda bi=bi: nc.vector.memset(GXt[bi][:, :, TP - 1:TP], 0.0),
             writes=(f"GX{bi}",))
        g.op("v", lambda bi=bi: nc.vector.memset(GYt[bi][:, :, TP - 1:TP], 0.0),
             writes=(f"GY{bi}",))
        for nm, tl in (("M0x", M0x), ("M1x", M1x)):
            g.op("v", lambda tl=tl, bi=bi: nc.vector.memset(
                tl[bi][:, :, 0:1], 0.0), writes=(f"{nm}{bi}",))

    def key(nm, G):
        return f"{nm}{G % 2}"

    # scratch phys allocation: names sharing an index alias the same tile
    PHYS = {"e0sq": 0, "u0": 0,
            "e2sq": 1, "u1": 1,
            "e0h": 2,
            "l1sq": 3, "p0": 3,
            "t1h": 4, "p1": 4,
            "t2h": 5, "q0": 5,
            "d1h": 6, "lnd": 6, "b1": 6,
            "deth": 7, "q1": 7,
            "s01h": 8, "s11h": 9,
            "sinf": 15, "cosf": 16,
            "r7": 10, "tmp": 10,
            "vdt": 11, "thr": 12, "m01": 13, "l1sq14": 14}

    def scr(nm, G):
        return SCR[G % 2][PHYS[nm]]

    def skey(nm, G):
        return f"S{PHYS[nm]}_{G % 2}"

    # ---------------- stages ----------------
    def dma_in(G):
        s = G % NSL
        bi = G % 2
        for pl in (2, 3, 4):
            g.op("s", lambda pl=pl: nc.sync.dma_start(
                zin[bi][:, pl], IN[s, pl]),
                writes=(key(f"zinL{pl}", G),), slot=s * 9 + (pl - 2))
        g.op("s", lambda: nc.sync.dma_start(
            zin[bi][:, 0:2], IN[s, 0:2].rearrange("k p c t -> p k c t")),
            writes=(key("zinZ", G),), slot=s * 9 + 3)
        g.op("s", lambda: nc.sync.dma_start(mu[bi][:], MU[s]),
            writes=(key("mu", G),), slot=s * 9 + 4)

    def dma_dt(G):
        s = G % NSL
        bi = G % 2
        g.op("s", lambda: nc.sync.dma_start(dtz[bi][:], DT[s]),
            writes=(key("dtz", G),), slot=s * 9 + 5)

    def act(out_ap, in_ap, func, bias=0.0, scale=1.0):
        return lambda: nc.scalar.activation(out_ap, in_ap, func,
                                            bias=bias, scale=scale)

    def leaf_acts_a(G):
        bi = G % 2
        z = zin[bi]
        l0 = z[:, 2]; l1 = z[:, 3]; l2 = z[:, 4]
        g.op("a", act(scr("e0sq", G)[:], l0, AF.Exp, bias=-LN2x7, scale=2.0),
             reads=(key("zinL2", G),), writes=(skey("e0sq", G),))
        g.op("a", act(scr("e2sq", G)[:], l2, AF.Exp, bias=-LN2x7, scale=2.0),
             reads=(key("zinL4", G),), writes=(skey("e2sq", G),))
        g.op("a", act(scr("e0h", G)[:], l0, AF.Exp, bias=-LN2x7),
             reads=(key("zinL2", G),), writes=(skey("e0h", G),))
        g.op("a", act(scr("l1sq", G)[:], l1, AF.Square, scale=HS),
             reads=(key("zinL3", G),), writes=(skey("l1sq", G),))
        g.op("a", act(scr("l1sq14", G)[:], l1, AF.Square, scale=EPS7),
             reads=(key("zinL3", G),), writes=(skey("l1sq14", G),))

    def t12(G):
        g.op("v", lambda G=G: nc.vector.tensor_scalar(
            scr("t1h", G)[:], scr("e0sq", G)[:], EPS7, None, op0=A.add),
            reads=(skey("e0sq", G),), writes=(skey("t1h", G),))
        g.op("v", lambda G=G: nc.vector.tensor_scalar(
            scr("t2h", G)[:], scr("e2sq", G)[:], EPS7, None, op0=A.add),
            reads=(skey("e2sq", G),), writes=(skey("t2h", G),))

    def tt(out, in0, in1, op):
        return lambda: nc.vector.tensor_tensor(out, in0, in1, op)

    def alg_front(G):
        g.op("v", tt(scr("d1h", G)[:], scr("t1h", G)[:], scr("t2h", G)[:], A.mult),
             reads=(skey("t1h", G), skey("t2h", G)), writes=(skey("d1h", G),))
        g.op("v", tt(scr("s01h", G)[:], scr("e0h", G)[:], zin[G % 2][:, 3], A.mult),
             reads=(skey("e0h", G), key("zinL3", G)), writes=(skey("s01h", G),))
        g.op("v", tt(scr("s11h", G)[:], scr("t2h", G)[:], scr("l1sq", G)[:], A.add),
             reads=(skey("t2h", G), skey("l1sq", G)), writes=(skey("s11h", G),))
        g.op("v", tt(scr("deth", G)[:], scr("d1h", G)[:], scr("l1sq14", G)[:],
                     A.add),
             reads=(skey("d1h", G), skey("l1sq14", G)), writes=(skey("deth", G),))

    def lnd_r7(G):
        g.op("a", act(scr("lnd", G)[:], scr("deth", G)[:], AF.Ln),
             reads=(skey("deth", G),), writes=(skey("lnd", G),))
        g.op("a", act(scr("r7", G)[:], scr("lnd", G)[:], AF.Exp,
                      bias=-LN2x7, scale=-1.0),
             reads=(skey("lnd", G),), writes=(skey("r7", G),))

    def alg_mid(G):
        bi = G % 2
        z = zin[bi]
        z0 = z[:, 0]; z1 = z[:, 1]
        zk = key("zinZ", G)
        g.op("v", tt(scr("u0", G)[:], scr("s11h", G)[:], scr("r7", G)[:], A.mult),
             reads=(skey("s11h", G), skey("r7", G)), writes=(skey("u0", G),))
        g.op("v", tt(scr("u1", G)[:], scr("t1h", G)[:], scr("r7", G)[:], A.mult),
             reads=(skey("t1h", G), skey("r7", G)), writes=(skey("u1", G),))
        # m00/m11 on DVE (fp16 TS, 2x packed) right where they're produced
        g.op("v", lambda bi=bi, G=G: nc.vector.tensor_scalar(
            M0x[bi][:, :, 1:T + 1], scr("u0", G)[:], -1.0, 1.0,
            op0=A.mult, op1=A.add),
            reads=(skey("u0", G),), writes=(key("M0x", G),))
        g.op("v", lambda bi=bi, G=G: nc.vector.tensor_scalar(
            M1x[bi][:, :, 1:T + 1], scr("u1", G)[:], -1.0, 1.0,
            op0=A.mult, op1=A.add),
            reads=(skey("u1", G),), writes=(key("M1x", G),))
        g.op("v", tt(scr("m01", G)[:], scr("s01h", G)[:], scr("r7", G)[:], A.mult),
             reads=(skey("s01h", G), skey("r7", G)), writes=(skey("m01", G),))
        g.op("v", tt(scr("p0", G)[:], scr("u0", G)[:], z0, A.mult),
             reads=(skey("u0", G), zk), writes=(skey("p0", G),))
        g.op("v", tt(scr("q0", G)[:], scr("m01", G)[:], z1, A.mult),
             reads=(skey("m01", G), zk), writes=(skey("q0", G),))
        g.op("v", tt(B0x[bi][:, :, 1:T + 1], scr("p0", G)[:], scr("q0", G)[:],
                     A.subtract),
             reads=(skey("p0", G), skey("q0", G)), writes=(key("B0x", G),))
        g.op("v", tt(scr("p1", G)[:], scr("u1", G)[:], z1, A.mult),
             reads=(skey("u1", G), zk), writes=(skey("p1", G),))
        g.op("v", tt(scr("q1", G)[:], scr("m01", G)[:], z0, A.mult),
             reads=(skey("m01", G), zk), writes=(skey("q1", G),))
        g.op("v", tt(scr("b1", G)[:], scr("p1", G)[:], scr("q1", G)[:], A.subtract),
             reads=(skey("p1", G), skey("q1", G)), writes=(skey("b1", G),))

    def col_inits(G):
        # tiny [P,CH] init-column copies; on GpSimd (idle, negligible size)
        bi = G % 2
        m = mu[bi]
        mk = key("mu", G)
        for dst_ap, dst_key, mi in (
                (B0x[bi][:, :, 0], "B0x", 3), (CVx[bi][:, :, 0], "CVx", 3),
                (CWx[bi][:, :, 0], "CWx", 4), (Gx[bi][:, :, 0], "Gx", 2),
                (GXt[bi][:, :, 0], "GX", 0), (GYt[bi][:, :, 0], "GY", 1)):
            g.op("a", (lambda dst_ap=dst_ap, m=m, mi=mi:
                       nc.scalar.activation(dst_ap, m[:, mi], AF.Identity)),
                 reads=(mk,), writes=(key(dst_key, G),))

    def scan(out, d0, d1):
        return lambda: nc.vector.tensor_tensor_scan(out, d0, d1, 0.0,
                                                    A.mult, A.add)

    def sweeps_a(G):
        bi = G % 2
        m01 = scr("m01", G)
        tmp = scr("tmp", G)
        b1 = scr("b1", G)
        g.op("v", scan(flat(Vt[bi]), flat(M0x[bi]), flat(B0x[bi])),
             reads=(key("M0x", G), key("B0x", G)), writes=(key("V", G),))
        g.op("v", tt(tmp[:], m01[:], Vt[bi][:, :, 0:T], A.mult),
             reads=(skey("m01", G), key("V", G)), writes=(skey("tmp", G),))
        g.op("v", tt(CWx[bi][:, :, 1:T + 1], tmp[:], b1[:], A.add),
             reads=(skey("tmp", G), skey("b1", G)), writes=(key("CWx", G),))
        g.op("v", scan(flat(Wt[bi]), flat(M1x[bi]), flat(CWx[bi])),
             reads=(key("M1x", G), key("CWx", G)), writes=(key("W", G),))

    def sweeps_b(G):
        bi = G % 2
        m01 = scr("m01", G)
        tmp = scr("tmp", G)
        b1 = scr("b1", G)
        g.op("v", tt(tmp[:], m01[:], Wt[bi][:, :, 0:T], A.mult),
             reads=(skey("m01", G), key("W", G)), writes=(skey("tmp", G),))
        g.op("v", tt(CVx[bi][:, :, 1:T + 1], tmp[:], B0x[bi][:, :, 1:T + 1], A.add),
             reads=(skey("tmp", G), key("B0x", G)), writes=(key("CVx", G),))
        g.op("v", scan(flat(Vt[bi]), flat(M0x[bi]), flat(CVx[bi])),
             reads=(key("M0x", G), key("CVx", G)), writes=(key("V", G),))
        g.op("v", tt(tmp[:], m01[:], Vt[bi][:, :, 0:T], A.mult),
             reads=(skey("m01", G), key("V", G)), writes=(skey("tmp", G),))
        g.op("v", tt(CWx[bi][:, :, 1:T + 1], tmp[:], b1[:], A.add),
             reads=(skey("tmp", G), skey("b1", G)), writes=(key("CWx", G),))
        g.op("v", scan(flat(Wt[bi]), flat(M1x[bi]), flat(CWx[bi])),
             reads=(key("M1x", G), key("CWx", G)), writes=(key("W", G),))
        g.op("v", tt(Gx[bi][:, :, 1:T + 1], Wt[bi][:, :, 0:T], dtz[bi][:],
                     A.mult),
             reads=(key("W", G), key("dtz", G)), writes=(key("Gx", G),))
        g.op("v", scan(flat(TH[bi]), flat(ONB[:, 0:2]), flat(Gx[bi])),
             reads=("ONB", key("Gx", G)), writes=(key("TH", G),))

    def kchain(G):
        # k1 = th/2pi + MAGIC (round-to-int trick), k2 = (k1-MAGIC)*2pi
        # (exact: k1-MAGIC is Sterbenz-exact), thr = th - k2 in [-pi, pi].
        bi = G % 2
        g.op("v", lambda bi=bi: nc.vector.tensor_scalar(
            K1[bi][:], TH[bi][:, :, 0:T], INV_2PI, MAGIC, op0=A.mult, op1=A.add),
            reads=(key("TH", G),), writes=(key("K1", G),))
        g.op("v", lambda bi=bi: nc.vector.tensor_scalar(
            K1[bi][:], K1[bi][:], -MAGIC, TWO_PI, op0=A.add, op1=A.mult),
            reads=(key("K1", G),), writes=(key("K1", G),))
        g.op("v", tt(scr("thr", G)[:], TH[bi][:, :, 0:T], K1[bi][:], A.subtract),
             reads=(key("TH", G), key("K1", G)), writes=(skey("thr", G),))

    def trig(G):
        thr = scr("thr", G)
        g.op("a", act(scr("sinf", G)[:], thr[:], AF.Sin),
             reads=(skey("thr", G),), writes=(skey("sinf", G),))
        g.op("a", act(scr("cosf", G)[:], thr[:], AF.Sin, scale=0.5),
             reads=(skey("thr", G),), writes=(skey("cosf", G),))

    def tail(G):
        bi = G % 2
        vdt = scr("vdt", G)
        cosf = scr("cosf", G)
        g.op("v", tt(vdt[:], Vt[bi][:, :, 0:T], dtz[bi][:], A.mult),
             reads=(key("V", G), key("dtz", G)), writes=(skey("vdt", G),))
        g.op("v", tt(GYt[bi][:, :, 1:T + 1], vdt[:], scr("sinf", G)[:], A.mult),
             reads=(skey("vdt", G), skey("sinf", G)), writes=(key("GY", G),))
        # y-scan runs while Act finishes the cos half-angle pass
        g.op("v", scan(flat(YT[bi]), flat(ONB[:, 0:2]), flat(GYt[bi])),
             reads=("ONB", key("GY", G)), writes=(key("YT", G),))
        # cos(th) = 1 - 2*sin(th/2)^2, square+affine on DVE
        g.op("v", tt(cosf[:], cosf[:], cosf[:], A.mult),
             reads=(skey("cosf", G),), writes=(skey("cosf", G),))
        g.op("v", lambda G=G: nc.vector.tensor_scalar(
            cosf[:], cosf[:], -2.0, 1.0, op0=A.mult, op1=A.add),
            reads=(skey("cosf", G),), writes=(skey("cosf", G),))
        g.op("v", tt(GXt[bi][:, :, 1:T + 1], vdt[:], cosf[:], A.mult),
             reads=(skey("vdt", G), skey("cosf", G)), writes=(key("GX", G),))
        g.op("v", scan(flat(XT[bi]), flat(ONB[:, 0:2]), flat(GXt[bi])),
             reads=("ONB", key("GX", G)), writes=(key("XT", G),))

    def dma_out(G):
        s = G % NSL
        bi = G % 2
        g.op("s", lambda: nc.sync.dma_start(OTH[s], TH[bi][:, :, 1:T + 1]),
             reads=(key("TH", G),), slot=s * 9 + 8)
        g.op("s", lambda: nc.sync.dma_start(OXY[s, 1], YT[bi][:, :, 1:T + 1]),
             reads=(key("YT", G),), slot=s * 9 + 7)
        g.op("s", lambda: nc.sync.dma_start(OXY[s, 0], XT[bi][:, :, 1:T + 1]),
             reads=(key("XT", G),), slot=s * 9 + 6)

    # ---------------- emission ----------------
    # dma+leaf are hoisted one slab ahead so Act's leaf work never blocks
    # the next slab's algebra behind trig of an older slab.
    dma_in(0)
    dma_dt(0)
    leaf_acts_a(0)
    for G in range(NSL):
        if G == 0:
            t12(0)
            alg_front(0)
            lnd_r7(0)
        if G + 1 < NSL:
            dma_in(G + 1)
            leaf_acts_a(G + 1)
        if G > 0:
            sweeps_a(G - 1)
        alg_mid(G)
        col_inits(G)
        if G > 0:
            sweeps_b(G - 1)
            kchain(G - 1)
            trig(G - 1)
        if G + 1 < NSL:
            t12(G + 1)
            alg_front(G + 1)
            lnd_r7(G + 1)
        if G > 0:
            tail(G - 1)
            dma_out(G - 1)
        if G + 1 < NSL:
            dma_dt(G + 1)
    Gl = NSL - 1
    sweeps_a(Gl)
    sweeps_b(Gl)
    kchain(Gl)
    trig(Gl)
    tail(Gl)
    dma_out(Gl)

    n_slots = NSL * 9
    sem_v = nc.alloc_semaphore()
    sem_g = nc.alloc_semaphore()
    sem_a = nc.alloc_semaphore()
    dma_sems = [nc.alloc_semaphore(f"dsem{i}") for i in range(n_slots)]
    with nc.Block() as block:
        sems = {"v": sem_v, "g": sem_g, "a": sem_a}

        @block.sync
        def _(sync):
            last = {}
            dlast = {}
            for op_eng, emit_fn, deps, ref in sch.ops:
                if op_eng != "s":
                    continue
                for dep in deps:
                    if dep[0] == "D":
                        _, slot, k = dep
                        if dlast.get(slot, 0) >= k:
                            continue
                        sync.wait_ge(dma_sems[slot], 16 * k)
                        dlast[slot] = k
                    else:
                        deng, dpos = dep
                        if deng == "s" or last.get(deng, 0) >= dpos:
                            continue
                        sync.wait_ge(sems[deng], dpos)
                        last[deng] = dpos
                emit_fn().then_inc(dma_sems[ref[1]], 16)

        @block.vector
        def _(vector):
            sch.emit("v", vector, sems, dma_sems)

        @block.gpsimd
        def _(gp):
            sch.emit("g", gp, sems, dma_sems)

        @block.scalar
        def _(scalar):
            sch.emit("a", scalar, sems, dma_sems)

    return nc


_cache = {}


def _get_nc():
    if "nc" not in _cache:
        _cache["nc"] = _build_nc()
    return _cache["nc"]


def _pack_core(z_core, mu_core, times_core):
    zt = np.ascontiguousarray(z_core.transpose(2, 1, 0))       # (5, NPC, T)
    dt = np.empty_like(times_core)
    dt[0] = 0.0
    dt[1:] = times_core[1:] - times_core[:-1]
    IN = np.ascontiguousarray(
        zt.reshape(5, NSL, P, CH, T).transpose(1, 0, 2, 3, 4)).astype(np.float16)
    DTa = np.ascontiguousarray(dt.T.reshape(NSL, P, CH, T)).astype(np.float16)
    MU = np.ascontiguousarray(
        mu_core.reshape(NSL, P, CH, 5).transpose(0, 1, 3, 2))  # (NSL,P,5,CH)
    return {"inp": IN, "mu": MU, "dt": DTa}


def kernel(z_and_L_hat, mu0, times):
    z_and_L_hat = np.asarray(z_and_L_hat, dtype=np.float32)
    mu0 = np.asarray(mu0, dtype=np.float32)
    times = np.asarray(times, dtype=np.float32)
    nc = _get_nc()
    in_maps = []
    for k in range(N_CORES):
        sl = slice(k * NPC, (k + 1) * NPC)
        in_maps.append(_pack_core(z_and_L_hat[:, sl, :], mu0[sl], times[:, sl]))
    res = run_bass_kernel_spmd(nc, in_maps, core_ids=list(range(N_CORES)))
    out = np.empty((T, N_TOT, 3), np.float32)
    for k in range(N_CORES):
        oxy = res.results[k]["oxy"]               # (NSL, 2, P, CH, T) f16
        oth = res.results[k]["oth"]               # (NSL, P, CH, T) f32
        sl = slice(k * NPC, (k + 1) * NPC)
        out[:, sl, 0] = oxy[:, 0].astype(np.float32).reshape(NPC, T).T
        out[:, sl, 1] = oxy[:, 1].astype(np.float32).reshape(NPC, T).T
        out[:, sl, 2] = oth.astype(np.float32).reshape(NPC, T).T
    return out
